# revision 17
# baseline (speedup 1.0000x reference)
"""Trainium2 Bass kernel for nn_CMCI_Mamba.

Strategy: data-parallel over the 2B=8 mamba streams (1 sequence per core).
Each launch runs 2 chained mamba layers fully on-chip in d-major layout
(features on partitions, time on the free axis).

Engine assignment (per layer):
- PE (fp16): in_proj with the causal conv FOLDED IN (4 shifted matmuls with
  host-prescaled weights diag(conv_w_k) @ in_w), z-proj, fused
  dt_w@xp_w[dt] projection, 32 stride-0 B/C broadcast matmuls, out_proj.
- Act: Silu(conv) / Silu(z) straight from PSUM, softplus via Exp+Ln (one
  table set), the 16 per-state dA = exp(A_s * delta) passes, PSUM->SBUF
  fp16 copies of the B/C broadcasts, layer-output copies.
- DVE: the 16 SSM scans (tensor_tensor_scan, batched 2 states per
  instruction with a zeroed dA column resetting the carry), dBu muls,
  half of the hs*C muls/accumulation.
- GPSIMD: the other half of the hs*C muls and y accumulation.

Host does the cheap cross-stream elementwise combines between launches.
"""
import sys
import numpy as np
from contextlib import ExitStack

for _p in ("/opt/trn_rl_repo",):
    if _p not in sys.path:
        sys.path.insert(0, _p)

import concourse.bass as bass
import concourse.bacc as bacc
import concourse.tile as tile
from concourse import mybir
from concourse import bass_utils

T, DM, DI, DS, DR, K, NL = 2048, 64, 128, 16, 4, 4, 2
B, C = 4, 2048
UF = T + K  # padded u width (2052)
FP = mybir.dt.float32
FH = mybir.dt.float16
AX = mybir.AluOpType
AF = mybir.ActivationFunctionType

# fp16 param blob column layout, (128, 1024) per layer
_B_WK = 0       # [0:64, 0:512]    4x conv-scaled in_proj-x lhsT (64,128) each
_B_Z = 512      # [0:64, 512:640]  z lhsT
_B_WD = 640     # [:, 640:768]     (dt_w @ xp_w[:DR]) lhsT
_B_BC = 768     # [:, 768:800]     B/C projection columns (32)
_B_OUT = 800    # [:, 800:864]     out_proj lhsT
_B_OUTD = 864   # [:, 864:928]     out_proj lhsT with D folded (for x*sz term)
_HBLOB_W = 1024
# fp32 blob (128, 20): [:, 0:16]=A (=-exp(A_log)), 16=conv_b, 17=dt_b, 18=D


def _pack_blobs(raw, l):
    hb = np.zeros((DI, _HBLOB_W), np.float16)
    in_w = raw["in_w"][l]          # (256, 64)
    conv_w = raw["conv_w"][l]      # (128, 4)
    for k in range(K):
        wk = in_w[:DI] * conv_w[:, k:k + 1]          # (128, 64)
        hb[:DM, _B_WK + 128 * k:_B_WK + 128 * (k + 1)] = wk.T
    hb[:DM, _B_Z:_B_Z + DI] = in_w[DI:2 * DI].T
    wd = raw["dt_w"][l] @ raw["xp_w"][l][:DR]        # (128, 128)
    hb[:, _B_WD:_B_WD + DI] = wd.T
    hb[:, _B_BC:_B_BC + 2 * DS] = raw["xp_w"][l][DR:DR + 2 * DS].T
    hb[:, _B_OUT:_B_OUT + DM] = raw["out_w"][l].T
    # out_proj with D folded in: out += (out_w * D) @ (x * silu(z))
    hb[:, _B_OUTD:_B_OUTD + DM] = (raw["out_w"][l] * raw["D"][l]).T
    fb = np.zeros((DI, 20), np.float32)
    fb[:, 0:DS] = -np.exp(raw["A_log"][l])
    fb[:, 16] = raw["conv_b"][l]
    fb[:, 17] = raw["dt_b"][l]
    fb[:, 18] = raw["D"][l]
    return hb, fb


def _build_layer(nc, pools, hb, fb, up, upo, out_specs, out_dma):
    """One mamba layer. up/upo: (64, UF) fp16 padded input (+1-shifted copy).
    out_specs: list of (tile, col_off) -- the (64, T) layer output is copied
    (in halves, on Act) into tile[:, off:off+T]. out_dma: DRAM ap or None.
    """
    const, big, sl, ps, gl = pools
    NCH = T // 512
    H = T // 2
    lid = gl["lid"]

    wkT = [hb[0:DM, _B_WK + 128 * k:_B_WK + 128 * (k + 1)] for k in range(K)]
    zT = hb[0:DM, _B_Z:_B_Z + DI]
    wdT = hb[:, _B_WD:_B_WD + DI]
    outT = hb[:, _B_OUT:_B_OUT + DM]
    outDT = hb[:, _B_OUTD:_B_OUTD + DM]
    Acols = fb[:, 0:DS]
    convb = fb[:, 16:17]
    dtb = fb[:, 17:18]

    def bc_mm(tag, col, name):
        """Stride-0 broadcast matmul of projection column `col` -> psum."""
        t = ps.tile([DI, T], FP, tag="bc", name=name)
        w = hb[:, _B_BC + col:_B_BC + col + 1].broadcast_to((DI, DI))
        for c in range(NCH):
            nc.tensor.matmul(t[:, c * 512:(c + 1) * 512], w,
                             xact[:, c * 512:(c + 1) * 512],
                             start=True, stop=True)
        return t

    # ---- in_proj-x with folded causal conv -> silu -> xact (fp16) ----
    # xc[:, t] = sum_k (diag(conv_w_k) @ in_w_x) @ u[:, t-3+k]; tap k reads
    # u_pad[:, c*512+k:]; odd k uses the 1-shifted copy so every rhs offset
    # stays 4B-aligned.  Silu is applied per half so the delta chain starts
    # as soon as the first half lands.
    xact = big.tile([DI, T], FH, tag="xact", name=f"xact{lid}")
    for h in range(2):
        mmx = ps.tile([DI, H], FP, tag="bc", name=f"mmx{lid}_{h}")
        for c in (2 * h, 2 * h + 1):
            o = c * 512
            cs = slice(o - h * H, o - h * H + 512)
            nc.tensor.matmul(mmx[:, cs], wkT[0], up[:, o:o + 512],
                             start=True, stop=False)
            nc.tensor.matmul(mmx[:, cs], wkT[1], upo[:, o:o + 512],
                             start=False, stop=False)
            nc.tensor.matmul(mmx[:, cs], wkT[2], up[:, o + 2:o + 514],
                             start=False, stop=False)
            nc.tensor.matmul(mmx[:, cs], wkT[3], upo[:, o + 2:o + 514],
                             start=False, stop=True)
        nc.scalar.activation(xact[:, h * H:(h + 1) * H], mmx[:], AF.Silu,
                             bias=convb)

    # ---- delta = softplus(dt_proj + dt_b) via Exp then Ln(1+x), halves ----
    delta = big.tile([DI, T], FH, tag="delta", name=f"delta{lid}")
    ev = big.tile([DI, T], FH, tag="ev", name=f"ev{lid}")
    dx = big.tile([DI, T], FH, tag="dx", name=f"dx{lid}")
    for h in range(2):
        mmd = ps.tile([DI, H], FP, tag="bc", name=f"mmd{lid}_{h}")
        for c in (2 * h, 2 * h + 1):
            o = c * 512
            nc.tensor.matmul(mmd[:, o - h * H:o - h * H + 512], wdT,
                             xact[:, o:o + 512], start=True, stop=True)
        nc.scalar.activation(ev[:, h * H:(h + 1) * H], mmd[:], AF.Exp,
                             bias=dtb)
    for h in range(2):
        hs_ = slice(h * H, (h + 1) * H)
        nc.scalar.activation(delta[:, hs_], ev[:, hs_], AF.Ln, bias=1.0)
        nc.vector.tensor_mul(dx[:, hs_], delta[:, hs_], xact[:, hs_])

    # ---- s-loop: single s=0 (PSUM-direct, shortest ramp), 7 pairs, s=15 ----
    ysn = big.tile([DI, T], FH, tag="ysn", name=f"ysn{lid}")
    yP = big.tile([DI, 2 * T], FH, tag="yP", name=f"yP{lid}")

    # s = 0
    dA0 = big.tile([DI, T], FH, tag="dAs", name=f"dA{lid}_s0")
    for h in range(2):
        hs_ = slice(h * H, (h + 1) * H)
        nc.scalar.activation(dA0[:, hs_], delta[:, hs_], AF.Exp,
                             scale=Acols[:, 0:1])
    bps0 = bc_mm("bc", 0, f"bps{lid}_0")
    dBu0 = big.tile([DI, T], FH, tag="dBus", name=f"dBu{lid}_s0")
    nc.vector.tensor_mul(dBu0[:], dx[:], bps0[:])
    hs0 = big.tile([DI, T], FH, tag="hss", name=f"hs{lid}_s0")
    nc.vector.tensor_tensor_scan(hs0[:], dA0[:], dBu0[:], 0.0, AX.mult, AX.add)
    cps0 = bc_mm("bc", DS + 0, f"cps{lid}_0")
    nc.vector.tensor_mul(ysn[:], hs0[:], cps0[:])

    # pairs (1,2) .. (13,14)
    for p in range(1, 8):
        s0, s1 = 2 * p - 1, 2 * p
        dA = sl.tile([DI, 2 * T], FH, tag="dA", name=f"dA{lid}_{p}")
        nc.scalar.activation(dA[:, 0:T], delta[:], AF.Exp,
                             scale=Acols[:, s0:s0 + 1])
        nc.scalar.activation(dA[:, T:2 * T], delta[:], AF.Exp,
                             scale=Acols[:, s1:s1 + 1])
        # zero the boundary column so the scan carry resets between states
        nc.scalar.activation(dA[:, T:T + 1], gl["zcol"][:], AF.Copy)
        bcrep = sl.tile([DI, 2 * T], FH, tag="bcrep", name=f"brep{lid}_{p}")
        dBu = sl.tile([DI, 2 * T], FH, tag="dBu", name=f"dBu{lid}_{p}")
        for i, s in ((0, s0), (1, s1)):
            bps = bc_mm("bc", s, f"bps{lid}_{s}")
            nc.scalar.activation(bcrep[:, i * T:(i + 1) * T], bps[:], AF.Copy)
            nc.vector.tensor_mul(dBu[:, i * T:(i + 1) * T], dx[:],
                                 bcrep[:, i * T:(i + 1) * T])
        hs = sl.tile([DI, 2 * T], FH, tag="hs", name=f"hs{lid}_{p}")
        nc.vector.tensor_tensor_scan(hs[:], dA[:], dBu[:], 0.0,
                                     AX.mult, AX.add)
        ccrep = sl.tile([DI, 2 * T], FH, tag="ccrep", name=f"crep{lid}_{p}")
        for i, s in ((0, s0), (1, s1)):
            cps = bc_mm("bc", DS + s, f"cps{lid}_{s}")
            nc.scalar.activation(ccrep[:, i * T:(i + 1) * T], cps[:], AF.Copy)
        if p == 1:
            nc.vector.tensor_mul(yP[:], hs[:], ccrep[:])
        else:
            hsc = sl.tile([DI, 2 * T], FH, tag="hsc", name=f"hsc{lid}_{p}")
            nc.vector.tensor_mul(hsc[:], hs[:], ccrep[:])
            nc.vector.tensor_add(yP[:], yP[:], hsc[:])

    # s = 15
    dA15 = big.tile([DI, T], FH, tag="dAs2", name=f"dA{lid}_s15")
    nc.scalar.activation(dA15[:], delta[:], AF.Exp, scale=Acols[:, 15:16])
    bps15 = bc_mm("bc", 15, f"bps{lid}_15")
    brep15 = big.tile([DI, T], FH, tag="dBus2", name=f"brep{lid}_15")
    nc.scalar.activation(brep15[:], bps15[:], AF.Copy)
    dBu15 = big.tile([DI, T], FH, tag="dBuf", name=f"dBu{lid}_15")
    nc.vector.tensor_mul(dBu15[:], dx[:], brep15[:])
    hs15 = big.tile([DI, T], FH, tag="hss2", name=f"hs{lid}_s15")
    nc.vector.tensor_tensor_scan(hs15[:], dA15[:], dBu15[:], 0.0,
                                 AX.mult, AX.add)
    cps15 = bc_mm("bc", DS + 15, f"cps{lid}_15")
    crep15 = big.tile([DI, T], FH, tag="creps", name=f"crep{lid}_15")
    nc.scalar.activation(crep15[:], cps15[:], AF.Copy)
    hsc15 = big.tile([DI, T], FH, tag="hscs", name=f"hsc{lid}_15")
    nc.vector.tensor_mul(hsc15[:], hs15[:], crep15[:])
    nc.vector.tensor_add(ysn[:], ysn[:], hsc15[:])

    # ---- z-proj late (keeps the Act head short; silu set reloads once) ----
    zs = big.tile([DI, T], FH, tag="zs", name=f"zs{lid}")
    mmz = ps.tile([DI, T], FP, tag="bc", name=f"mmz{lid}")
    for c in range(NCH):
        o = c * 512
        nc.tensor.matmul(mmz[:, o:o + 512], zT, upo[:, o + 2:o + 514],
                         start=True, stop=True)
    nc.scalar.activation(zs[:], mmz[:], AF.Silu)
    xsz = big.tile([DI, T], FH, tag="xsz", name=f"xsz{lid}")
    nc.vector.tensor_mul(xsz[:], xact[:], zs[:])

    # ---- y = (sum_s hs*C)*silu(z); out = out_w@y + (out_w*D)@(x*silu(z)) ----
    yf = big.tile([DI, T], FH, tag="yf", name=f"yf{lid}")
    nc.vector.tensor_add(yf[:], yP[:, 0:T], yP[:, T:2 * T])
    nc.vector.tensor_add(yf[:], yf[:], ysn[:])
    nc.vector.tensor_mul(yf[:], yf[:], zs[:])

    mmo = ps.tile([DI, T], FP, tag="bc", name=f"mmo{lid}")
    for c in range(NCH):
        o = c * 512
        nc.tensor.matmul(mmo[0:DM, o:o + 512], outDT, xsz[:, o:o + 512],
                         start=True, stop=False)
        nc.tensor.matmul(mmo[0:DM, o:o + 512], outT, yf[:, o:o + 512],
                         start=False, stop=True)
    # chunked output copies: half h feeds the next layer's half-h head ops
    for h in range(2):
        src = mmo[0:DM, h * H:(h + 1) * H]
        for j, (tl, off) in enumerate(out_specs):
            dst = tl[:, off + h * H:off + (h + 1) * H]
            if j == 0:
                nc.scalar.activation(dst, src, AF.Copy)
            else:
                nc.vector.tensor_copy(dst, src)
        if out_dma is not None:
            nc.sync.dma_start(out_dma[:, h * H:(h + 1) * H],
                              out_specs[0][0][:, out_specs[0][1] + h * H:
                                              out_specs[0][1] + (h + 1) * H])


def _build_kernel(ctx, tc, u0, u0o, hblobs, fblobs, outs):
    nc = tc.nc
    const = ctx.enter_context(tc.tile_pool(name="const", bufs=1))
    big = ctx.enter_context(tc.tile_pool(name="big", bufs=1))
    sl = ctx.enter_context(tc.tile_pool(name="sl", bufs=2))
    ps = ctx.enter_context(tc.tile_pool(name="ps", bufs=2, space="PSUM"))

    hb = [const.tile([DI, _HBLOB_W], FH, tag=f"hb{l}", name=f"hb{l}")
          for l in range(NL)]
    fb = [const.tile([DI, 20], FP, tag=f"fb{l}", name=f"fb{l}")
          for l in range(NL)]
    upA = const.tile([DM, UF], FH, tag="upA", name="upA")
    upAo = const.tile([DM, UF], FH, tag="upAo", name="upAo")
    nc.sync.dma_start(hb[0][:], hblobs[0][:])
    nc.sync.dma_start(upA[:], u0[:])
    nc.sync.dma_start(upAo[:], u0o[:])
    nc.sync.dma_start(fb[0][:], fblobs[0][:])
    nc.sync.dma_start(hb[1][:], hblobs[1][:])
    nc.sync.dma_start(fb[1][:], fblobs[1][:])
    upB = const.tile([DM, UF], FH, tag="upB", name="upB")
    upBo = const.tile([DM, UF], FH, tag="upBo", name="upBo")
    nc.gpsimd.memset(upB[:, 0:K - 1], 0.0)
    nc.gpsimd.memset(upB[:, UF - 1:UF], 0.0)
    nc.gpsimd.memset(upBo[:, 0:K - 2], 0.0)
    nc.gpsimd.memset(upBo[:, UF - 2:UF], 0.0)
    o2 = const.tile([DM, T], FH, tag="o2", name="o2")

    # PE warm-up: ~4us of dummy matmuls while input DMAs land, so the HAM
    # clock gate is already at 8/8 when the real in_proj matmuls start.
    wz = const.tile([DI, 512], FH, tag="wz", name="wz")
    nc.gpsimd.memset(wz[:], 0.0)
    wps = ps.tile([DI, T], FP, tag="bc", name="warm")
    for i in range(16):
        nc.tensor.matmul(wps[:, 0:512], wz[:, 0:DI], wz[:],
                         start=True, stop=True)

    zcol = const.tile([DI, 1], FH, tag="zcol", name="zcol")
    nc.gpsimd.memset(zcol[:], 0.0)

    pools = (const, big, sl, ps, {"lid": 0, "zcol": zcol})
    # layer 1: outputs go to upB[:, 3:3+T] and upBo[:, 2:2+T]
    _build_layer(nc, pools, hb[0], fb[0], upA, upAo,
                 [(upB, K - 1), (upBo, K - 2)], outs[0])
    pools = (const, big, sl, ps, {"lid": 1, "zcol": zcol})
    _build_layer(nc, pools, hb[1], fb[1], upB, upBo, [(o2, 0)], outs[1])


def build_program():
    nc = bacc.Bacc("TRN2", target_bir_lowering=False, debug=False)
    u0 = nc.dram_tensor("u0", [DM, UF], FH, kind="ExternalInput").ap()
    u0o = nc.dram_tensor("u0o", [DM, UF], FH, kind="ExternalInput").ap()
    hblobs = [nc.dram_tensor(f"hblob{l}", [DI, _HBLOB_W], FH,
                             kind="ExternalInput").ap() for l in range(NL)]
    fblobs = [nc.dram_tensor(f"fblob{l}", [DI, 20], FP,
                             kind="ExternalInput").ap() for l in range(NL)]
    outs = [nc.dram_tensor(f"o{l + 1}T", [DM, T], FH,
                           kind="ExternalOutput").ap() for l in range(NL)]
    with tile.TileContext(nc) as tc:
        with ExitStack() as ctx:
            _build_kernel(ctx, tc, u0, u0o, hblobs, fblobs, outs)
    nc.compile()
    return nc


_PROG = None


def _get_prog():
    global _PROG
    if _PROG is None:
        _PROG = build_program()
    return _PROG


def _pad_u(u):
    """u: (64, T) f32 -> (u_pad, u_pad_odd) fp16 (64, UF)."""
    up = np.zeros((DM, UF), np.float16)
    up[:, K - 1:K - 1 + T] = u.astype(np.float16)
    upo = np.zeros((DM, UF), np.float16)
    upo[:, 0:UF - 1] = up[:, 1:UF]
    return up, upo


def _run_launch(u_list_T, raw, trace=False, trace_kwargs=None):
    """u_list_T: list of 8 arrays (64, 2048) f32. raw: param dict (np).
    Returns (o1_list, o2_list, res) with (64, 2048) fp16 outputs."""
    nc = _get_prog()
    blobs = [_pack_blobs(raw, l) for l in range(NL)]
    in_maps = []
    for b in range(8):
        up, upo = _pad_u(np.asarray(u_list_T[b], np.float32))
        in_maps.append({
            "u0": up, "u0o": upo,
            "hblob0": blobs[0][0], "fblob0": blobs[0][1],
            "hblob1": blobs[1][0], "fblob1": blobs[1][1],
        })
    res = bass_utils.run_bass_kernel_spmd(
        nc, in_maps, core_ids=list(range(8)), trace=trace,
        **(trace_kwargs or {}))
    o1 = [res.results[b]["o1T"] for b in range(8)]
    o2 = [res.results[b]["o2T"] for b in range(8)]
    return o1, o2, res


def kernel(**inputs):
    inp = {k: np.asarray(v, np.float32) for k, v in inputs.items()}
    Ms = inp["Ms_feature"]
    Pan = inp["Pan_feature"]
    h = C // 2
    names = ("in_w", "conv_w", "conv_b", "xp_w", "dt_w", "dt_b",
             "A_log", "D", "out_w")
    rawa = {n: inp["a_" + n] for n in names}
    rawb = {n: inp["b_" + n] for n in names}

    cf1 = np.concatenate([Ms[:, :h], Pan[:, h:]], axis=1)
    cf2 = np.concatenate([Pan[:, :h], Ms[:, h:]], axis=1)
    u_list = [cf1[b].T for b in range(B)] + [cf2[b].T for b in range(B)]
    o1, o2, _ = _run_launch(u_list, rawa)
    cf1_1 = np.stack([o1[b].T.astype(np.float32) for b in range(B)])
    cf2_1 = np.stack([o1[B + b].T.astype(np.float32) for b in range(B)])
    cf1_2 = np.stack([o2[b].T.astype(np.float32) for b in range(B)])
    cf2_2 = np.stack([o2[B + b].T.astype(np.float32) for b in range(B)])
    Ms1 = np.maximum((cf1_1 + cf2_1) * 0.5 + Ms, 0.0)
    Ms2 = np.maximum((cf1_2 + cf2_2) * 0.5 + Ms1, 0.0)

    cf3 = np.stack([Pan[:, ::2], Ms2[:, 1::2]], axis=2).reshape(B, C, DM)
    cf4 = np.stack([Ms2[:, ::2], Pan[:, 1::2]], axis=2).reshape(B, C, DM)
    u_list = [cf3[b].T for b in range(B)] + [cf4[b].T for b in range(B)]
    o1, o2, _ = _run_launch(u_list, rawb)
    cf3_1 = np.stack([o1[b].T.astype(np.float32) for b in range(B)])
    cf4_1 = np.stack([o1[B + b].T.astype(np.float32) for b in range(B)])
    cf3_2 = np.stack([o2[b].T.astype(np.float32) for b in range(B)])
    cf4_2 = np.stack([o2[B + b].T.astype(np.float32) for b in range(B)])
    Pan1 = np.maximum((cf3_1 + cf4_1) * 0.5 + Pan, 0.0)
    Pan2 = np.maximum((cf3_2 + cf4_2) * 0.5 + Pan1, 0.0)
    return Ms2, Pan2


# revision 19
# speedup vs baseline: 1.1586x; 1.1586x over previous
"""Trainium2 Bass kernel for nn_CMCI_Mamba.

Strategy: data-parallel over the 2B=8 mamba streams (1 sequence per core).
Each launch runs 2 chained mamba layers fully on-chip in d-major layout
(features on partitions, time on the free axis).

Engine assignment (per layer):
- PE (fp16): in_proj with the causal conv FOLDED IN (4 shifted matmuls with
  host-prescaled weights diag(conv_w_k) @ in_w), z-proj, fused
  dt_w@xp_w[dt] projection, 32 stride-0 B/C broadcast matmuls, out_proj.
- Act: Silu(conv) / Silu(z) straight from PSUM, softplus via Exp+Ln (one
  table set), the 16 per-state dA = exp(A_s * delta) passes, PSUM->SBUF
  fp16 copies of the B/C broadcasts, layer-output copies.
- DVE: the 16 SSM scans (tensor_tensor_scan, batched 2 states per
  instruction with a zeroed dA column resetting the carry), dBu muls,
  half of the hs*C muls/accumulation.
- GPSIMD: the other half of the hs*C muls and y accumulation.

Host does the cheap cross-stream elementwise combines between launches.
"""
import sys
import numpy as np
from contextlib import ExitStack

for _p in ("/opt/trn_rl_repo",):
    if _p not in sys.path:
        sys.path.insert(0, _p)

import concourse.bass as bass
import concourse.bacc as bacc
import concourse.tile as tile
from concourse import mybir
from concourse import bass_utils

T, DM, DI, DS, DR, K, NL = 2048, 64, 128, 16, 4, 4, 2
B, C = 4, 2048
UF = T + K  # padded u width (2052)
FP = mybir.dt.float32
FH = mybir.dt.float16
AX = mybir.AluOpType
AF = mybir.ActivationFunctionType

# fp16 param blob column layout, (128, 1024) per layer
_B_WK = 0       # [0:64, 0:512]    4x conv-scaled in_proj-x lhsT (64,128) each
_B_Z = 512      # [0:64, 512:640]  z lhsT
_B_WD = 640     # [:, 640:768]     (dt_w @ xp_w[:DR]) lhsT
_B_BC = 768     # [:, 768:800]     B/C projection columns (32)
_B_OUT = 800    # [:, 800:864]     out_proj lhsT
_B_OUTD = 864   # [:, 864:928]     out_proj lhsT with D folded (for x*sz term)
_B_EYE = 928    # [0:32, 928:960]  eye(32) one-hot selectors for row broadcast
_HBLOB_W = 1024
# fp32 blob (128, 20): [:, 0:16]=A (=-exp(A_log)), 16=conv_b, 17=dt_b, 18=D


def _pack_blobs(raw, l):
    hb = np.zeros((DI, _HBLOB_W), np.float16)
    in_w = raw["in_w"][l]          # (256, 64)
    conv_w = raw["conv_w"][l]      # (128, 4)
    for k in range(K):
        wk = in_w[:DI] * conv_w[:, k:k + 1]          # (128, 64)
        hb[:DM, _B_WK + 128 * k:_B_WK + 128 * (k + 1)] = wk.T
    hb[:DM, _B_Z:_B_Z + DI] = in_w[DI:2 * DI].T
    wd = raw["dt_w"][l] @ raw["xp_w"][l][:DR]        # (128, 128)
    hb[:, _B_WD:_B_WD + DI] = wd.T
    hb[:, _B_BC:_B_BC + 2 * DS] = raw["xp_w"][l][DR:DR + 2 * DS].T
    hb[:, _B_OUT:_B_OUT + DM] = raw["out_w"][l].T
    # out_proj with D folded in: out += (out_w * D) @ (x * silu(z))
    hb[:, _B_OUTD:_B_OUTD + DM] = (raw["out_w"][l] * raw["D"][l]).T
    hb[0:2 * DS, _B_EYE:_B_EYE + 2 * DS] = np.eye(2 * DS, dtype=np.float16)
    fb = np.zeros((DI, 20), np.float32)
    fb[:, 0:DS] = -np.exp(raw["A_log"][l])
    fb[:, 16] = raw["conv_b"][l]
    fb[:, 17] = raw["dt_b"][l]
    fb[:, 18] = raw["D"][l]
    return hb, fb


def _build_layer(nc, pools, hb, fb, up, upo, out_specs, out_dma):
    """One mamba layer. up/upo: (64, UF) fp16 padded input (+1-shifted copy).
    out_specs: list of (tile, col_off) -- the (64, T) layer output is copied
    (in halves, on Act) into tile[:, off:off+T]. out_dma: DRAM ap or None.
    """
    const, big, sl, ps, gl = pools
    NCH = T // 512
    H = T // 2
    lid = gl["lid"]

    wkT = [hb[0:DM, _B_WK + 128 * k:_B_WK + 128 * (k + 1)] for k in range(K)]
    zT = hb[0:DM, _B_Z:_B_Z + DI]
    wdT = hb[:, _B_WD:_B_WD + DI]
    outT = hb[:, _B_OUT:_B_OUT + DM]
    outDT = hb[:, _B_OUTD:_B_OUTD + DM]
    Acols = fb[:, 0:DS]
    convb = fb[:, 16:17]
    dtb = fb[:, 17:18]

    def bc_mm(tag, col, name):
        """Row-broadcast matmul: contraction-1 ones x precomputed B/C row.
        128x fewer active MACs than the full stride-0 re-projection, which
        matters because this kernel runs power-throttled."""
        t = ps.tile([DI, T], FP, tag="bc", name=name)
        sel = hb[0:2 * DS, _B_EYE + col:_B_EYE + col + 1].broadcast_to(
            (2 * DS, DI))
        for c in range(NCH):
            nc.tensor.matmul(t[:, c * 512:(c + 1) * 512], sel,
                             bcr[:, c * 512:(c + 1) * 512],
                             start=True, stop=True)
        return t

    # ---- in_proj-x with folded causal conv -> silu -> xact (fp16) ----
    # xc[:, t] = sum_k (diag(conv_w_k) @ in_w_x) @ u[:, t-3+k]; tap k reads
    # u_pad[:, c*512+k:]; odd k uses the 1-shifted copy so every rhs offset
    # stays 4B-aligned.  Silu is applied per half so the delta chain starts
    # as soon as the first half lands.
    xact = big.tile([DI, T], FH, tag="xact", name=f"xact{lid}")
    mmx = ps.tile([DI, T], FP, tag="bc", name=f"mmx{lid}")
    for c in range(NCH):
        o = c * 512
        cs = slice(o, o + 512)
        nc.tensor.matmul(mmx[:, cs], wkT[0], up[:, o:o + 512],
                         start=True, stop=False)
        nc.tensor.matmul(mmx[:, cs], wkT[1], upo[:, o:o + 512],
                         start=False, stop=False)
        nc.tensor.matmul(mmx[:, cs], wkT[2], up[:, o + 2:o + 514],
                         start=False, stop=False)
        nc.tensor.matmul(mmx[:, cs], wkT[3], upo[:, o + 2:o + 514],
                         start=False, stop=True)
    for h in range(2):
        hs_ = slice(h * H, (h + 1) * H)
        nc.scalar.activation(xact[:, hs_], mmx[:, hs_], AF.Silu, bias=convb)

    # ---- delta = softplus(dt_proj + dt_b) via Exp then Ln(1+x), halves ----
    delta = big.tile([DI, T], FH, tag="delta", name=f"delta{lid}")
    ev = big.tile([DI, T], FH, tag="ev", name=f"ev{lid}")
    dx = big.tile([DI, T], FH, tag="dx", name=f"dx{lid}")
    mmd = ps.tile([DI, T], FP, tag="bc", name=f"mmd{lid}")
    for c in range(NCH):
        o = c * 512
        nc.tensor.matmul(mmd[:, o:o + 512], wdT, xact[:, o:o + 512],
                         start=True, stop=True)
    for h in range(2):
        hs_ = slice(h * H, (h + 1) * H)
        nc.scalar.activation(ev[:, hs_], mmd[:, hs_], AF.Exp, bias=dtb)
    for h in range(2):
        hs_ = slice(h * H, (h + 1) * H)
        nc.scalar.activation(delta[:, hs_], ev[:, hs_], AF.Ln, bias=1.0)
        nc.vector.tensor_mul(dx[:, hs_], delta[:, hs_], xact[:, hs_])

    # ---- B/C projection rows (32, T): one matmul group + one copy ----
    bcr = big.tile([2 * DS, T], FH, tag="bcr", name=f"bcr{lid}")
    mmb = ps.tile([DI, T], FP, tag="bc", name=f"mmb{lid}")
    for c in range(NCH):
        o = c * 512
        nc.tensor.matmul(mmb[0:2 * DS, o:o + 512], hb[:, _B_BC:_B_BC + 2 * DS],
                         xact[:, o:o + 512], start=True, stop=True)
    nc.scalar.activation(bcr[:], mmb[0:2 * DS, :], AF.Copy)

    # ---- s-loop: single s=0 (PSUM-direct, shortest ramp), 7 pairs, s=15 ----
    ysn = big.tile([DI, T], FH, tag="ysn", name=f"ysn{lid}")
    yP = big.tile([DI, 2 * T], FH, tag="yP", name=f"yP{lid}")

    # s = 0
    dA0 = big.tile([DI, T], FH, tag="dAs", name=f"dA{lid}_s0")
    for h in range(2):
        hs_ = slice(h * H, (h + 1) * H)
        nc.scalar.activation(dA0[:, hs_], delta[:, hs_], AF.Exp,
                             scale=Acols[:, 0:1])
    bps0 = bc_mm("bc", 0, f"bps{lid}_0")
    dBu0 = big.tile([DI, T], FH, tag="dBus", name=f"dBu{lid}_s0")
    nc.vector.tensor_mul(dBu0[:], dx[:], bps0[:])
    hs0 = big.tile([DI, T], FH, tag="hss", name=f"hs{lid}_s0")
    nc.vector.tensor_tensor_scan(hs0[:], dA0[:], dBu0[:], 0.0, AX.mult, AX.add)
    cps0 = bc_mm("bc", DS + 0, f"cps{lid}_0")
    nc.vector.tensor_mul(ysn[:], hs0[:], cps0[:])

    # pairs (1,2) .. (13,14)
    for p in range(1, 8):
        s0, s1 = 2 * p - 1, 2 * p
        dA = sl.tile([DI, 2 * T], FH, tag="dA", name=f"dA{lid}_{p}")
        nc.scalar.activation(dA[:, 0:T], delta[:], AF.Exp,
                             scale=Acols[:, s0:s0 + 1])
        nc.scalar.activation(dA[:, T:2 * T], delta[:], AF.Exp,
                             scale=Acols[:, s1:s1 + 1])
        # zero the boundary column so the scan carry resets between states
        nc.scalar.activation(dA[:, T:T + 1], gl["zcol"][:], AF.Copy)
        bcrep = sl.tile([DI, 2 * T], FH, tag="bcrep", name=f"brep{lid}_{p}")
        dBu = sl.tile([DI, 2 * T], FH, tag="dBu", name=f"dBu{lid}_{p}")
        for i, s in ((0, s0), (1, s1)):
            bps = bc_mm("bc", s, f"bps{lid}_{s}")
            nc.scalar.activation(bcrep[:, i * T:(i + 1) * T], bps[:], AF.Copy)
            nc.vector.tensor_mul(dBu[:, i * T:(i + 1) * T], dx[:],
                                 bcrep[:, i * T:(i + 1) * T])
        hs = sl.tile([DI, 2 * T], FH, tag="hs", name=f"hs{lid}_{p}")
        nc.vector.tensor_tensor_scan(hs[:], dA[:], dBu[:], 0.0,
                                     AX.mult, AX.add)
        ccrep = sl.tile([DI, 2 * T], FH, tag="ccrep", name=f"crep{lid}_{p}")
        for i, s in ((0, s0), (1, s1)):
            cps = bc_mm("bc", DS + s, f"cps{lid}_{s}")
            nc.scalar.activation(ccrep[:, i * T:(i + 1) * T], cps[:], AF.Copy)
        if p == 1:
            nc.vector.tensor_mul(yP[:], hs[:], ccrep[:])
        else:
            hsc = sl.tile([DI, 2 * T], FH, tag="hsc", name=f"hsc{lid}_{p}")
            nc.vector.tensor_mul(hsc[:], hs[:], ccrep[:])
            nc.vector.tensor_add(yP[:], yP[:], hsc[:])

    # s = 15
    dA15 = big.tile([DI, T], FH, tag="dAs2", name=f"dA{lid}_s15")
    nc.scalar.activation(dA15[:], delta[:], AF.Exp, scale=Acols[:, 15:16])
    bps15 = bc_mm("bc", 15, f"bps{lid}_15")
    brep15 = big.tile([DI, T], FH, tag="dBus2", name=f"brep{lid}_15")
    nc.scalar.activation(brep15[:], bps15[:], AF.Copy)
    dBu15 = big.tile([DI, T], FH, tag="dBuf", name=f"dBu{lid}_15")
    nc.vector.tensor_mul(dBu15[:], dx[:], brep15[:])
    hs15 = big.tile([DI, T], FH, tag="hss2", name=f"hs{lid}_s15")
    nc.vector.tensor_tensor_scan(hs15[:], dA15[:], dBu15[:], 0.0,
                                 AX.mult, AX.add)
    cps15 = bc_mm("bc", DS + 15, f"cps{lid}_15")
    crep15 = big.tile([DI, T], FH, tag="creps", name=f"crep{lid}_15")
    nc.scalar.activation(crep15[:], cps15[:], AF.Copy)
    hsc15 = big.tile([DI, T], FH, tag="hscs", name=f"hsc{lid}_15")
    nc.vector.tensor_mul(hsc15[:], hs15[:], crep15[:])
    nc.vector.tensor_add(ysn[:], ysn[:], hsc15[:])

    # ---- z-proj late (keeps the Act head short; silu set reloads once) ----
    zs = big.tile([DI, T], FH, tag="zs", name=f"zs{lid}")
    mmz = ps.tile([DI, T], FP, tag="bc", name=f"mmz{lid}")
    for c in range(NCH):
        o = c * 512
        nc.tensor.matmul(mmz[:, o:o + 512], zT, upo[:, o + 2:o + 514],
                         start=True, stop=True)
    nc.scalar.activation(zs[:], mmz[:], AF.Silu)
    xsz = big.tile([DI, T], FH, tag="xsz", name=f"xsz{lid}")
    nc.vector.tensor_mul(xsz[:], xact[:], zs[:])

    # ---- y = (sum_s hs*C)*silu(z); out = out_w@y + (out_w*D)@(x*silu(z)) ----
    yf = big.tile([DI, T], FH, tag="yf", name=f"yf{lid}")
    nc.vector.tensor_add(yf[:], yP[:, 0:T], yP[:, T:2 * T])
    nc.vector.tensor_add(yf[:], yf[:], ysn[:])
    nc.vector.tensor_mul(yf[:], yf[:], zs[:])

    mmo = ps.tile([DI, T], FP, tag="bc", name=f"mmo{lid}")
    for c in range(NCH):
        o = c * 512
        nc.tensor.matmul(mmo[0:DM, o:o + 512], outDT, xsz[:, o:o + 512],
                         start=True, stop=False)
        nc.tensor.matmul(mmo[0:DM, o:o + 512], outT, yf[:, o:o + 512],
                         start=False, stop=True)
    # chunked output copies: half h feeds the next layer's half-h head ops
    for h in range(2):
        src = mmo[0:DM, h * H:(h + 1) * H]
        for tl, off in out_specs:
            nc.scalar.activation(tl[:, off + h * H:off + (h + 1) * H],
                                 src, AF.Copy)
        if out_dma is not None:
            nc.sync.dma_start(out_dma[:, h * H:(h + 1) * H],
                              out_specs[0][0][:, out_specs[0][1] + h * H:
                                              out_specs[0][1] + (h + 1) * H])


def _build_kernel(ctx, tc, u0, u0o, hblobs, fblobs, outs):
    nc = tc.nc
    const = ctx.enter_context(tc.tile_pool(name="const", bufs=1))
    big = ctx.enter_context(tc.tile_pool(name="big", bufs=1))
    sl = ctx.enter_context(tc.tile_pool(name="sl", bufs=2))
    ps = ctx.enter_context(tc.tile_pool(name="ps", bufs=2, space="PSUM"))

    hb = [const.tile([DI, _HBLOB_W], FH, tag=f"hb{l}", name=f"hb{l}")
          for l in range(NL)]
    fb = [const.tile([DI, 20], FP, tag=f"fb{l}", name=f"fb{l}")
          for l in range(NL)]
    upA = const.tile([DM, UF], FH, tag="upA", name="upA")
    upAo = const.tile([DM, UF], FH, tag="upAo", name="upAo")
    nc.sync.dma_start(hb[0][:], hblobs[0][:])
    nc.sync.dma_start(upA[:], u0[:])
    nc.sync.dma_start(upAo[:], u0o[:])
    nc.sync.dma_start(fb[0][:], fblobs[0][:])
    nc.sync.dma_start(hb[1][:], hblobs[1][:])
    nc.sync.dma_start(fb[1][:], fblobs[1][:])
    upB = const.tile([DM, UF], FH, tag="upB", name="upB")
    upBo = const.tile([DM, UF], FH, tag="upBo", name="upBo")
    nc.gpsimd.memset(upB[:, 0:K - 1], 0.0)
    nc.gpsimd.memset(upB[:, UF - 1:UF], 0.0)
    nc.gpsimd.memset(upBo[:, 0:K - 2], 0.0)
    nc.gpsimd.memset(upBo[:, UF - 2:UF], 0.0)
    o2 = const.tile([DM, T], FH, tag="o2", name="o2")


    zcol = const.tile([DI, 1], FH, tag="zcol", name="zcol")
    nc.gpsimd.memset(zcol[:], 0.0)
    ones = const.tile([1, 2], FH, tag="ones", name="ones")
    nc.gpsimd.memset(ones[:], 1.0)

    pools = (const, big, sl, ps, {"lid": 0, "zcol": zcol, "ones": ones})
    # layer 1: outputs go to upB[:, 3:3+T] and upBo[:, 2:2+T]
    _build_layer(nc, pools, hb[0], fb[0], upA, upAo,
                 [(upB, K - 1), (upBo, K - 2)], outs[0])
    pools = (const, big, sl, ps, {"lid": 1, "zcol": zcol, "ones": ones})
    _build_layer(nc, pools, hb[1], fb[1], upB, upBo, [(o2, 0)], outs[1])


def build_program():
    nc = bacc.Bacc("TRN2", target_bir_lowering=False, debug=False)
    u0 = nc.dram_tensor("u0", [DM, UF], FH, kind="ExternalInput").ap()
    u0o = nc.dram_tensor("u0o", [DM, UF], FH, kind="ExternalInput").ap()
    hblobs = [nc.dram_tensor(f"hblob{l}", [DI, _HBLOB_W], FH,
                             kind="ExternalInput").ap() for l in range(NL)]
    fblobs = [nc.dram_tensor(f"fblob{l}", [DI, 20], FP,
                             kind="ExternalInput").ap() for l in range(NL)]
    outs = [nc.dram_tensor(f"o{l + 1}T", [DM, T], FH,
                           kind="ExternalOutput").ap() for l in range(NL)]
    with tile.TileContext(nc) as tc:
        with ExitStack() as ctx:
            _build_kernel(ctx, tc, u0, u0o, hblobs, fblobs, outs)
    nc.compile()
    return nc


_PROG = None


def _get_prog():
    global _PROG
    if _PROG is None:
        _PROG = build_program()
    return _PROG


def _pad_u(u):
    """u: (64, T) f32 -> (u_pad, u_pad_odd) fp16 (64, UF)."""
    up = np.zeros((DM, UF), np.float16)
    up[:, K - 1:K - 1 + T] = u.astype(np.float16)
    upo = np.zeros((DM, UF), np.float16)
    upo[:, 0:UF - 1] = up[:, 1:UF]
    return up, upo


def _run_launch(u_list_T, raw, trace=False, trace_kwargs=None):
    """u_list_T: list of 8 arrays (64, 2048) f32. raw: param dict (np).
    Returns (o1_list, o2_list, res) with (64, 2048) fp16 outputs."""
    nc = _get_prog()
    blobs = [_pack_blobs(raw, l) for l in range(NL)]
    in_maps = []
    for b in range(8):
        up, upo = _pad_u(np.asarray(u_list_T[b], np.float32))
        in_maps.append({
            "u0": up, "u0o": upo,
            "hblob0": blobs[0][0], "fblob0": blobs[0][1],
            "hblob1": blobs[1][0], "fblob1": blobs[1][1],
        })
    res = bass_utils.run_bass_kernel_spmd(
        nc, in_maps, core_ids=list(range(8)), trace=trace,
        **(trace_kwargs or {}))
    o1 = [res.results[b]["o1T"] for b in range(8)]
    o2 = [res.results[b]["o2T"] for b in range(8)]
    return o1, o2, res


def kernel(**inputs):
    inp = {k: np.asarray(v, np.float32) for k, v in inputs.items()}
    Ms = inp["Ms_feature"]
    Pan = inp["Pan_feature"]
    h = C // 2
    names = ("in_w", "conv_w", "conv_b", "xp_w", "dt_w", "dt_b",
             "A_log", "D", "out_w")
    rawa = {n: inp["a_" + n] for n in names}
    rawb = {n: inp["b_" + n] for n in names}

    cf1 = np.concatenate([Ms[:, :h], Pan[:, h:]], axis=1)
    cf2 = np.concatenate([Pan[:, :h], Ms[:, h:]], axis=1)
    u_list = [cf1[b].T for b in range(B)] + [cf2[b].T for b in range(B)]
    o1, o2, _ = _run_launch(u_list, rawa)
    cf1_1 = np.stack([o1[b].T.astype(np.float32) for b in range(B)])
    cf2_1 = np.stack([o1[B + b].T.astype(np.float32) for b in range(B)])
    cf1_2 = np.stack([o2[b].T.astype(np.float32) for b in range(B)])
    cf2_2 = np.stack([o2[B + b].T.astype(np.float32) for b in range(B)])
    Ms1 = np.maximum((cf1_1 + cf2_1) * 0.5 + Ms, 0.0)
    Ms2 = np.maximum((cf1_2 + cf2_2) * 0.5 + Ms1, 0.0)

    cf3 = np.stack([Pan[:, ::2], Ms2[:, 1::2]], axis=2).reshape(B, C, DM)
    cf4 = np.stack([Ms2[:, ::2], Pan[:, 1::2]], axis=2).reshape(B, C, DM)
    u_list = [cf3[b].T for b in range(B)] + [cf4[b].T for b in range(B)]
    o1, o2, _ = _run_launch(u_list, rawb)
    cf3_1 = np.stack([o1[b].T.astype(np.float32) for b in range(B)])
    cf4_1 = np.stack([o1[B + b].T.astype(np.float32) for b in range(B)])
    cf3_2 = np.stack([o2[b].T.astype(np.float32) for b in range(B)])
    cf4_2 = np.stack([o2[B + b].T.astype(np.float32) for b in range(B)])
    Pan1 = np.maximum((cf3_1 + cf4_1) * 0.5 + Pan, 0.0)
    Pan2 = np.maximum((cf3_2 + cf4_2) * 0.5 + Pan1, 0.0)
    return Ms2, Pan2


# revision 20
# speedup vs baseline: 1.1769x; 1.0158x over previous
"""Trainium2 Bass kernel for nn_CMCI_Mamba.

Strategy: data-parallel over the 2B=8 mamba streams (1 sequence per core).
Each launch runs 2 chained mamba layers fully on-chip in d-major layout
(features on partitions, time on the free axis).

Engine assignment (per layer):
- PE (fp16): in_proj with the causal conv FOLDED IN (4 shifted matmuls with
  host-prescaled weights diag(conv_w_k) @ in_w), z-proj, fused
  dt_w@xp_w[dt] projection, 32 stride-0 B/C broadcast matmuls, out_proj.
- Act: Silu(conv) / Silu(z) straight from PSUM, softplus via Exp+Ln (one
  table set), the 16 per-state dA = exp(A_s * delta) passes, PSUM->SBUF
  fp16 copies of the B/C broadcasts, layer-output copies.
- DVE: the 16 SSM scans (tensor_tensor_scan, batched 2 states per
  instruction with a zeroed dA column resetting the carry), dBu muls,
  half of the hs*C muls/accumulation.
- GPSIMD: the other half of the hs*C muls and y accumulation.

Host does the cheap cross-stream elementwise combines between launches.
"""
import sys
import numpy as np
from contextlib import ExitStack

for _p in ("/opt/trn_rl_repo",):
    if _p not in sys.path:
        sys.path.insert(0, _p)

import concourse.bass as bass
import concourse.bacc as bacc
import concourse.tile as tile
from concourse import mybir
from concourse import bass_utils

T, DM, DI, DS, DR, K, NL = 2048, 64, 128, 16, 4, 4, 2
B, C = 4, 2048
UF = T + K  # padded u width (2052)
FP = mybir.dt.float32
FH = mybir.dt.float16
AX = mybir.AluOpType
AF = mybir.ActivationFunctionType

# fp16 param blob column layout, (128, 1024) per layer
_B_WK = 0       # [0:64, 0:512]    4x conv-scaled in_proj-x lhsT (64,128) each
_B_Z = 512      # [0:64, 512:640]  z lhsT
_B_WD = 640     # [:, 640:768]     (dt_w @ xp_w[:DR]) lhsT
_B_BC = 768     # [:, 768:800]     B/C projection columns (32)
_B_OUT = 800    # [:, 800:864]     out_proj lhsT
_B_OUTD = 864   # [:, 864:928]     out_proj lhsT with D folded (for x*sz term)
_B_EYE = 928    # [0:32, 928:960]  eye(32) one-hot selectors for row broadcast
_HBLOB_W = 1024
# fp32 blob (128, 20): [:, 0:16]=A (=-exp(A_log)), 16=conv_b, 17=dt_b, 18=D


def _pack_blobs(raw, l):
    hb = np.zeros((DI, _HBLOB_W), np.float16)
    in_w = raw["in_w"][l]          # (256, 64)
    conv_w = raw["conv_w"][l]      # (128, 4)
    for k in range(K):
        wk = in_w[:DI] * conv_w[:, k:k + 1]          # (128, 64)
        hb[:DM, _B_WK + 128 * k:_B_WK + 128 * (k + 1)] = wk.T
    hb[:DM, _B_Z:_B_Z + DI] = in_w[DI:2 * DI].T
    wd = raw["dt_w"][l] @ raw["xp_w"][l][:DR]        # (128, 128)
    hb[:, _B_WD:_B_WD + DI] = wd.T
    hb[:, _B_BC:_B_BC + 2 * DS] = raw["xp_w"][l][DR:DR + 2 * DS].T
    hb[:, _B_OUT:_B_OUT + DM] = raw["out_w"][l].T
    # out_proj with D folded in: out += (out_w * D) @ (x * silu(z))
    hb[:, _B_OUTD:_B_OUTD + DM] = (raw["out_w"][l] * raw["D"][l]).T
    hb[0:2 * DS, _B_EYE:_B_EYE + 2 * DS] = np.eye(2 * DS, dtype=np.float16)
    fb = np.zeros((DI, 20), np.float32)
    fb[:, 0:DS] = -np.exp(raw["A_log"][l])
    fb[:, 16] = raw["conv_b"][l]
    fb[:, 17] = raw["dt_b"][l]
    fb[:, 18] = raw["D"][l]
    return hb, fb


def _build_layer(nc, pools, hb, fb, up, upo, out_specs, out_dma):
    """One mamba layer. up/upo: (64, UF) fp16 padded input (+1-shifted copy).
    out_specs: list of (tile, col_off) -- the (64, T) layer output is copied
    (in halves, on Act) into tile[:, off:off+T]. out_dma: DRAM ap or None.
    """
    const, big, sl, ps, gl = pools
    NCH = T // 512
    H = T // 2
    lid = gl["lid"]

    wkT = [hb[0:DM, _B_WK + 128 * k:_B_WK + 128 * (k + 1)] for k in range(K)]
    zT = hb[0:DM, _B_Z:_B_Z + DI]
    wdT = hb[:, _B_WD:_B_WD + DI]
    outT = hb[:, _B_OUT:_B_OUT + DM]
    outDT = hb[:, _B_OUTD:_B_OUTD + DM]
    Acols = fb[:, 0:DS]
    convb = fb[:, 16:17]
    dtb = fb[:, 17:18]

    def bc_mm(tag, col, name):
        """Row-broadcast matmul: contraction-1 ones x precomputed B/C row.
        128x fewer active MACs than the full stride-0 re-projection, which
        matters because this kernel runs power-throttled."""
        t = ps.tile([DI, T], FP, tag="bc", name=name)
        sel = hb[0:2 * DS, _B_EYE + col:_B_EYE + col + 1].broadcast_to(
            (2 * DS, DI))
        for c in range(NCH):
            nc.tensor.matmul(t[:, c * 512:(c + 1) * 512], sel,
                             bcr[:, c * 512:(c + 1) * 512],
                             start=True, stop=True)
        return t

    # ---- in_proj-x with folded causal conv -> silu -> xact (fp16) ----
    # xc[:, t] = sum_k (diag(conv_w_k) @ in_w_x) @ u[:, t-3+k]; tap k reads
    # u_pad[:, c*512+k:]; odd k uses the 1-shifted copy so every rhs offset
    # stays 4B-aligned.  Silu is applied per half so the delta chain starts
    # as soon as the first half lands.
    xact = big.tile([DI, T], FH, tag="xact", name=f"xact{lid}")
    for h in range(2):
        mmx = ps.tile([DI, H], FP, tag="bc", name=f"mmx{lid}_{h}")
        for c in (2 * h, 2 * h + 1):
            o = c * 512
            cs = slice(o - h * H, o - h * H + 512)
            nc.tensor.matmul(mmx[:, cs], wkT[0], up[:, o:o + 512],
                             start=True, stop=False)
            nc.tensor.matmul(mmx[:, cs], wkT[1], upo[:, o:o + 512],
                             start=False, stop=False)
            nc.tensor.matmul(mmx[:, cs], wkT[2], up[:, o + 2:o + 514],
                             start=False, stop=False)
            nc.tensor.matmul(mmx[:, cs], wkT[3], upo[:, o + 2:o + 514],
                             start=False, stop=True)
        nc.scalar.activation(xact[:, h * H:(h + 1) * H], mmx[:], AF.Silu,
                             bias=convb)

    # ---- delta = softplus(dt_proj + dt_b) via Exp then Ln(1+x), halves ----
    delta = big.tile([DI, T], FH, tag="delta", name=f"delta{lid}")
    ev = big.tile([DI, T], FH, tag="ev", name=f"ev{lid}")
    dx = big.tile([DI, T], FH, tag="dx", name=f"dx{lid}")
    for h in range(2):
        mmd = ps.tile([DI, H], FP, tag="bc", name=f"mmd{lid}_{h}")
        for c in (2 * h, 2 * h + 1):
            o = c * 512
            nc.tensor.matmul(mmd[:, o - h * H:o - h * H + 512], wdT,
                             xact[:, o:o + 512], start=True, stop=True)
        nc.scalar.activation(ev[:, h * H:(h + 1) * H], mmd[:], AF.Exp,
                             bias=dtb)
    for h in range(2):
        hs_ = slice(h * H, (h + 1) * H)
        nc.scalar.activation(delta[:, hs_], ev[:, hs_], AF.Ln, bias=1.0)
        nc.vector.tensor_mul(dx[:, hs_], delta[:, hs_], xact[:, hs_])

    # ---- s-loop: single s=0 first (via stride-0 direct broadcasts and a
    # half-chained scan, so the first scan starts during the Act ramp),
    # then 7 pairs off precomputed B/C rows, then single s=15 ----
    ysn = big.tile([DI, T], FH, tag="ysn", name=f"ysn{lid}")
    yP = big.tile([DI, 2 * T], FH, tag="yP", name=f"yP{lid}")

    dA0 = big.tile([DI, T], FH, tag="dAs", name=f"dA{lid}_s0")
    dBu0 = big.tile([DI, T], FH, tag="dBus", name=f"dBu{lid}_s0")
    hs0 = big.tile([DI, T], FH, tag="hss", name=f"hs{lid}_s0")
    bps0 = ps.tile([DI, T], FP, tag="bc", name=f"bps{lid}_0")
    bw = hb[:, _B_BC:_B_BC + 1].broadcast_to((DI, DI))
    for h in range(2):
        hh = slice(h * H, (h + 1) * H)
        nc.scalar.activation(dA0[:, hh], delta[:, hh], AF.Exp,
                             scale=Acols[:, 0:1])
        for c in (2 * h, 2 * h + 1):
            nc.tensor.matmul(bps0[:, c * 512:(c + 1) * 512], bw,
                             xact[:, c * 512:(c + 1) * 512],
                             start=True, stop=True)
        nc.vector.tensor_mul(dBu0[:, hh], dx[:, hh], bps0[:, hh])
        nc.vector.tensor_tensor_scan(
            hs0[:, hh], dA0[:, hh], dBu0[:, hh],
            0.0 if h == 0 else hs0[:, H - 1:H], AX.mult, AX.add)
    cw = hb[:, _B_BC + DS:_B_BC + DS + 1].broadcast_to((DI, DI))
    cps0 = ps.tile([DI, T], FP, tag="bc", name=f"cps{lid}_0")
    for c in range(NCH):
        nc.tensor.matmul(cps0[:, c * 512:(c + 1) * 512], cw,
                         xact[:, c * 512:(c + 1) * 512], start=True, stop=True)
    nc.vector.tensor_mul(ysn[:], hs0[:], cps0[:])

    # ---- B/C projection rows (32, T) for the pair loop ----
    bcr = big.tile([2 * DS, T], FH, tag="bcr", name=f"bcr{lid}")
    mmb = ps.tile([DI, T], FP, tag="bc", name=f"mmb{lid}")
    for c in range(NCH):
        o = c * 512
        nc.tensor.matmul(mmb[0:2 * DS, o:o + 512], hb[:, _B_BC:_B_BC + 2 * DS],
                         xact[:, o:o + 512], start=True, stop=True)
    nc.scalar.activation(bcr[:], mmb[0:2 * DS, :], AF.Copy)

    # pairs (1,2) .. (13,14)
    for p in range(1, 8):
        s0, s1 = 2 * p - 1, 2 * p
        dA = sl.tile([DI, 2 * T], FH, tag="dA", name=f"dA{lid}_{p}")
        nc.scalar.activation(dA[:, 0:T], delta[:], AF.Exp,
                             scale=Acols[:, s0:s0 + 1])
        nc.scalar.activation(dA[:, T:2 * T], delta[:], AF.Exp,
                             scale=Acols[:, s1:s1 + 1])
        # zero the boundary column so the scan carry resets between states
        nc.scalar.activation(dA[:, T:T + 1], gl["zcol"][:], AF.Copy)
        bcrep = sl.tile([DI, 2 * T], FH, tag="bcrep", name=f"brep{lid}_{p}")
        dBu = sl.tile([DI, 2 * T], FH, tag="dBu", name=f"dBu{lid}_{p}")
        for i, s in ((0, s0), (1, s1)):
            bps = bc_mm("bc", s, f"bps{lid}_{s}")
            nc.scalar.activation(bcrep[:, i * T:(i + 1) * T], bps[:], AF.Copy)
            nc.vector.tensor_mul(dBu[:, i * T:(i + 1) * T], dx[:],
                                 bcrep[:, i * T:(i + 1) * T])
        hs = sl.tile([DI, 2 * T], FH, tag="hs", name=f"hs{lid}_{p}")
        nc.vector.tensor_tensor_scan(hs[:], dA[:], dBu[:], 0.0,
                                     AX.mult, AX.add)
        ccrep = sl.tile([DI, 2 * T], FH, tag="ccrep", name=f"crep{lid}_{p}")
        for i, s in ((0, s0), (1, s1)):
            cps = bc_mm("bc", DS + s, f"cps{lid}_{s}")
            nc.scalar.activation(ccrep[:, i * T:(i + 1) * T], cps[:], AF.Copy)
        if p == 1:
            nc.vector.tensor_mul(yP[:], hs[:], ccrep[:])
        else:
            hsc = sl.tile([DI, 2 * T], FH, tag="hsc", name=f"hsc{lid}_{p}")
            nc.vector.tensor_mul(hsc[:], hs[:], ccrep[:])
            nc.vector.tensor_add(yP[:], yP[:], hsc[:])

    # s = 15
    dA15 = big.tile([DI, T], FH, tag="dAs2", name=f"dA{lid}_s15")
    nc.scalar.activation(dA15[:], delta[:], AF.Exp, scale=Acols[:, 15:16])
    bps15 = bc_mm("bc", 15, f"bps{lid}_15")
    brep15 = big.tile([DI, T], FH, tag="dBus2", name=f"brep{lid}_15")
    nc.scalar.activation(brep15[:], bps15[:], AF.Copy)
    dBu15 = big.tile([DI, T], FH, tag="dBuf", name=f"dBu{lid}_15")
    nc.vector.tensor_mul(dBu15[:], dx[:], brep15[:])
    hs15 = big.tile([DI, T], FH, tag="hss2", name=f"hs{lid}_s15")
    nc.vector.tensor_tensor_scan(hs15[:], dA15[:], dBu15[:], 0.0,
                                 AX.mult, AX.add)
    cps15 = bc_mm("bc", DS + 15, f"cps{lid}_15")
    crep15 = big.tile([DI, T], FH, tag="creps", name=f"crep{lid}_15")
    nc.scalar.activation(crep15[:], cps15[:], AF.Copy)
    hsc15 = big.tile([DI, T], FH, tag="hscs", name=f"hsc{lid}_15")
    nc.vector.tensor_mul(hsc15[:], hs15[:], crep15[:])
    nc.vector.tensor_add(ysn[:], ysn[:], hsc15[:])

    # ---- z-proj late (keeps the Act head short; silu set reloads once) ----
    zs = big.tile([DI, T], FH, tag="zs", name=f"zs{lid}")
    mmz = ps.tile([DI, T], FP, tag="bc", name=f"mmz{lid}")
    for c in range(NCH):
        o = c * 512
        nc.tensor.matmul(mmz[:, o:o + 512], zT, upo[:, o + 2:o + 514],
                         start=True, stop=True)
    nc.scalar.activation(zs[:], mmz[:], AF.Silu)
    xsz = big.tile([DI, T], FH, tag="xsz", name=f"xsz{lid}")
    nc.vector.tensor_mul(xsz[:], xact[:], zs[:])

    # ---- y = (sum_s hs*C)*silu(z); out = out_w@y + (out_w*D)@(x*silu(z)) ----
    yf = big.tile([DI, T], FH, tag="yf", name=f"yf{lid}")
    nc.vector.tensor_add(yf[:], yP[:, 0:T], yP[:, T:2 * T])
    nc.vector.tensor_add(yf[:], yf[:], ysn[:])
    nc.vector.tensor_mul(yf[:], yf[:], zs[:])

    mmo = ps.tile([DI, T], FP, tag="bc", name=f"mmo{lid}")
    for c in range(NCH):
        o = c * 512
        nc.tensor.matmul(mmo[0:DM, o:o + 512], outDT, xsz[:, o:o + 512],
                         start=True, stop=False)
        nc.tensor.matmul(mmo[0:DM, o:o + 512], outT, yf[:, o:o + 512],
                         start=False, stop=True)
    # chunked output copies: half h feeds the next layer's half-h head ops
    for h in range(2):
        src = mmo[0:DM, h * H:(h + 1) * H]
        for j, (tl, off) in enumerate(out_specs):
            dst = tl[:, off + h * H:off + (h + 1) * H]
            if j == 0:
                nc.scalar.activation(dst, src, AF.Copy)
            else:
                nc.vector.tensor_copy(dst, src)
        if out_dma is not None:
            nc.sync.dma_start(out_dma[:, h * H:(h + 1) * H],
                              out_specs[0][0][:, out_specs[0][1] + h * H:
                                              out_specs[0][1] + (h + 1) * H])


def _build_kernel(ctx, tc, u0, u0o, hblobs, fblobs, outs):
    nc = tc.nc
    const = ctx.enter_context(tc.tile_pool(name="const", bufs=1))
    big = ctx.enter_context(tc.tile_pool(name="big", bufs=1))
    sl = ctx.enter_context(tc.tile_pool(name="sl", bufs=2))
    ps = ctx.enter_context(tc.tile_pool(name="ps", bufs=2, space="PSUM"))

    hb = [const.tile([DI, _HBLOB_W], FH, tag=f"hb{l}", name=f"hb{l}")
          for l in range(NL)]
    fb = [const.tile([DI, 20], FP, tag=f"fb{l}", name=f"fb{l}")
          for l in range(NL)]
    upA = const.tile([DM, UF], FH, tag="upA", name="upA")
    upAo = const.tile([DM, UF], FH, tag="upAo", name="upAo")
    nc.sync.dma_start(hb[0][:], hblobs[0][:])
    nc.sync.dma_start(upA[:], u0[:])
    nc.sync.dma_start(upAo[:], u0o[:])
    nc.sync.dma_start(fb[0][:], fblobs[0][:])
    nc.sync.dma_start(hb[1][:], hblobs[1][:])
    nc.sync.dma_start(fb[1][:], fblobs[1][:])
    upB = const.tile([DM, UF], FH, tag="upB", name="upB")
    upBo = const.tile([DM, UF], FH, tag="upBo", name="upBo")
    nc.gpsimd.memset(upB[:, 0:K - 1], 0.0)
    nc.gpsimd.memset(upB[:, UF - 1:UF], 0.0)
    nc.gpsimd.memset(upBo[:, 0:K - 2], 0.0)
    nc.gpsimd.memset(upBo[:, UF - 2:UF], 0.0)
    o2 = const.tile([DM, T], FH, tag="o2", name="o2")


    zcol = const.tile([DI, 1], FH, tag="zcol", name="zcol")
    nc.gpsimd.memset(zcol[:], 0.0)
    ones = const.tile([1, 2], FH, tag="ones", name="ones")
    nc.gpsimd.memset(ones[:], 1.0)

    pools = (const, big, sl, ps, {"lid": 0, "zcol": zcol, "ones": ones})
    # layer 1: outputs go to upB[:, 3:3+T] and upBo[:, 2:2+T]
    _build_layer(nc, pools, hb[0], fb[0], upA, upAo,
                 [(upB, K - 1), (upBo, K - 2)], outs[0])
    pools = (const, big, sl, ps, {"lid": 1, "zcol": zcol, "ones": ones})
    _build_layer(nc, pools, hb[1], fb[1], upB, upBo, [(o2, 0)], outs[1])


def build_program():
    nc = bacc.Bacc("TRN2", target_bir_lowering=False, debug=False)
    u0 = nc.dram_tensor("u0", [DM, UF], FH, kind="ExternalInput").ap()
    u0o = nc.dram_tensor("u0o", [DM, UF], FH, kind="ExternalInput").ap()
    hblobs = [nc.dram_tensor(f"hblob{l}", [DI, _HBLOB_W], FH,
                             kind="ExternalInput").ap() for l in range(NL)]
    fblobs = [nc.dram_tensor(f"fblob{l}", [DI, 20], FP,
                             kind="ExternalInput").ap() for l in range(NL)]
    outs = [nc.dram_tensor(f"o{l + 1}T", [DM, T], FH,
                           kind="ExternalOutput").ap() for l in range(NL)]
    with tile.TileContext(nc) as tc:
        with ExitStack() as ctx:
            _build_kernel(ctx, tc, u0, u0o, hblobs, fblobs, outs)
    nc.compile()
    return nc


_PROG = None


def _get_prog():
    global _PROG
    if _PROG is None:
        _PROG = build_program()
    return _PROG


def _pad_u(u):
    """u: (64, T) f32 -> (u_pad, u_pad_odd) fp16 (64, UF)."""
    up = np.zeros((DM, UF), np.float16)
    up[:, K - 1:K - 1 + T] = u.astype(np.float16)
    upo = np.zeros((DM, UF), np.float16)
    upo[:, 0:UF - 1] = up[:, 1:UF]
    return up, upo


def _run_launch(u_list_T, raw, trace=False, trace_kwargs=None):
    """u_list_T: list of 8 arrays (64, 2048) f32. raw: param dict (np).
    Returns (o1_list, o2_list, res) with (64, 2048) fp16 outputs."""
    nc = _get_prog()
    blobs = [_pack_blobs(raw, l) for l in range(NL)]
    in_maps = []
    for b in range(8):
        up, upo = _pad_u(np.asarray(u_list_T[b], np.float32))
        in_maps.append({
            "u0": up, "u0o": upo,
            "hblob0": blobs[0][0], "fblob0": blobs[0][1],
            "hblob1": blobs[1][0], "fblob1": blobs[1][1],
        })
    res = bass_utils.run_bass_kernel_spmd(
        nc, in_maps, core_ids=list(range(8)), trace=trace,
        **(trace_kwargs or {}))
    o1 = [res.results[b]["o1T"] for b in range(8)]
    o2 = [res.results[b]["o2T"] for b in range(8)]
    return o1, o2, res


def kernel(**inputs):
    inp = {k: np.asarray(v, np.float32) for k, v in inputs.items()}
    Ms = inp["Ms_feature"]
    Pan = inp["Pan_feature"]
    h = C // 2
    names = ("in_w", "conv_w", "conv_b", "xp_w", "dt_w", "dt_b",
             "A_log", "D", "out_w")
    rawa = {n: inp["a_" + n] for n in names}
    rawb = {n: inp["b_" + n] for n in names}

    cf1 = np.concatenate([Ms[:, :h], Pan[:, h:]], axis=1)
    cf2 = np.concatenate([Pan[:, :h], Ms[:, h:]], axis=1)
    u_list = [cf1[b].T for b in range(B)] + [cf2[b].T for b in range(B)]
    o1, o2, _ = _run_launch(u_list, rawa)
    cf1_1 = np.stack([o1[b].T.astype(np.float32) for b in range(B)])
    cf2_1 = np.stack([o1[B + b].T.astype(np.float32) for b in range(B)])
    cf1_2 = np.stack([o2[b].T.astype(np.float32) for b in range(B)])
    cf2_2 = np.stack([o2[B + b].T.astype(np.float32) for b in range(B)])
    Ms1 = np.maximum((cf1_1 + cf2_1) * 0.5 + Ms, 0.0)
    Ms2 = np.maximum((cf1_2 + cf2_2) * 0.5 + Ms1, 0.0)

    cf3 = np.stack([Pan[:, ::2], Ms2[:, 1::2]], axis=2).reshape(B, C, DM)
    cf4 = np.stack([Ms2[:, ::2], Pan[:, 1::2]], axis=2).reshape(B, C, DM)
    u_list = [cf3[b].T for b in range(B)] + [cf4[b].T for b in range(B)]
    o1, o2, _ = _run_launch(u_list, rawb)
    cf3_1 = np.stack([o1[b].T.astype(np.float32) for b in range(B)])
    cf4_1 = np.stack([o1[B + b].T.astype(np.float32) for b in range(B)])
    cf3_2 = np.stack([o2[b].T.astype(np.float32) for b in range(B)])
    cf4_2 = np.stack([o2[B + b].T.astype(np.float32) for b in range(B)])
    Pan1 = np.maximum((cf3_1 + cf4_1) * 0.5 + Pan, 0.0)
    Pan2 = np.maximum((cf3_2 + cf4_2) * 0.5 + Pan1, 0.0)
    return Ms2, Pan2


# revision 21
# speedup vs baseline: 1.1896x; 1.0108x over previous
"""Trainium2 Bass kernel for nn_CMCI_Mamba.

Strategy: data-parallel over the 2B=8 mamba streams (1 sequence per core).
Each launch runs 2 chained mamba layers fully on-chip in d-major layout
(features on partitions, time on the free axis).

Engine assignment (per layer):
- PE (fp16): in_proj with the causal conv FOLDED IN (4 shifted matmuls with
  host-prescaled weights diag(conv_w_k) @ in_w), z-proj, fused
  dt_w@xp_w[dt] projection, 32 stride-0 B/C broadcast matmuls, out_proj.
- Act: Silu(conv) / Silu(z) straight from PSUM, softplus via Exp+Ln (one
  table set), the 16 per-state dA = exp(A_s * delta) passes, PSUM->SBUF
  fp16 copies of the B/C broadcasts, layer-output copies.
- DVE: the 16 SSM scans (tensor_tensor_scan, batched 2 states per
  instruction with a zeroed dA column resetting the carry), dBu muls,
  half of the hs*C muls/accumulation.
- GPSIMD: the other half of the hs*C muls and y accumulation.

Host does the cheap cross-stream elementwise combines between launches.
"""
import sys
import numpy as np
from contextlib import ExitStack

for _p in ("/opt/trn_rl_repo",):
    if _p not in sys.path:
        sys.path.insert(0, _p)

import concourse.bass as bass
import concourse.bacc as bacc
import concourse.tile as tile
from concourse import mybir
from concourse import bass_utils

T, DM, DI, DS, DR, K, NL = 2048, 64, 128, 16, 4, 4, 2
B, C = 4, 2048
UF = T + K  # padded u width (2052)
FP = mybir.dt.float32
FH = mybir.dt.float16
AX = mybir.AluOpType
AF = mybir.ActivationFunctionType

# fp16 param blob column layout, (128, 1024) per layer
_B_WK = 0       # [0:64, 0:512]    4x conv-scaled in_proj-x lhsT (64,128) each
_B_Z = 512      # [0:64, 512:640]  z lhsT
_B_WD = 640     # [:, 640:768]     (dt_w @ xp_w[:DR]) lhsT
_B_BC = 768     # [:, 768:800]     B/C projection columns (32)
_B_OUT = 800    # [:, 800:864]     out_proj lhsT
_B_OUTD = 864   # [:, 864:928]     out_proj lhsT with D folded (for x*sz term)
_B_EYE = 928    # [0:32, 928:960]  eye(32) one-hot selectors for row broadcast
_HBLOB_W = 1024
# fp32 blob (128, 20): [:, 0:16]=A (=-exp(A_log)), 16=conv_b, 17=dt_b, 18=D


def _pack_blobs(raw, l):
    hb = np.zeros((DI, _HBLOB_W), np.float16)
    in_w = raw["in_w"][l]          # (256, 64)
    conv_w = raw["conv_w"][l]      # (128, 4)
    for k in range(K):
        wk = in_w[:DI] * conv_w[:, k:k + 1]          # (128, 64)
        hb[:DM, _B_WK + 128 * k:_B_WK + 128 * (k + 1)] = wk.T
    hb[:DM, _B_Z:_B_Z + DI] = in_w[DI:2 * DI].T
    wd = raw["dt_w"][l] @ raw["xp_w"][l][:DR]        # (128, 128)
    hb[:, _B_WD:_B_WD + DI] = wd.T
    hb[:, _B_BC:_B_BC + 2 * DS] = raw["xp_w"][l][DR:DR + 2 * DS].T
    hb[:, _B_OUT:_B_OUT + DM] = raw["out_w"][l].T
    # out_proj with D folded in: out += (out_w * D) @ (x * silu(z))
    hb[:, _B_OUTD:_B_OUTD + DM] = (raw["out_w"][l] * raw["D"][l]).T
    hb[0:2 * DS, _B_EYE:_B_EYE + 2 * DS] = np.eye(2 * DS, dtype=np.float16)
    fb = np.zeros((DI, 20), np.float32)
    fb[:, 0:DS] = -np.exp(raw["A_log"][l])
    fb[:, 16] = raw["conv_b"][l]
    fb[:, 17] = raw["dt_b"][l]
    fb[:, 18] = raw["D"][l]
    return hb, fb


def _build_layer(nc, pools, hb, fb, up, upo, out_specs, out_dma):
    """One mamba layer. up/upo: (64, UF) fp16 padded input (+1-shifted copy).
    out_specs: list of (tile, col_off) -- the (64, T) layer output is copied
    (in halves, on Act) into tile[:, off:off+T]. out_dma: DRAM ap or None.
    """
    const, big, sl, ps, gl = pools
    NCH = T // 512
    H = T // 2
    lid = gl["lid"]

    wkT = [hb[0:DM, _B_WK + 128 * k:_B_WK + 128 * (k + 1)] for k in range(K)]
    zT = hb[0:DM, _B_Z:_B_Z + DI]
    wdT = hb[:, _B_WD:_B_WD + DI]
    outT = hb[:, _B_OUT:_B_OUT + DM]
    outDT = hb[:, _B_OUTD:_B_OUTD + DM]
    Acols = fb[:, 0:DS]
    convb = fb[:, 16:17]
    dtb = fb[:, 17:18]

    def bc_mm(tag, col, name, direct=False):
        """Row-broadcast matmul. Default: one-hot selector over the
        precomputed B/C rows (4x fewer active MACs than the stride-0
        re-projection - this kernel runs power-throttled). direct=True
        re-projects from xact (used where waiting for bcr would stall)."""
        t = ps.tile([DI, T], FP, tag="bc", name=name)
        if direct:
            w = hb[:, _B_BC + col:_B_BC + col + 1].broadcast_to((DI, DI))
            rhs, np_ = xact, DI
        else:
            w = hb[0:2 * DS, _B_EYE + col:_B_EYE + col + 1].broadcast_to(
                (2 * DS, DI))
            rhs, np_ = bcr, 2 * DS
        for c in range(NCH):
            nc.tensor.matmul(t[:, c * 512:(c + 1) * 512], w,
                             rhs[0:np_, c * 512:(c + 1) * 512] if not direct
                             else rhs[:, c * 512:(c + 1) * 512],
                             start=True, stop=True)
        return t

    # ---- in_proj-x with folded causal conv -> silu -> xact (fp16) ----
    # xc[:, t] = sum_k (diag(conv_w_k) @ in_w_x) @ u[:, t-3+k]; tap k reads
    # u_pad[:, c*512+k:]; odd k uses the 1-shifted copy so every rhs offset
    # stays 4B-aligned.  Silu is applied per half so the delta chain starts
    # as soon as the first half lands.
    xact = big.tile([DI, T], FH, tag="xact", name=f"xact{lid}")
    for h in range(2):
        mmx = ps.tile([DI, H], FP, tag="bc", name=f"mmx{lid}_{h}")
        for c in (2 * h, 2 * h + 1):
            o = c * 512
            cs = slice(o - h * H, o - h * H + 512)
            nc.tensor.matmul(mmx[:, cs], wkT[0], up[:, o:o + 512],
                             start=True, stop=False)
            nc.tensor.matmul(mmx[:, cs], wkT[1], upo[:, o:o + 512],
                             start=False, stop=False)
            nc.tensor.matmul(mmx[:, cs], wkT[2], up[:, o + 2:o + 514],
                             start=False, stop=False)
            nc.tensor.matmul(mmx[:, cs], wkT[3], upo[:, o + 2:o + 514],
                             start=False, stop=True)
        nc.scalar.activation(xact[:, h * H:(h + 1) * H], mmx[:], AF.Silu,
                             bias=convb)

    # ---- delta = softplus(dt_proj + dt_b) via Exp then Ln(1+x), halves ----
    delta = big.tile([DI, T], FH, tag="delta", name=f"delta{lid}")
    ev = big.tile([DI, T], FH, tag="ev", name=f"ev{lid}")
    dx = big.tile([DI, T], FH, tag="dx", name=f"dx{lid}")
    for h in range(2):
        mmd = ps.tile([DI, H], FP, tag="bc", name=f"mmd{lid}_{h}")
        for c in (2 * h, 2 * h + 1):
            o = c * 512
            nc.tensor.matmul(mmd[:, o - h * H:o - h * H + 512], wdT,
                             xact[:, o:o + 512], start=True, stop=True)
        nc.scalar.activation(ev[:, h * H:(h + 1) * H], mmd[:], AF.Exp,
                             bias=dtb)
    for h in range(2):
        hs_ = slice(h * H, (h + 1) * H)
        nc.scalar.activation(delta[:, hs_], ev[:, hs_], AF.Ln, bias=1.0)
        nc.vector.tensor_mul(dx[:, hs_], delta[:, hs_], xact[:, hs_])

    # ---- s-loop: single s=0 first (via stride-0 direct broadcasts and a
    # half-chained scan, so the first scan starts during the Act ramp),
    # then 7 pairs off precomputed B/C rows, then single s=15 ----
    ysn = big.tile([DI, T], FH, tag="ysn", name=f"ysn{lid}")
    yP = big.tile([DI, 2 * T], FH, tag="yP", name=f"yP{lid}")

    dA0 = big.tile([DI, T], FH, tag="dAs", name=f"dA{lid}_s0")
    dBu0 = big.tile([DI, T], FH, tag="dBus", name=f"dBu{lid}_s0")
    hs0 = big.tile([DI, T], FH, tag="hss", name=f"hs{lid}_s0")
    bps0 = ps.tile([DI, T], FP, tag="bc", name=f"bps{lid}_0")
    bw = hb[:, _B_BC:_B_BC + 1].broadcast_to((DI, DI))
    for h in range(2):
        hh = slice(h * H, (h + 1) * H)
        nc.scalar.activation(dA0[:, hh], delta[:, hh], AF.Exp,
                             scale=Acols[:, 0:1])
        for c in (2 * h, 2 * h + 1):
            nc.tensor.matmul(bps0[:, c * 512:(c + 1) * 512], bw,
                             xact[:, c * 512:(c + 1) * 512],
                             start=True, stop=True)
        nc.vector.tensor_mul(dBu0[:, hh], dx[:, hh], bps0[:, hh])
        nc.vector.tensor_tensor_scan(
            hs0[:, hh], dA0[:, hh], dBu0[:, hh],
            0.0 if h == 0 else hs0[:, H - 1:H], AX.mult, AX.add)
    cw = hb[:, _B_BC + DS:_B_BC + DS + 1].broadcast_to((DI, DI))
    cps0 = ps.tile([DI, T], FP, tag="bc", name=f"cps{lid}_0")
    for c in range(NCH):
        nc.tensor.matmul(cps0[:, c * 512:(c + 1) * 512], cw,
                         xact[:, c * 512:(c + 1) * 512], start=True, stop=True)
    nc.vector.tensor_mul(ysn[:], hs0[:], cps0[:])


    # pairs (1,2) .. (13,14); pair 1 uses the stride-0 direct broadcast so
    # it doesn't wait for the bcr rows (emitted after it, below)
    def pair(p):
        s0, s1 = 2 * p - 1, 2 * p
        dA = sl.tile([DI, 2 * T], FH, tag="dA", name=f"dA{lid}_{p}")
        nc.scalar.activation(dA[:, 0:T], delta[:], AF.Exp,
                             scale=Acols[:, s0:s0 + 1])
        nc.scalar.activation(dA[:, T:2 * T], delta[:], AF.Exp,
                             scale=Acols[:, s1:s1 + 1])
        # zero the boundary column so the scan carry resets between states
        nc.scalar.activation(dA[:, T:T + 1], gl["zcol"][:], AF.Copy)
        bcrep = sl.tile([DI, 2 * T], FH, tag="bcrep", name=f"brep{lid}_{p}")
        dBu = sl.tile([DI, 2 * T], FH, tag="dBu", name=f"dBu{lid}_{p}")
        for i, s in ((0, s0), (1, s1)):
            bps = bc_mm("bc", s, f"bps{lid}_{s}", direct=(p == 1))
            nc.scalar.activation(bcrep[:, i * T:(i + 1) * T], bps[:], AF.Copy)
            nc.vector.tensor_mul(dBu[:, i * T:(i + 1) * T], dx[:],
                                 bcrep[:, i * T:(i + 1) * T])
        hs = sl.tile([DI, 2 * T], FH, tag="hs", name=f"hs{lid}_{p}")
        nc.vector.tensor_tensor_scan(hs[:], dA[:], dBu[:], 0.0,
                                     AX.mult, AX.add)
        ccrep = sl.tile([DI, 2 * T], FH, tag="ccrep", name=f"crep{lid}_{p}")
        for i, s in ((0, s0), (1, s1)):
            cps = bc_mm("bc", DS + s, f"cps{lid}_{s}", direct=(p == 1))
            nc.scalar.activation(ccrep[:, i * T:(i + 1) * T], cps[:], AF.Copy)
        if p == 1:
            nc.vector.tensor_mul(yP[:], hs[:], ccrep[:])
        else:
            hsc = sl.tile([DI, 2 * T], FH, tag="hsc", name=f"hsc{lid}_{p}")
            nc.vector.tensor_mul(hsc[:], hs[:], ccrep[:])
            nc.vector.tensor_add(yP[:], yP[:], hsc[:])

    pair(1)

    # ---- B/C projection rows (32, T) for pairs 2..7 ----
    bcr = big.tile([2 * DS, T], FH, tag="bcr", name=f"bcr{lid}")
    mmb = ps.tile([DI, T], FP, tag="bc", name=f"mmb{lid}")
    for c in range(NCH):
        o = c * 512
        nc.tensor.matmul(mmb[0:2 * DS, o:o + 512], hb[:, _B_BC:_B_BC + 2 * DS],
                         xact[:, o:o + 512], start=True, stop=True)
    nc.scalar.activation(bcr[:], mmb[0:2 * DS, :], AF.Copy)
    for p in range(2, 8):
        pair(p)

    # s = 15
    dA15 = big.tile([DI, T], FH, tag="dAs2", name=f"dA{lid}_s15")
    nc.scalar.activation(dA15[:], delta[:], AF.Exp, scale=Acols[:, 15:16])
    bps15 = bc_mm("bc", 15, f"bps{lid}_15")
    brep15 = big.tile([DI, T], FH, tag="dBus2", name=f"brep{lid}_15")
    nc.scalar.activation(brep15[:], bps15[:], AF.Copy)
    dBu15 = big.tile([DI, T], FH, tag="dBuf", name=f"dBu{lid}_15")
    nc.vector.tensor_mul(dBu15[:], dx[:], brep15[:])
    hs15 = big.tile([DI, T], FH, tag="hss2", name=f"hs{lid}_s15")
    nc.vector.tensor_tensor_scan(hs15[:], dA15[:], dBu15[:], 0.0,
                                 AX.mult, AX.add)
    cps15 = bc_mm("bc", DS + 15, f"cps{lid}_15")
    crep15 = big.tile([DI, T], FH, tag="creps", name=f"crep{lid}_15")
    nc.scalar.activation(crep15[:], cps15[:], AF.Copy)
    hsc15 = big.tile([DI, T], FH, tag="hscs", name=f"hsc{lid}_15")
    nc.vector.tensor_mul(hsc15[:], hs15[:], crep15[:])
    nc.vector.tensor_add(ysn[:], ysn[:], hsc15[:])

    # ---- z-proj late (keeps the Act head short; silu set reloads once) ----
    zs = big.tile([DI, T], FH, tag="zs", name=f"zs{lid}")
    mmz = ps.tile([DI, T], FP, tag="bc", name=f"mmz{lid}")
    for c in range(NCH):
        o = c * 512
        nc.tensor.matmul(mmz[:, o:o + 512], zT, upo[:, o + 2:o + 514],
                         start=True, stop=True)
    nc.scalar.activation(zs[:], mmz[:], AF.Silu)
    xsz = big.tile([DI, T], FH, tag="xsz", name=f"xsz{lid}")
    nc.vector.tensor_mul(xsz[:], xact[:], zs[:])

    # ---- y = (sum_s hs*C)*silu(z); out = out_w@y + (out_w*D)@(x*silu(z)) ----
    yf = big.tile([DI, T], FH, tag="yf", name=f"yf{lid}")
    nc.vector.tensor_add(yf[:], yP[:, 0:T], yP[:, T:2 * T])
    nc.vector.tensor_add(yf[:], yf[:], ysn[:])
    nc.vector.tensor_mul(yf[:], yf[:], zs[:])

    mmo = ps.tile([DI, T], FP, tag="bc", name=f"mmo{lid}")
    for c in range(NCH):
        o = c * 512
        nc.tensor.matmul(mmo[0:DM, o:o + 512], outDT, xsz[:, o:o + 512],
                         start=True, stop=False)
        nc.tensor.matmul(mmo[0:DM, o:o + 512], outT, yf[:, o:o + 512],
                         start=False, stop=True)
    # chunked output copies: chunk q feeds the next layer's head ops / DMA
    NQ = 2 if len(out_specs) > 1 else 4
    Q = T // NQ
    for q in range(NQ):
        src = mmo[0:DM, q * Q:(q + 1) * Q]
        for j, (tl, off) in enumerate(out_specs):
            dst = tl[:, off + q * Q:off + (q + 1) * Q]
            if j == 0:
                nc.scalar.activation(dst, src, AF.Copy)
            else:
                nc.vector.tensor_copy(dst, src)
        if out_dma is not None:
            nc.sync.dma_start(out_dma[:, q * Q:(q + 1) * Q],
                              out_specs[0][0][:, out_specs[0][1] + q * Q:
                                              out_specs[0][1] + (q + 1) * Q])


def _build_kernel(ctx, tc, u0, u0o, hblobs, fblobs, outs):
    nc = tc.nc
    const = ctx.enter_context(tc.tile_pool(name="const", bufs=1))
    big = ctx.enter_context(tc.tile_pool(name="big", bufs=1))
    sl = ctx.enter_context(tc.tile_pool(name="sl", bufs=2))
    ps = ctx.enter_context(tc.tile_pool(name="ps", bufs=2, space="PSUM"))

    hb = [const.tile([DI, _HBLOB_W], FH, tag=f"hb{l}", name=f"hb{l}")
          for l in range(NL)]
    fb = [const.tile([DI, 20], FP, tag=f"fb{l}", name=f"fb{l}")
          for l in range(NL)]
    upA = const.tile([DM, UF], FH, tag="upA", name="upA")
    upAo = const.tile([DM, UF], FH, tag="upAo", name="upAo")
    nc.sync.dma_start(hb[0][:], hblobs[0][:])
    nc.sync.dma_start(upA[:], u0[:])
    nc.sync.dma_start(upAo[:], u0o[:])
    nc.sync.dma_start(fb[0][:], fblobs[0][:])
    nc.sync.dma_start(hb[1][:], hblobs[1][:])
    nc.sync.dma_start(fb[1][:], fblobs[1][:])
    upB = const.tile([DM, UF], FH, tag="upB", name="upB")
    upBo = const.tile([DM, UF], FH, tag="upBo", name="upBo")
    nc.gpsimd.memset(upB[:, 0:K - 1], 0.0)
    nc.gpsimd.memset(upB[:, UF - 1:UF], 0.0)
    nc.gpsimd.memset(upBo[:, 0:K - 2], 0.0)
    nc.gpsimd.memset(upBo[:, UF - 2:UF], 0.0)
    o2 = const.tile([DM, T], FH, tag="o2", name="o2")


    zcol = const.tile([DI, 1], FH, tag="zcol", name="zcol")
    nc.gpsimd.memset(zcol[:], 0.0)
    ones = const.tile([1, 2], FH, tag="ones", name="ones")
    nc.gpsimd.memset(ones[:], 1.0)

    pools = (const, big, sl, ps, {"lid": 0, "zcol": zcol, "ones": ones})
    # layer 1: outputs go to upB[:, 3:3+T] and upBo[:, 2:2+T]
    _build_layer(nc, pools, hb[0], fb[0], upA, upAo,
                 [(upB, K - 1), (upBo, K - 2)], outs[0])
    pools = (const, big, sl, ps, {"lid": 1, "zcol": zcol, "ones": ones})
    _build_layer(nc, pools, hb[1], fb[1], upB, upBo, [(o2, 0)], outs[1])


def build_program():
    nc = bacc.Bacc("TRN2", target_bir_lowering=False, debug=False)
    u0 = nc.dram_tensor("u0", [DM, UF], FH, kind="ExternalInput").ap()
    u0o = nc.dram_tensor("u0o", [DM, UF], FH, kind="ExternalInput").ap()
    hblobs = [nc.dram_tensor(f"hblob{l}", [DI, _HBLOB_W], FH,
                             kind="ExternalInput").ap() for l in range(NL)]
    fblobs = [nc.dram_tensor(f"fblob{l}", [DI, 20], FP,
                             kind="ExternalInput").ap() for l in range(NL)]
    outs = [nc.dram_tensor(f"o{l + 1}T", [DM, T], FH,
                           kind="ExternalOutput").ap() for l in range(NL)]
    with tile.TileContext(nc) as tc:
        with ExitStack() as ctx:
            _build_kernel(ctx, tc, u0, u0o, hblobs, fblobs, outs)
    nc.compile()
    return nc


_PROG = None


def _get_prog():
    global _PROG
    if _PROG is None:
        _PROG = build_program()
    return _PROG


def _pad_u(u):
    """u: (64, T) f32 -> (u_pad, u_pad_odd) fp16 (64, UF)."""
    up = np.zeros((DM, UF), np.float16)
    up[:, K - 1:K - 1 + T] = u.astype(np.float16)
    upo = np.zeros((DM, UF), np.float16)
    upo[:, 0:UF - 1] = up[:, 1:UF]
    return up, upo


def _run_launch(u_list_T, raw, trace=False, trace_kwargs=None):
    """u_list_T: list of 8 arrays (64, 2048) f32. raw: param dict (np).
    Returns (o1_list, o2_list, res) with (64, 2048) fp16 outputs."""
    nc = _get_prog()
    blobs = [_pack_blobs(raw, l) for l in range(NL)]
    in_maps = []
    for b in range(8):
        up, upo = _pad_u(np.asarray(u_list_T[b], np.float32))
        in_maps.append({
            "u0": up, "u0o": upo,
            "hblob0": blobs[0][0], "fblob0": blobs[0][1],
            "hblob1": blobs[1][0], "fblob1": blobs[1][1],
        })
    res = bass_utils.run_bass_kernel_spmd(
        nc, in_maps, core_ids=list(range(8)), trace=trace,
        **(trace_kwargs or {}))
    o1 = [res.results[b]["o1T"] for b in range(8)]
    o2 = [res.results[b]["o2T"] for b in range(8)]
    return o1, o2, res


def kernel(**inputs):
    inp = {k: np.asarray(v, np.float32) for k, v in inputs.items()}
    Ms = inp["Ms_feature"]
    Pan = inp["Pan_feature"]
    h = C // 2
    names = ("in_w", "conv_w", "conv_b", "xp_w", "dt_w", "dt_b",
             "A_log", "D", "out_w")
    rawa = {n: inp["a_" + n] for n in names}
    rawb = {n: inp["b_" + n] for n in names}

    cf1 = np.concatenate([Ms[:, :h], Pan[:, h:]], axis=1)
    cf2 = np.concatenate([Pan[:, :h], Ms[:, h:]], axis=1)
    u_list = [cf1[b].T for b in range(B)] + [cf2[b].T for b in range(B)]
    o1, o2, _ = _run_launch(u_list, rawa)
    cf1_1 = np.stack([o1[b].T.astype(np.float32) for b in range(B)])
    cf2_1 = np.stack([o1[B + b].T.astype(np.float32) for b in range(B)])
    cf1_2 = np.stack([o2[b].T.astype(np.float32) for b in range(B)])
    cf2_2 = np.stack([o2[B + b].T.astype(np.float32) for b in range(B)])
    Ms1 = np.maximum((cf1_1 + cf2_1) * 0.5 + Ms, 0.0)
    Ms2 = np.maximum((cf1_2 + cf2_2) * 0.5 + Ms1, 0.0)

    cf3 = np.stack([Pan[:, ::2], Ms2[:, 1::2]], axis=2).reshape(B, C, DM)
    cf4 = np.stack([Ms2[:, ::2], Pan[:, 1::2]], axis=2).reshape(B, C, DM)
    u_list = [cf3[b].T for b in range(B)] + [cf4[b].T for b in range(B)]
    o1, o2, _ = _run_launch(u_list, rawb)
    cf3_1 = np.stack([o1[b].T.astype(np.float32) for b in range(B)])
    cf4_1 = np.stack([o1[B + b].T.astype(np.float32) for b in range(B)])
    cf3_2 = np.stack([o2[b].T.astype(np.float32) for b in range(B)])
    cf4_2 = np.stack([o2[B + b].T.astype(np.float32) for b in range(B)])
    Pan1 = np.maximum((cf3_1 + cf4_1) * 0.5 + Pan, 0.0)
    Pan2 = np.maximum((cf3_2 + cf4_2) * 0.5 + Pan1, 0.0)
    return Ms2, Pan2


# revision 22
# speedup vs baseline: 1.1989x; 1.0078x over previous
"""Trainium2 Bass kernel for nn_CMCI_Mamba.

Strategy: data-parallel over the 2B=8 mamba streams (1 sequence per core).
Each launch runs 2 chained mamba layers fully on-chip in d-major layout
(features on partitions, time on the free axis).

Engine assignment (per layer):
- PE (fp16): in_proj with the causal conv FOLDED IN (4 shifted matmuls with
  host-prescaled weights diag(conv_w_k) @ in_w), z-proj, fused
  dt_w@xp_w[dt] projection, 32 stride-0 B/C broadcast matmuls, out_proj.
- Act: Silu(conv) / Silu(z) straight from PSUM, softplus via Exp+Ln (one
  table set), the 16 per-state dA = exp(A_s * delta) passes, PSUM->SBUF
  fp16 copies of the B/C broadcasts, layer-output copies.
- DVE: the 16 SSM scans (tensor_tensor_scan, batched 2 states per
  instruction with a zeroed dA column resetting the carry), all dBu and
  hs*C muls and the y accumulation.  GPSIMD is deliberately idle: its
  tensor ops starve the DVE's shared SBUF port (measured 7x slowdown on
  concurrent DVE tensor_tensor).

Host does the cheap cross-stream elementwise combines between launches.
"""
import sys
import numpy as np
from contextlib import ExitStack

for _p in ("/opt/trn_rl_repo",):
    if _p not in sys.path:
        sys.path.insert(0, _p)

import concourse.bass as bass
import concourse.bacc as bacc
import concourse.tile as tile
from concourse import mybir
from concourse import bass_utils

T, DM, DI, DS, DR, K, NL = 2048, 64, 128, 16, 4, 4, 2
B, C = 4, 2048
UF = T + K  # padded u width (2052)
FP = mybir.dt.float32
FH = mybir.dt.float16
AX = mybir.AluOpType
AF = mybir.ActivationFunctionType

# fp16 param blob column layout, (128, 1024) per layer
_B_WK = 0       # [0:64, 0:512]    4x conv-scaled in_proj-x lhsT (64,128) each
_B_Z = 512      # [0:64, 512:640]  z lhsT
_B_WD = 640     # [:, 640:768]     (dt_w @ xp_w[:DR]) lhsT
_B_BC = 768     # [:, 768:800]     B/C projection columns (32)
_B_OUT = 800    # [:, 800:864]     out_proj lhsT
_B_OUTD = 864   # [:, 864:928]     out_proj lhsT with D folded (for x*sz term)
_B_EYE = 928    # [0:32, 928:960]  eye(32) one-hot selectors for row broadcast
_HBLOB_W = 1024
# fp32 blob (128, 20): [:, 0:16]=A (=-exp(A_log)), 16=conv_b, 17=dt_b, 18=D


def _pack_blobs(raw, l):
    hb = np.zeros((DI, _HBLOB_W), np.float16)
    in_w = raw["in_w"][l]          # (256, 64)
    conv_w = raw["conv_w"][l]      # (128, 4)
    for k in range(K):
        wk = in_w[:DI] * conv_w[:, k:k + 1]          # (128, 64)
        hb[:DM, _B_WK + 128 * k:_B_WK + 128 * (k + 1)] = wk.T
    hb[:DM, _B_Z:_B_Z + DI] = in_w[DI:2 * DI].T
    wd = raw["dt_w"][l] @ raw["xp_w"][l][:DR]        # (128, 128)
    hb[:, _B_WD:_B_WD + DI] = wd.T
    hb[:, _B_BC:_B_BC + 2 * DS] = raw["xp_w"][l][DR:DR + 2 * DS].T
    hb[:, _B_OUT:_B_OUT + DM] = raw["out_w"][l].T
    # out_proj with D folded in: out += (out_w * D) @ (x * silu(z))
    hb[:, _B_OUTD:_B_OUTD + DM] = (raw["out_w"][l] * raw["D"][l]).T
    hb[0:2 * DS, _B_EYE:_B_EYE + 2 * DS] = np.eye(2 * DS, dtype=np.float16)
    fb = np.zeros((DI, 20), np.float32)
    fb[:, 0:DS] = -np.exp(raw["A_log"][l])
    fb[:, 16] = raw["conv_b"][l]
    fb[:, 17] = raw["dt_b"][l]
    fb[:, 18] = raw["D"][l]
    return hb, fb


def _build_layer(nc, pools, hb, fb, up, upo, out_specs, out_dma):
    """One mamba layer. up/upo: (64, UF) fp16 padded input (+1-shifted copy).
    out_specs: list of (tile, col_off) -- the (64, T) layer output is copied
    (in halves, on Act) into tile[:, off:off+T]. out_dma: DRAM ap or None.
    """
    const, big, sl, ps, gl = pools
    NCH = T // 512
    H = T // 2
    lid = gl["lid"]

    wkT = [hb[0:DM, _B_WK + 128 * k:_B_WK + 128 * (k + 1)] for k in range(K)]
    zT = hb[0:DM, _B_Z:_B_Z + DI]
    wdT = hb[:, _B_WD:_B_WD + DI]
    outT = hb[:, _B_OUT:_B_OUT + DM]
    outDT = hb[:, _B_OUTD:_B_OUTD + DM]
    Acols = fb[:, 0:DS]
    convb = fb[:, 16:17]
    dtb = fb[:, 17:18]

    def bc_mm(tag, col, name, direct=False):
        """Row-broadcast matmul. Default: one-hot selector over the
        precomputed B/C rows (4x fewer active MACs than the stride-0
        re-projection - this kernel runs power-throttled). direct=True
        re-projects from xact (used where waiting for bcr would stall)."""
        t = ps.tile([DI, T], FP, tag="bc", name=name)
        if direct:
            w = hb[:, _B_BC + col:_B_BC + col + 1].broadcast_to((DI, DI))
            rhs, np_ = xact, DI
        else:
            w = hb[0:2 * DS, _B_EYE + col:_B_EYE + col + 1].broadcast_to(
                (2 * DS, DI))
            rhs, np_ = bcr, 2 * DS
        for c in range(NCH):
            nc.tensor.matmul(t[:, c * 512:(c + 1) * 512], w,
                             rhs[0:np_, c * 512:(c + 1) * 512] if not direct
                             else rhs[:, c * 512:(c + 1) * 512],
                             start=True, stop=True)
        return t

    # ---- in_proj-x with folded causal conv -> silu -> xact (fp16) ----
    # xc[:, t] = sum_k (diag(conv_w_k) @ in_w_x) @ u[:, t-3+k]; tap k reads
    # u_pad[:, c*512+k:]; odd k uses the 1-shifted copy so every rhs offset
    # stays 4B-aligned.  Silu is applied per half so the delta chain starts
    # as soon as the first half lands.
    xact = big.tile([DI, T], FH, tag="xact", name=f"xact{lid}")
    for h in range(2):
        mmx = ps.tile([DI, H], FP, tag="bc", name=f"mmx{lid}_{h}")
        for c in (2 * h, 2 * h + 1):
            o = c * 512
            cs = slice(o - h * H, o - h * H + 512)
            nc.tensor.matmul(mmx[:, cs], wkT[0], up[:, o:o + 512],
                             start=True, stop=False)
            nc.tensor.matmul(mmx[:, cs], wkT[1], upo[:, o:o + 512],
                             start=False, stop=False)
            nc.tensor.matmul(mmx[:, cs], wkT[2], up[:, o + 2:o + 514],
                             start=False, stop=False)
            nc.tensor.matmul(mmx[:, cs], wkT[3], upo[:, o + 2:o + 514],
                             start=False, stop=True)
        nc.scalar.activation(xact[:, h * H:(h + 1) * H], mmx[:], AF.Silu,
                             bias=convb)

    # ---- delta = softplus(dt_proj + dt_b) via Exp then Ln(1+x), halves ----
    delta = big.tile([DI, T], FH, tag="delta", name=f"delta{lid}")
    ev = big.tile([DI, T], FH, tag="ev", name=f"ev{lid}")
    dx = big.tile([DI, T], FH, tag="dx", name=f"dx{lid}")
    for h in range(2):
        mmd = ps.tile([DI, H], FP, tag="bc", name=f"mmd{lid}_{h}")
        for c in (2 * h, 2 * h + 1):
            o = c * 512
            nc.tensor.matmul(mmd[:, o - h * H:o - h * H + 512], wdT,
                             xact[:, o:o + 512], start=True, stop=True)
        nc.scalar.activation(ev[:, h * H:(h + 1) * H], mmd[:], AF.Exp,
                             bias=dtb)
    for h in range(2):
        hs_ = slice(h * H, (h + 1) * H)
        nc.scalar.activation(delta[:, hs_], ev[:, hs_], AF.Ln, bias=1.0)
        nc.vector.tensor_mul(dx[:, hs_], delta[:, hs_], xact[:, hs_])

    # ---- s-loop: single s=0 first (via stride-0 direct broadcasts and a
    # half-chained scan, so the first scan starts during the Act ramp),
    # then 7 pairs off precomputed B/C rows, then single s=15 ----
    ysn = big.tile([DI, T], FH, tag="ysn", name=f"ysn{lid}")
    yP = big.tile([DI, 2 * T], FH, tag="yP", name=f"yP{lid}")

    dA0 = big.tile([DI, T], FH, tag="dAs", name=f"dA{lid}_s0")
    dBu0 = big.tile([DI, T], FH, tag="dBus", name=f"dBu{lid}_s0")
    hs0 = big.tile([DI, T], FH, tag="hss", name=f"hs{lid}_s0")
    bps0 = ps.tile([DI, T], FP, tag="bc", name=f"bps{lid}_0")
    bw = hb[:, _B_BC:_B_BC + 1].broadcast_to((DI, DI))
    for h in range(2):
        hh = slice(h * H, (h + 1) * H)
        nc.scalar.activation(dA0[:, hh], delta[:, hh], AF.Exp,
                             scale=Acols[:, 0:1])
        for c in (2 * h, 2 * h + 1):
            nc.tensor.matmul(bps0[:, c * 512:(c + 1) * 512], bw,
                             xact[:, c * 512:(c + 1) * 512],
                             start=True, stop=True)
        nc.vector.tensor_mul(dBu0[:, hh], dx[:, hh], bps0[:, hh])
        nc.vector.tensor_tensor_scan(
            hs0[:, hh], dA0[:, hh], dBu0[:, hh],
            0.0 if h == 0 else hs0[:, H - 1:H], AX.mult, AX.add)
    cw = hb[:, _B_BC + DS:_B_BC + DS + 1].broadcast_to((DI, DI))
    cps0 = ps.tile([DI, T], FP, tag="bc", name=f"cps{lid}_0")
    for c in range(NCH):
        nc.tensor.matmul(cps0[:, c * 512:(c + 1) * 512], cw,
                         xact[:, c * 512:(c + 1) * 512], start=True, stop=True)
    nc.vector.tensor_mul(ysn[:], hs0[:], cps0[:])


    # pairs (1,2) .. (13,14); pair 1 uses the stride-0 direct broadcast so
    # it doesn't wait for the bcr rows (emitted after it, below)
    def pair(p):
        s0, s1 = 2 * p - 1, 2 * p
        dA = sl.tile([DI, 2 * T], FH, tag="dA", name=f"dA{lid}_{p}")
        nc.scalar.activation(dA[:, 0:T], delta[:], AF.Exp,
                             scale=Acols[:, s0:s0 + 1])
        nc.scalar.activation(dA[:, T:2 * T], delta[:], AF.Exp,
                             scale=Acols[:, s1:s1 + 1])
        # zero the boundary column so the scan carry resets between states
        nc.scalar.activation(dA[:, T:T + 1], gl["zcol"][:], AF.Copy)
        bcrep = sl.tile([DI, 2 * T], FH, tag="bcrep", name=f"brep{lid}_{p}")
        dBu = sl.tile([DI, 2 * T], FH, tag="dBu", name=f"dBu{lid}_{p}")
        for i, s in ((0, s0), (1, s1)):
            bps = bc_mm("bc", s, f"bps{lid}_{s}", direct=(p == 1))
            nc.scalar.activation(bcrep[:, i * T:(i + 1) * T], bps[:], AF.Copy)
            nc.vector.tensor_mul(dBu[:, i * T:(i + 1) * T], dx[:],
                                 bcrep[:, i * T:(i + 1) * T])
        hs = sl.tile([DI, 2 * T], FH, tag="hs", name=f"hs{lid}_{p}")
        nc.vector.tensor_tensor_scan(hs[:], dA[:], dBu[:], 0.0,
                                     AX.mult, AX.add)
        ccrep = sl.tile([DI, 2 * T], FH, tag="ccrep", name=f"crep{lid}_{p}")
        for i, s in ((0, s0), (1, s1)):
            cps = bc_mm("bc", DS + s, f"cps{lid}_{s}", direct=(p == 1))
            nc.scalar.activation(ccrep[:, i * T:(i + 1) * T], cps[:], AF.Copy)
        if p == 1:
            nc.vector.tensor_mul(yP[:], hs[:], ccrep[:])
        else:
            hsc = sl.tile([DI, 2 * T], FH, tag="hsc", name=f"hsc{lid}_{p}")
            nc.vector.tensor_mul(hsc[:], hs[:], ccrep[:])
            nc.vector.tensor_add(yP[:], yP[:], hsc[:])

    pair(1)

    # ---- B/C projection rows (32, T) for pairs 2..7 ----
    bcr = big.tile([2 * DS, T], FH, tag="bcr", name=f"bcr{lid}")
    mmb = ps.tile([DI, T], FP, tag="bc", name=f"mmb{lid}")
    for c in range(NCH):
        o = c * 512
        nc.tensor.matmul(mmb[0:2 * DS, o:o + 512], hb[:, _B_BC:_B_BC + 2 * DS],
                         xact[:, o:o + 512], start=True, stop=True)
    nc.scalar.activation(bcr[:], mmb[0:2 * DS, :], AF.Copy)
    for p in range(2, 8):
        pair(p)

    # s = 15
    dA15 = big.tile([DI, T], FH, tag="dAs2", name=f"dA{lid}_s15")
    nc.scalar.activation(dA15[:], delta[:], AF.Exp, scale=Acols[:, 15:16])
    bps15 = bc_mm("bc", 15, f"bps{lid}_15")
    brep15 = big.tile([DI, T], FH, tag="dBus2", name=f"brep{lid}_15")
    nc.scalar.activation(brep15[:], bps15[:], AF.Copy)
    dBu15 = big.tile([DI, T], FH, tag="dBuf", name=f"dBu{lid}_15")
    nc.vector.tensor_mul(dBu15[:], dx[:], brep15[:])
    hs15 = big.tile([DI, T], FH, tag="hss2", name=f"hs{lid}_s15")
    nc.vector.tensor_tensor_scan(hs15[:], dA15[:], dBu15[:], 0.0,
                                 AX.mult, AX.add)
    cps15 = bc_mm("bc", DS + 15, f"cps{lid}_15")
    crep15 = big.tile([DI, T], FH, tag="creps", name=f"crep{lid}_15")
    nc.scalar.activation(crep15[:], cps15[:], AF.Copy)
    hsc15 = big.tile([DI, T], FH, tag="hscs", name=f"hsc{lid}_15")
    nc.vector.tensor_mul(hsc15[:], hs15[:], crep15[:])
    nc.vector.tensor_add(ysn[:], ysn[:], hsc15[:])

    # ---- z-proj late (keeps the Act head short; silu set reloads once) ----
    zs = big.tile([DI, T], FH, tag="zs", name=f"zs{lid}")
    mmz = ps.tile([DI, T], FP, tag="bc", name=f"mmz{lid}")
    for c in range(NCH):
        o = c * 512
        nc.tensor.matmul(mmz[:, o:o + 512], zT, upo[:, o + 2:o + 514],
                         start=True, stop=True)
    nc.scalar.activation(zs[:], mmz[:], AF.Silu)
    xsz = big.tile([DI, T], FH, tag="xsz", name=f"xsz{lid}")
    nc.vector.tensor_mul(xsz[:], xact[:], zs[:])

    # ---- y = (sum_s hs*C)*silu(z); out = out_w@y + (out_w*D)@(x*silu(z)) ----
    yf = big.tile([DI, T], FH, tag="yf", name=f"yf{lid}")
    for q in range(4):
        qq = slice(q * 512, (q + 1) * 512)
        qT = slice(T + q * 512, T + (q + 1) * 512)
        nc.vector.tensor_add(yf[:, qq], yP[:, qq], yP[:, qT])
        nc.vector.tensor_add(yf[:, qq], yf[:, qq], ysn[:, qq])
        nc.vector.tensor_mul(yf[:, qq], yf[:, qq], zs[:, qq])

    mmo = ps.tile([DI, T], FP, tag="bc", name=f"mmo{lid}")
    for c in range(NCH):
        o = c * 512
        nc.tensor.matmul(mmo[0:DM, o:o + 512], outDT, xsz[:, o:o + 512],
                         start=True, stop=False)
        nc.tensor.matmul(mmo[0:DM, o:o + 512], outT, yf[:, o:o + 512],
                         start=False, stop=True)
    # chunked output copies: chunk q feeds the next layer's head ops / DMA
    NQ = 2 if len(out_specs) > 1 else 4
    Q = T // NQ
    for q in range(NQ):
        src = mmo[0:DM, q * Q:(q + 1) * Q]
        for j, (tl, off) in enumerate(out_specs):
            dst = tl[:, off + q * Q:off + (q + 1) * Q]
            if j == 0:
                nc.scalar.activation(dst, src, AF.Copy)
            else:
                nc.vector.tensor_copy(dst, src)
        if out_dma is not None:
            nc.sync.dma_start(out_dma[:, q * Q:(q + 1) * Q],
                              out_specs[0][0][:, out_specs[0][1] + q * Q:
                                              out_specs[0][1] + (q + 1) * Q])


def _build_kernel(ctx, tc, u0, u0o, hblobs, fblobs, outs):
    nc = tc.nc
    const = ctx.enter_context(tc.tile_pool(name="const", bufs=1))
    big = ctx.enter_context(tc.tile_pool(name="big", bufs=1))
    sl = ctx.enter_context(tc.tile_pool(name="sl", bufs=2))
    ps = ctx.enter_context(tc.tile_pool(name="ps", bufs=2, space="PSUM"))

    hb = [const.tile([DI, _HBLOB_W], FH, tag=f"hb{l}", name=f"hb{l}")
          for l in range(NL)]
    fb = [const.tile([DI, 20], FP, tag=f"fb{l}", name=f"fb{l}")
          for l in range(NL)]
    upA = const.tile([DM, UF], FH, tag="upA", name="upA")
    upAo = const.tile([DM, UF], FH, tag="upAo", name="upAo")
    nc.sync.dma_start(hb[0][:], hblobs[0][:])
    nc.sync.dma_start(upA[:], u0[:])
    nc.sync.dma_start(upAo[:], u0o[:])
    nc.sync.dma_start(fb[0][:], fblobs[0][:])
    nc.sync.dma_start(hb[1][:], hblobs[1][:])
    nc.sync.dma_start(fb[1][:], fblobs[1][:])
    upB = const.tile([DM, UF], FH, tag="upB", name="upB")
    upBo = const.tile([DM, UF], FH, tag="upBo", name="upBo")
    nc.gpsimd.memset(upB[:, 0:K - 1], 0.0)
    nc.gpsimd.memset(upB[:, UF - 1:UF], 0.0)
    nc.gpsimd.memset(upBo[:, 0:K - 2], 0.0)
    nc.gpsimd.memset(upBo[:, UF - 2:UF], 0.0)
    o2 = const.tile([DM, T], FH, tag="o2", name="o2")


    zcol = const.tile([DI, 1], FH, tag="zcol", name="zcol")
    nc.gpsimd.memset(zcol[:], 0.0)
    ones = const.tile([1, 2], FH, tag="ones", name="ones")
    nc.gpsimd.memset(ones[:], 1.0)

    pools = (const, big, sl, ps, {"lid": 0, "zcol": zcol, "ones": ones})
    # layer 1: outputs go to upB[:, 3:3+T] and upBo[:, 2:2+T]
    _build_layer(nc, pools, hb[0], fb[0], upA, upAo,
                 [(upB, K - 1), (upBo, K - 2)], outs[0])
    pools = (const, big, sl, ps, {"lid": 1, "zcol": zcol, "ones": ones})
    _build_layer(nc, pools, hb[1], fb[1], upB, upBo, [(o2, 0)], outs[1])


def build_program():
    nc = bacc.Bacc("TRN2", target_bir_lowering=False, debug=False)
    u0 = nc.dram_tensor("u0", [DM, UF], FH, kind="ExternalInput").ap()
    u0o = nc.dram_tensor("u0o", [DM, UF], FH, kind="ExternalInput").ap()
    hblobs = [nc.dram_tensor(f"hblob{l}", [DI, _HBLOB_W], FH,
                             kind="ExternalInput").ap() for l in range(NL)]
    fblobs = [nc.dram_tensor(f"fblob{l}", [DI, 20], FP,
                             kind="ExternalInput").ap() for l in range(NL)]
    outs = [nc.dram_tensor(f"o{l + 1}T", [DM, T], FH,
                           kind="ExternalOutput").ap() for l in range(NL)]
    with tile.TileContext(nc) as tc:
        with ExitStack() as ctx:
            _build_kernel(ctx, tc, u0, u0o, hblobs, fblobs, outs)
    nc.compile()
    return nc


_PROG = None


def _get_prog():
    global _PROG
    if _PROG is None:
        _PROG = build_program()
    return _PROG


def _pad_u(u):
    """u: (64, T) f32 -> (u_pad, u_pad_odd) fp16 (64, UF)."""
    up = np.zeros((DM, UF), np.float16)
    up[:, K - 1:K - 1 + T] = u.astype(np.float16)
    upo = np.zeros((DM, UF), np.float16)
    upo[:, 0:UF - 1] = up[:, 1:UF]
    return up, upo


def _run_launch(u_list_T, raw, trace=False, trace_kwargs=None):
    """u_list_T: list of 8 arrays (64, 2048) f32. raw: param dict (np).
    Returns (o1_list, o2_list, res) with (64, 2048) fp16 outputs."""
    nc = _get_prog()
    blobs = [_pack_blobs(raw, l) for l in range(NL)]
    in_maps = []
    for b in range(8):
        up, upo = _pad_u(np.asarray(u_list_T[b], np.float32))
        in_maps.append({
            "u0": up, "u0o": upo,
            "hblob0": blobs[0][0], "fblob0": blobs[0][1],
            "hblob1": blobs[1][0], "fblob1": blobs[1][1],
        })
    res = bass_utils.run_bass_kernel_spmd(
        nc, in_maps, core_ids=list(range(8)), trace=trace,
        **(trace_kwargs or {}))
    o1 = [res.results[b]["o1T"] for b in range(8)]
    o2 = [res.results[b]["o2T"] for b in range(8)]
    return o1, o2, res


def kernel(**inputs):
    inp = {k: np.asarray(v, np.float32) for k, v in inputs.items()}
    Ms = inp["Ms_feature"]
    Pan = inp["Pan_feature"]
    h = C // 2
    names = ("in_w", "conv_w", "conv_b", "xp_w", "dt_w", "dt_b",
             "A_log", "D", "out_w")
    rawa = {n: inp["a_" + n] for n in names}
    rawb = {n: inp["b_" + n] for n in names}

    cf1 = np.concatenate([Ms[:, :h], Pan[:, h:]], axis=1)
    cf2 = np.concatenate([Pan[:, :h], Ms[:, h:]], axis=1)
    u_list = [cf1[b].T for b in range(B)] + [cf2[b].T for b in range(B)]
    o1, o2, _ = _run_launch(u_list, rawa)
    cf1_1 = np.stack([o1[b].T.astype(np.float32) for b in range(B)])
    cf2_1 = np.stack([o1[B + b].T.astype(np.float32) for b in range(B)])
    cf1_2 = np.stack([o2[b].T.astype(np.float32) for b in range(B)])
    cf2_2 = np.stack([o2[B + b].T.astype(np.float32) for b in range(B)])
    Ms1 = np.maximum((cf1_1 + cf2_1) * 0.5 + Ms, 0.0)
    Ms2 = np.maximum((cf1_2 + cf2_2) * 0.5 + Ms1, 0.0)

    cf3 = np.stack([Pan[:, ::2], Ms2[:, 1::2]], axis=2).reshape(B, C, DM)
    cf4 = np.stack([Ms2[:, ::2], Pan[:, 1::2]], axis=2).reshape(B, C, DM)
    u_list = [cf3[b].T for b in range(B)] + [cf4[b].T for b in range(B)]
    o1, o2, _ = _run_launch(u_list, rawb)
    cf3_1 = np.stack([o1[b].T.astype(np.float32) for b in range(B)])
    cf4_1 = np.stack([o1[B + b].T.astype(np.float32) for b in range(B)])
    cf3_2 = np.stack([o2[b].T.astype(np.float32) for b in range(B)])
    cf4_2 = np.stack([o2[B + b].T.astype(np.float32) for b in range(B)])
    Pan1 = np.maximum((cf3_1 + cf4_1) * 0.5 + Pan, 0.0)
    Pan2 = np.maximum((cf3_2 + cf4_2) * 0.5 + Pan1, 0.0)
    return Ms2, Pan2


# revision 23
# speedup vs baseline: 1.1997x; 1.0006x over previous
"""Trainium2 Bass kernel for nn_CMCI_Mamba.

Strategy: data-parallel over the 2B=8 mamba streams (1 sequence per core).
Each launch runs 2 chained mamba layers fully on-chip in d-major layout
(features on partitions, time on the free axis).

Engine assignment (per layer):
- PE (fp16): in_proj with the causal conv FOLDED IN (4 shifted matmuls with
  host-prescaled weights diag(conv_w_k) @ in_w), z-proj, fused
  dt_w@xp_w[dt] projection, 32 stride-0 B/C broadcast matmuls, out_proj.
- Act: Silu(conv) / Silu(z) straight from PSUM, softplus via Exp+Ln (one
  table set), the 16 per-state dA = exp(A_s * delta) passes, PSUM->SBUF
  fp16 copies of the B/C broadcasts, layer-output copies.
- DVE: the 16 SSM scans (tensor_tensor_scan, batched 2 states per
  instruction with a zeroed dA column resetting the carry), all dBu and
  hs*C muls and the y accumulation.  GPSIMD is deliberately idle: its
  tensor ops starve the DVE's shared SBUF port (measured 7x slowdown on
  concurrent DVE tensor_tensor).

Host does the cheap cross-stream elementwise combines between launches.
"""
import sys
import numpy as np
from contextlib import ExitStack

for _p in ("/opt/trn_rl_repo",):
    if _p not in sys.path:
        sys.path.insert(0, _p)

import concourse.bass as bass
import concourse.bacc as bacc
import concourse.tile as tile
from concourse import mybir
from concourse import bass_utils

T, DM, DI, DS, DR, K, NL = 2048, 64, 128, 16, 4, 4, 2
B, C = 4, 2048
UF = T + K  # padded u width (2052)
FP = mybir.dt.float32
FH = mybir.dt.float16
AX = mybir.AluOpType
AF = mybir.ActivationFunctionType

# fp16 param blob column layout, (128, 1024) per layer
_B_WK = 0       # [0:64, 0:512]    4x conv-scaled in_proj-x lhsT (64,128) each
_B_Z = 512      # [0:64, 512:640]  z lhsT
_B_WD = 640     # [:, 640:768]     (dt_w @ xp_w[:DR]) lhsT
_B_BC = 768     # [:, 768:800]     B/C projection columns (32)
_B_OUT = 800    # [:, 800:864]     out_proj lhsT
_B_OUTD = 864   # [:, 864:928]     out_proj lhsT with D folded (for x*sz term)
_B_EYE = 928    # [0:32, 928:960]  eye(32) one-hot selectors for row broadcast
_HBLOB_W = 1024
# fp32 blob (128, 20): [:, 0:16]=A (=-exp(A_log)), 16=conv_b, 17=dt_b, 18=D


def _pack_blobs(raw, l):
    hb = np.zeros((DI, _HBLOB_W), np.float16)
    in_w = raw["in_w"][l]          # (256, 64)
    conv_w = raw["conv_w"][l]      # (128, 4)
    for k in range(K):
        wk = in_w[:DI] * conv_w[:, k:k + 1]          # (128, 64)
        hb[:DM, _B_WK + 128 * k:_B_WK + 128 * (k + 1)] = wk.T
    hb[:DM, _B_Z:_B_Z + DI] = in_w[DI:2 * DI].T
    wd = raw["dt_w"][l] @ raw["xp_w"][l][:DR]        # (128, 128)
    hb[:, _B_WD:_B_WD + DI] = wd.T
    hb[:, _B_BC:_B_BC + 2 * DS] = raw["xp_w"][l][DR:DR + 2 * DS].T
    hb[:, _B_OUT:_B_OUT + DM] = raw["out_w"][l].T
    # out_proj with D folded in: out += (out_w * D) @ (x * silu(z))
    hb[:, _B_OUTD:_B_OUTD + DM] = (raw["out_w"][l] * raw["D"][l]).T
    hb[0:2 * DS, _B_EYE:_B_EYE + 2 * DS] = np.eye(2 * DS, dtype=np.float16)
    fb = np.zeros((DI, 20), np.float32)
    fb[:, 0:DS] = -np.exp(raw["A_log"][l])
    fb[:, 16] = raw["conv_b"][l]
    fb[:, 17] = raw["dt_b"][l]
    fb[:, 18] = raw["D"][l]
    return hb, fb


def _build_layer(nc, pools, hb, fb, up, upo, out_specs, out_dma):
    """One mamba layer. up/upo: (64, UF) fp16 padded input (+1-shifted copy).
    out_specs: list of (tile, col_off) -- the (64, T) layer output is copied
    (in halves, on Act) into tile[:, off:off+T]. out_dma: DRAM ap or None.
    """
    const, big, sl, ps, gl = pools
    NCH = T // 512
    H = T // 2
    lid = gl["lid"]

    wkT = [hb[0:DM, _B_WK + 128 * k:_B_WK + 128 * (k + 1)] for k in range(K)]
    zT = hb[0:DM, _B_Z:_B_Z + DI]
    wdT = hb[:, _B_WD:_B_WD + DI]
    outT = hb[:, _B_OUT:_B_OUT + DM]
    outDT = hb[:, _B_OUTD:_B_OUTD + DM]
    Acols = fb[:, 0:DS]
    convb = fb[:, 16:17]
    dtb = fb[:, 17:18]

    def bc_mm(tag, col, name, direct=False):
        """Row-broadcast matmul. Default: one-hot selector over the
        precomputed B/C rows (4x fewer active MACs than the stride-0
        re-projection - this kernel runs power-throttled). direct=True
        re-projects from xact (used where waiting for bcr would stall)."""
        t = ps.tile([DI, T], FP, tag="bc", name=name)
        if direct:
            w = hb[:, _B_BC + col:_B_BC + col + 1].broadcast_to((DI, DI))
            rhs, np_ = xact, DI
        else:
            w = hb[0:2 * DS, _B_EYE + col:_B_EYE + col + 1].broadcast_to(
                (2 * DS, DI))
            rhs, np_ = bcr, 2 * DS
        for c in range(NCH):
            nc.tensor.matmul(t[:, c * 512:(c + 1) * 512], w,
                             rhs[0:np_, c * 512:(c + 1) * 512] if not direct
                             else rhs[:, c * 512:(c + 1) * 512],
                             start=True, stop=True)
        return t

    # ---- in_proj-x with folded causal conv -> silu -> xact (fp16) ----
    # xc[:, t] = sum_k (diag(conv_w_k) @ in_w_x) @ u[:, t-3+k]; tap k reads
    # u_pad[:, c*512+k:]; odd k uses the 1-shifted copy so every rhs offset
    # stays 4B-aligned.  Silu is applied per half so the delta chain starts
    # as soon as the first half lands.
    xact = big.tile([DI, T], FH, tag="xact", name=f"xact{lid}")
    for h in range(2):
        mmx = ps.tile([DI, H], FP, tag="bc", name=f"mmx{lid}_{h}")
        for c in (2 * h, 2 * h + 1):
            o = c * 512
            cs = slice(o - h * H, o - h * H + 512)
            nc.tensor.matmul(mmx[:, cs], wkT[0], up[:, o:o + 512],
                             start=True, stop=False)
            nc.tensor.matmul(mmx[:, cs], wkT[1], upo[:, o:o + 512],
                             start=False, stop=False)
            nc.tensor.matmul(mmx[:, cs], wkT[2], up[:, o + 2:o + 514],
                             start=False, stop=False)
            nc.tensor.matmul(mmx[:, cs], wkT[3], upo[:, o + 2:o + 514],
                             start=False, stop=True)
        nc.scalar.activation(xact[:, h * H:(h + 1) * H], mmx[:], AF.Silu,
                             bias=convb)

    # ---- delta = softplus(dt_proj + dt_b) via Exp then Ln(1+x), halves ----
    delta = big.tile([DI, T], FH, tag="delta", name=f"delta{lid}")
    ev = big.tile([DI, T], FH, tag="ev", name=f"ev{lid}")
    dx = big.tile([DI, T], FH, tag="dx", name=f"dx{lid}")
    for h in range(2):
        mmd = ps.tile([DI, H], FP, tag="bc", name=f"mmd{lid}_{h}")
        for c in (2 * h, 2 * h + 1):
            o = c * 512
            nc.tensor.matmul(mmd[:, o - h * H:o - h * H + 512], wdT,
                             xact[:, o:o + 512], start=True, stop=True)
        nc.scalar.activation(ev[:, h * H:(h + 1) * H], mmd[:], AF.Exp,
                             bias=dtb)
    for h in range(2):
        hs_ = slice(h * H, (h + 1) * H)
        nc.scalar.activation(delta[:, hs_], ev[:, hs_], AF.Ln, bias=1.0)
        nc.vector.tensor_mul(dx[:, hs_], delta[:, hs_], xact[:, hs_])

    # ---- s-loop: single s=0 first (via stride-0 direct broadcasts and a
    # half-chained scan, so the first scan starts during the Act ramp),
    # then 7 pairs off precomputed B/C rows, then single s=15 ----
    ysn = big.tile([DI, T], FH, tag="ysn", name=f"ysn{lid}")
    yP = big.tile([DI, 2 * T], FH, tag="yP", name=f"yP{lid}")

    dA0 = big.tile([DI, T], FH, tag="dAs", name=f"dA{lid}_s0")
    dBu0 = big.tile([DI, T], FH, tag="dBus", name=f"dBu{lid}_s0")
    hs0 = big.tile([DI, T], FH, tag="hss", name=f"hs{lid}_s0")
    bps0 = ps.tile([DI, T], FP, tag="bc", name=f"bps{lid}_0")
    bw = hb[:, _B_BC:_B_BC + 1].broadcast_to((DI, DI))
    for h in range(2):
        hh = slice(h * H, (h + 1) * H)
        nc.scalar.activation(dA0[:, hh], delta[:, hh], AF.Exp,
                             scale=Acols[:, 0:1])
        for c in (2 * h, 2 * h + 1):
            nc.tensor.matmul(bps0[:, c * 512:(c + 1) * 512], bw,
                             xact[:, c * 512:(c + 1) * 512],
                             start=True, stop=True)
        nc.vector.tensor_mul(dBu0[:, hh], dx[:, hh], bps0[:, hh])
        nc.vector.tensor_tensor_scan(
            hs0[:, hh], dA0[:, hh], dBu0[:, hh],
            0.0 if h == 0 else hs0[:, H - 1:H], AX.mult, AX.add)
    cw = hb[:, _B_BC + DS:_B_BC + DS + 1].broadcast_to((DI, DI))
    cps0 = ps.tile([DI, T], FP, tag="bc", name=f"cps{lid}_0")
    for c in range(NCH):
        nc.tensor.matmul(cps0[:, c * 512:(c + 1) * 512], cw,
                         xact[:, c * 512:(c + 1) * 512], start=True, stop=True)
    nc.vector.tensor_mul(ysn[:], hs0[:], cps0[:])


    # pairs (1,2) .. (13,14); pair 1 uses the stride-0 direct broadcast so
    # it doesn't wait for the bcr rows (emitted after it, below)
    def pair(p):
        s0, s1 = 2 * p - 1, 2 * p
        dA = sl.tile([DI, 2 * T], FH, tag="dA", name=f"dA{lid}_{p}")
        nc.scalar.activation(dA[:, 0:T], delta[:], AF.Exp,
                             scale=Acols[:, s0:s0 + 1])
        nc.scalar.activation(dA[:, T:2 * T], delta[:], AF.Exp,
                             scale=Acols[:, s1:s1 + 1])
        # zero the boundary column so the scan carry resets between states
        nc.scalar.activation(dA[:, T:T + 1], gl["zcol"][:], AF.Copy)
        bcrep = sl.tile([DI, 2 * T], FH, tag="bcrep", name=f"brep{lid}_{p}")
        dBu = sl.tile([DI, 2 * T], FH, tag="dBu", name=f"dBu{lid}_{p}")
        for i, s in ((0, s0), (1, s1)):
            bps = bc_mm("bc", s, f"bps{lid}_{s}", direct=(p == 1))
            nc.scalar.activation(bcrep[:, i * T:(i + 1) * T], bps[:], AF.Copy)
            nc.vector.tensor_mul(dBu[:, i * T:(i + 1) * T], dx[:],
                                 bcrep[:, i * T:(i + 1) * T])
        hs = sl.tile([DI, 2 * T], FH, tag="hs", name=f"hs{lid}_{p}")
        nc.vector.tensor_tensor_scan(hs[:], dA[:], dBu[:], 0.0,
                                     AX.mult, AX.add)
        ccrep = sl.tile([DI, 2 * T], FH, tag="ccrep", name=f"crep{lid}_{p}")
        for i, s in ((0, s0), (1, s1)):
            cps = bc_mm("bc", DS + s, f"cps{lid}_{s}", direct=(p == 1))
            nc.scalar.activation(ccrep[:, i * T:(i + 1) * T], cps[:], AF.Copy)
        if p == 1:
            nc.vector.tensor_mul(yP[:], hs[:], ccrep[:])
        else:
            hsc = sl.tile([DI, 2 * T], FH, tag="hsc", name=f"hsc{lid}_{p}")
            nc.vector.tensor_mul(hsc[:], hs[:], ccrep[:])
            nc.vector.tensor_add(yP[:], yP[:], hsc[:])

    pair(1)

    # ---- B/C projection rows (32, T) for pairs 2..7 ----
    bcr = big.tile([2 * DS, T], FH, tag="bcr", name=f"bcr{lid}")
    mmb = ps.tile([DI, T], FP, tag="bc", name=f"mmb{lid}")
    for c in range(NCH):
        o = c * 512
        nc.tensor.matmul(mmb[0:2 * DS, o:o + 512], hb[:, _B_BC:_B_BC + 2 * DS],
                         xact[:, o:o + 512], start=True, stop=True)
    nc.scalar.activation(bcr[:], mmb[0:2 * DS, :], AF.Copy)
    for p in range(2, 8):
        pair(p)

    # s = 15
    dA15 = big.tile([DI, T], FH, tag="dAs2", name=f"dA{lid}_s15")
    nc.scalar.activation(dA15[:], delta[:], AF.Exp, scale=Acols[:, 15:16])
    bps15 = bc_mm("bc", 15, f"bps{lid}_15")
    brep15 = big.tile([DI, T], FH, tag="dBus2", name=f"brep{lid}_15")
    nc.scalar.activation(brep15[:], bps15[:], AF.Copy)
    dBu15 = big.tile([DI, T], FH, tag="dBuf", name=f"dBu{lid}_15")
    nc.vector.tensor_mul(dBu15[:], dx[:], brep15[:])
    hs15 = big.tile([DI, T], FH, tag="hss2", name=f"hs{lid}_s15")
    nc.vector.tensor_tensor_scan(hs15[:], dA15[:], dBu15[:], 0.0,
                                 AX.mult, AX.add)
    cps15 = bc_mm("bc", DS + 15, f"cps{lid}_15")
    crep15 = big.tile([DI, T], FH, tag="creps", name=f"crep{lid}_15")
    nc.scalar.activation(crep15[:], cps15[:], AF.Copy)
    hsc15 = big.tile([DI, T], FH, tag="hscs", name=f"hsc{lid}_15")
    nc.vector.tensor_mul(hsc15[:], hs15[:], crep15[:])
    nc.vector.tensor_add(ysn[:], ysn[:], hsc15[:])

    # ---- z-proj late (keeps the Act head short; silu set reloads once) ----
    zs = big.tile([DI, T], FH, tag="zs", name=f"zs{lid}")
    mmz = ps.tile([DI, T], FP, tag="bc", name=f"mmz{lid}")
    for c in range(NCH):
        o = c * 512
        nc.tensor.matmul(mmz[:, o:o + 512], zT, upo[:, o + 2:o + 514],
                         start=True, stop=True)
    nc.scalar.activation(zs[:], mmz[:], AF.Silu)
    xsz = big.tile([DI, T], FH, tag="xsz", name=f"xsz{lid}")
    nc.vector.tensor_mul(xsz[:], xact[:], zs[:])

    # ---- y = (sum_s hs*C)*silu(z); out = out_w@y + (out_w*D)@(x*silu(z)) ----
    yf = big.tile([DI, T], FH, tag="yf", name=f"yf{lid}")
    for q in range(4):
        qq = slice(q * 512, (q + 1) * 512)
        qT = slice(T + q * 512, T + (q + 1) * 512)
        nc.vector.tensor_add(yf[:, qq], yP[:, qq], yP[:, qT])
        nc.vector.tensor_add(yf[:, qq], yf[:, qq], ysn[:, qq])
        nc.vector.tensor_mul(yf[:, qq], yf[:, qq], zs[:, qq])

    mmo = ps.tile([DI, T], FP, tag="bc", name=f"mmo{lid}")
    for c in range(NCH):
        o = c * 512
        nc.tensor.matmul(mmo[0:DM, o:o + 512], outDT, xsz[:, o:o + 512],
                         start=True, stop=False)
        nc.tensor.matmul(mmo[0:DM, o:o + 512], outT, yf[:, o:o + 512],
                         start=False, stop=True)
    # chunked output copies: chunk q feeds the next layer's head ops / DMA
    NQ = 2 if len(out_specs) > 1 else 4
    Q = T // NQ
    for q in range(NQ):
        src = mmo[0:DM, q * Q:(q + 1) * Q]
        for j, (tl, off) in enumerate(out_specs):
            dst = tl[:, off + q * Q:off + (q + 1) * Q]
            if j == 0:
                nc.scalar.activation(dst, src, AF.Copy)
            else:
                nc.vector.tensor_copy(dst, src)
        if out_dma is not None:
            nc.sync.dma_start(out_dma[:, q * Q:(q + 1) * Q],
                              out_specs[0][0][:, out_specs[0][1] + q * Q:
                                              out_specs[0][1] + (q + 1) * Q])


def _build_kernel(ctx, tc, u0, u0o, hblobs, fblobs, outs):
    nc = tc.nc
    const = ctx.enter_context(tc.tile_pool(name="const", bufs=1))
    big = ctx.enter_context(tc.tile_pool(name="big", bufs=1))
    sl = ctx.enter_context(tc.tile_pool(name="sl", bufs=2))
    ps = ctx.enter_context(tc.tile_pool(name="ps", bufs=2, space="PSUM"))

    hb = [const.tile([DI, _HBLOB_W], FH, tag=f"hb{l}", name=f"hb{l}")
          for l in range(NL)]
    fb = [const.tile([DI, 20], FP, tag=f"fb{l}", name=f"fb{l}")
          for l in range(NL)]
    upA = const.tile([DM, UF], FH, tag="upA", name="upA")
    upAo = const.tile([DM, UF], FH, tag="upAo", name="upAo")
    nc.sync.dma_start(hb[0][:], hblobs[0][:])
    nc.sync.dma_start(upA[:], u0[:])
    nc.sync.dma_start(upAo[:], u0o[:])
    nc.sync.dma_start(fb[0][:], fblobs[0][:])
    nc.sync.dma_start(hb[1][:], hblobs[1][:])
    nc.sync.dma_start(fb[1][:], fblobs[1][:])
    upB = const.tile([DM, UF], FH, tag="upB", name="upB")
    upBo = const.tile([DM, UF], FH, tag="upBo", name="upBo")
    nc.gpsimd.memset(upB[:, 0:K - 1], 0.0)
    nc.gpsimd.memset(upB[:, UF - 1:UF], 0.0)
    nc.gpsimd.memset(upBo[:, 0:K - 2], 0.0)
    nc.gpsimd.memset(upBo[:, UF - 2:UF], 0.0)
    o2 = const.tile([DM, T], FH, tag="o2", name="o2")


    zcol = const.tile([DI, 1], FH, tag="zcol", name="zcol")
    nc.gpsimd.memset(zcol[:], 0.0)

    pools = (const, big, sl, ps, {"lid": 0, "zcol": zcol})
    # layer 1: outputs go to upB[:, 3:3+T] and upBo[:, 2:2+T]
    _build_layer(nc, pools, hb[0], fb[0], upA, upAo,
                 [(upB, K - 1), (upBo, K - 2)], outs[0])
    pools = (const, big, sl, ps, {"lid": 1, "zcol": zcol})
    _build_layer(nc, pools, hb[1], fb[1], upB, upBo, [(o2, 0)], outs[1])


def build_program():
    nc = bacc.Bacc("TRN2", target_bir_lowering=False, debug=False)
    u0 = nc.dram_tensor("u0", [DM, UF], FH, kind="ExternalInput").ap()
    u0o = nc.dram_tensor("u0o", [DM, UF], FH, kind="ExternalInput").ap()
    hblobs = [nc.dram_tensor(f"hblob{l}", [DI, _HBLOB_W], FH,
                             kind="ExternalInput").ap() for l in range(NL)]
    fblobs = [nc.dram_tensor(f"fblob{l}", [DI, 20], FP,
                             kind="ExternalInput").ap() for l in range(NL)]
    outs = [nc.dram_tensor(f"o{l + 1}T", [DM, T], FH,
                           kind="ExternalOutput").ap() for l in range(NL)]
    with tile.TileContext(nc) as tc:
        with ExitStack() as ctx:
            _build_kernel(ctx, tc, u0, u0o, hblobs, fblobs, outs)
    nc.compile()
    return nc


_PROG = None


def _get_prog():
    global _PROG
    if _PROG is None:
        _PROG = build_program()
    return _PROG


def _pad_u(u):
    """u: (64, T) f32 -> (u_pad, u_pad_odd) fp16 (64, UF)."""
    up = np.zeros((DM, UF), np.float16)
    up[:, K - 1:K - 1 + T] = u.astype(np.float16)
    upo = np.zeros((DM, UF), np.float16)
    upo[:, 0:UF - 1] = up[:, 1:UF]
    return up, upo


def _run_launch(u_list_T, raw, trace=False, trace_kwargs=None):
    """u_list_T: list of 8 arrays (64, 2048) f32. raw: param dict (np).
    Returns (o1_list, o2_list, res) with (64, 2048) fp16 outputs."""
    nc = _get_prog()
    blobs = [_pack_blobs(raw, l) for l in range(NL)]
    in_maps = []
    for b in range(8):
        up, upo = _pad_u(np.asarray(u_list_T[b], np.float32))
        in_maps.append({
            "u0": up, "u0o": upo,
            "hblob0": blobs[0][0], "fblob0": blobs[0][1],
            "hblob1": blobs[1][0], "fblob1": blobs[1][1],
        })
    res = bass_utils.run_bass_kernel_spmd(
        nc, in_maps, core_ids=list(range(8)), trace=trace,
        **(trace_kwargs or {}))
    o1 = [res.results[b]["o1T"] for b in range(8)]
    o2 = [res.results[b]["o2T"] for b in range(8)]
    return o1, o2, res


def kernel(**inputs):
    inp = {k: np.asarray(v, np.float32) for k, v in inputs.items()}
    Ms = inp["Ms_feature"]
    Pan = inp["Pan_feature"]
    h = C // 2
    names = ("in_w", "conv_w", "conv_b", "xp_w", "dt_w", "dt_b",
             "A_log", "D", "out_w")
    rawa = {n: inp["a_" + n] for n in names}
    rawb = {n: inp["b_" + n] for n in names}

    cf1 = np.concatenate([Ms[:, :h], Pan[:, h:]], axis=1)
    cf2 = np.concatenate([Pan[:, :h], Ms[:, h:]], axis=1)
    u_list = [cf1[b].T for b in range(B)] + [cf2[b].T for b in range(B)]
    o1, o2, _ = _run_launch(u_list, rawa)
    cf1_1 = np.stack([o1[b].T.astype(np.float32) for b in range(B)])
    cf2_1 = np.stack([o1[B + b].T.astype(np.float32) for b in range(B)])
    cf1_2 = np.stack([o2[b].T.astype(np.float32) for b in range(B)])
    cf2_2 = np.stack([o2[B + b].T.astype(np.float32) for b in range(B)])
    Ms1 = np.maximum((cf1_1 + cf2_1) * 0.5 + Ms, 0.0)
    Ms2 = np.maximum((cf1_2 + cf2_2) * 0.5 + Ms1, 0.0)

    cf3 = np.stack([Pan[:, ::2], Ms2[:, 1::2]], axis=2).reshape(B, C, DM)
    cf4 = np.stack([Ms2[:, ::2], Pan[:, 1::2]], axis=2).reshape(B, C, DM)
    u_list = [cf3[b].T for b in range(B)] + [cf4[b].T for b in range(B)]
    o1, o2, _ = _run_launch(u_list, rawb)
    cf3_1 = np.stack([o1[b].T.astype(np.float32) for b in range(B)])
    cf4_1 = np.stack([o1[B + b].T.astype(np.float32) for b in range(B)])
    cf3_2 = np.stack([o2[b].T.astype(np.float32) for b in range(B)])
    cf4_2 = np.stack([o2[B + b].T.astype(np.float32) for b in range(B)])
    Pan1 = np.maximum((cf3_1 + cf4_1) * 0.5 + Pan, 0.0)
    Pan2 = np.maximum((cf3_2 + cf4_2) * 0.5 + Pan1, 0.0)
    return Ms2, Pan2


# revision 25
# speedup vs baseline: 1.2070x; 1.0061x over previous
"""Trainium2 Bass kernel for nn_CMCI_Mamba.

Strategy: data-parallel over the 2B=8 mamba streams (1 sequence per core).
Each launch runs 2 chained mamba layers fully on-chip in d-major layout
(features on partitions, time on the free axis).

Engine assignment (per layer):
- PE (fp16): in_proj with the causal conv FOLDED IN (4 shifted matmuls with
  host-prescaled weights diag(conv_w_k) @ in_w), z-proj, fused
  dt_w@xp_w[dt] projection, 32 stride-0 B/C broadcast matmuls, out_proj.
- Act: Silu(conv) / Silu(z) straight from PSUM, softplus via Exp+Ln (one
  table set), the 16 per-state dA = exp(A_s * delta) passes, PSUM->SBUF
  fp16 copies of the B/C broadcasts, layer-output copies.
- DVE: the 16 SSM scans (tensor_tensor_scan, batched 2 states per
  instruction with a zeroed dA column resetting the carry), all dBu and
  hs*C muls and the y accumulation.  GPSIMD is deliberately idle: its
  tensor ops starve the DVE's shared SBUF port (measured 7x slowdown on
  concurrent DVE tensor_tensor).

Host does the cheap cross-stream elementwise combines between launches.
"""
import sys
import numpy as np
from contextlib import ExitStack

for _p in ("/opt/trn_rl_repo",):
    if _p not in sys.path:
        sys.path.insert(0, _p)

import concourse.bass as bass
import concourse.bacc as bacc
import concourse.tile as tile
from concourse import mybir
from concourse import bass_utils

T, DM, DI, DS, DR, K, NL = 2048, 64, 128, 16, 4, 4, 2
B, C = 4, 2048
UF = T + K  # padded u width (2052)
FP = mybir.dt.float32
FH = mybir.dt.float16
AX = mybir.AluOpType
AF = mybir.ActivationFunctionType

# fp16 param blob column layout, (128, 1024) per layer
_B_WK = 0       # [0:64, 0:512]    4x conv-scaled in_proj-x lhsT (64,128) each
_B_Z = 512      # [0:64, 512:640]  z lhsT
_B_WD = 640     # [:, 640:768]     (dt_w @ xp_w[:DR]) lhsT
_B_BC = 768     # [:, 768:800]     B/C projection columns (32)
_B_OUT = 800    # [:, 800:864]     out_proj lhsT
_B_OUTD = 864   # [:, 864:928]     out_proj lhsT with D folded (for x*sz term)
_B_EYE = 928    # [0:32, 928:960]  eye(32) one-hot selectors for row broadcast
_B_W02 = 960    # [:, 960:1088]    taps 0+2 stacked lhsT (contraction 128)
_B_W13 = 1088   # [:, 1088:1216]   taps 1+3 stacked lhsT
_HBLOB_W = 1280
# fp32 blob (128, 20): [:, 0:16]=A (=-exp(A_log)), 16=conv_b, 17=dt_b, 18=D


def _pack_blobs(raw, l):
    hb = np.zeros((DI, _HBLOB_W), np.float16)
    in_w = raw["in_w"][l]          # (256, 64)
    conv_w = raw["conv_w"][l]      # (128, 4)
    for k in range(K):
        wk = in_w[:DI] * conv_w[:, k:k + 1]          # (128, 64)
        hb[:DM, _B_WK + 128 * k:_B_WK + 128 * (k + 1)] = wk.T
    hb[:DM, _B_Z:_B_Z + DI] = in_w[DI:2 * DI].T
    wd = raw["dt_w"][l] @ raw["xp_w"][l][:DR]        # (128, 128)
    hb[:, _B_WD:_B_WD + DI] = wd.T
    hb[:, _B_BC:_B_BC + 2 * DS] = raw["xp_w"][l][DR:DR + 2 * DS].T
    hb[:, _B_OUT:_B_OUT + DM] = raw["out_w"][l].T
    # out_proj with D folded in: out += (out_w * D) @ (x * silu(z))
    hb[:, _B_OUTD:_B_OUTD + DM] = (raw["out_w"][l] * raw["D"][l]).T
    hb[0:2 * DS, _B_EYE:_B_EYE + 2 * DS] = np.eye(2 * DS, dtype=np.float16)
    # layer-1 ramp path: taps (0,2) and (1,3) stacked into c=128 matmuls
    for j, (ka, kb) in enumerate(((0, 2), (1, 3))):
        col = (_B_W02, _B_W13)[j]
        hb[0:DM, col:col + DI] = (in_w[:DI] * conv_w[:, ka:ka + 1]).T
        hb[DM:2 * DM, col:col + DI] = (in_w[:DI] * conv_w[:, kb:kb + 1]).T
    fb = np.zeros((DI, 20), np.float32)
    fb[:, 0:DS] = -np.exp(raw["A_log"][l])
    fb[:, 16] = raw["conv_b"][l]
    fb[:, 17] = raw["dt_b"][l]
    fb[:, 18] = raw["D"][l]
    return hb, fb


def _build_layer(nc, pools, hb, fb, up, upo, out_specs, out_dma):
    """One mamba layer. up/upo: (64, UF) fp16 padded input (+1-shifted copy).
    out_specs: list of (tile, col_off) -- the (64, T) layer output is copied
    (in halves, on Act) into tile[:, off:off+T]. out_dma: DRAM ap or None.
    """
    const, big, sl, ps, gl = pools
    NCH = T // 512
    H = T // 2
    lid = gl["lid"]

    wkT = [hb[0:DM, _B_WK + 128 * k:_B_WK + 128 * (k + 1)] for k in range(K)]
    zT = hb[0:DM, _B_Z:_B_Z + DI]
    wdT = hb[:, _B_WD:_B_WD + DI]
    outT = hb[:, _B_OUT:_B_OUT + DM]
    outDT = hb[:, _B_OUTD:_B_OUTD + DM]
    Acols = fb[:, 0:DS]
    convb = fb[:, 16:17]
    dtb = fb[:, 17:18]

    def bc_mm(tag, col, name, direct=False):
        """Row-broadcast matmul. Default: one-hot selector over the
        precomputed B/C rows (4x fewer active MACs than the stride-0
        re-projection - this kernel runs power-throttled). direct=True
        re-projects from xact (used where waiting for bcr would stall)."""
        t = ps.tile([DI, T], FP, tag="bc", name=name)
        if direct:
            w = hb[:, _B_BC + col:_B_BC + col + 1].broadcast_to((DI, DI))
            rhs, np_ = xact, DI
        else:
            w = hb[0:2 * DS, _B_EYE + col:_B_EYE + col + 1].broadcast_to(
                (2 * DS, DI))
            rhs, np_ = bcr, 2 * DS
        for c in range(NCH):
            nc.tensor.matmul(t[:, c * 512:(c + 1) * 512], w,
                             rhs[0:np_, c * 512:(c + 1) * 512] if not direct
                             else rhs[:, c * 512:(c + 1) * 512],
                             start=True, stop=True)
        return t

    # ---- in_proj-x with folded causal conv -> silu -> xact (fp16) ----
    # xc[:, t] = sum_k (diag(conv_w_k) @ in_w_x) @ u[:, t-3+k]; tap k reads
    # u_pad[:, c*512+k:]; odd k uses the 1-shifted copy so every rhs offset
    # stays 4B-aligned.  Silu is applied per half so the delta chain starts
    # as soon as the first half lands.
    xact = big.tile([DI, T], FH, tag="xact", name=f"xact{lid}")
    stacked = gl.get("stacked", False)
    w02T = hb[:, _B_W02:_B_W02 + DI]
    w13T = hb[:, _B_W13:_B_W13 + DI]
    for h in range(2):
        mmx = ps.tile([DI, H], FP, tag="bc", name=f"mmx{lid}_{h}")
        for c in (2 * h, 2 * h + 1):
            o = c * 512
            cs = slice(o - h * H, o - h * H + 512)
            if stacked:
                # up/upo hold [u_pad; u_pad<<2] on 128 partitions: 2 taps/mm
                nc.tensor.matmul(mmx[:, cs], w02T, up[:, o:o + 512],
                                 start=True, stop=False)
                nc.tensor.matmul(mmx[:, cs], w13T, upo[:, o:o + 512],
                                 start=False, stop=True)
            else:
                nc.tensor.matmul(mmx[:, cs], wkT[0], up[:, o:o + 512],
                                 start=True, stop=False)
                nc.tensor.matmul(mmx[:, cs], wkT[1], upo[:, o:o + 512],
                                 start=False, stop=False)
                nc.tensor.matmul(mmx[:, cs], wkT[2], up[:, o + 2:o + 514],
                                 start=False, stop=False)
                nc.tensor.matmul(mmx[:, cs], wkT[3], upo[:, o + 2:o + 514],
                                 start=False, stop=True)
        nc.scalar.activation(xact[:, h * H:(h + 1) * H], mmx[:], AF.Silu,
                             bias=convb)

    # ---- delta = softplus(dt_proj + dt_b) via Exp then Ln(1+x), halves ----
    delta = big.tile([DI, T], FH, tag="delta", name=f"delta{lid}")
    ev = big.tile([DI, T], FH, tag="ev", name=f"ev{lid}")
    dx = big.tile([DI, T], FH, tag="dx", name=f"dx{lid}")
    for h in range(2):
        mmd = ps.tile([DI, H], FP, tag="bc", name=f"mmd{lid}_{h}")
        for c in (2 * h, 2 * h + 1):
            o = c * 512
            nc.tensor.matmul(mmd[:, o - h * H:o - h * H + 512], wdT,
                             xact[:, o:o + 512], start=True, stop=True)
        nc.scalar.activation(ev[:, h * H:(h + 1) * H], mmd[:], AF.Exp,
                             bias=dtb)
    for h in range(2):
        hs_ = slice(h * H, (h + 1) * H)
        nc.scalar.activation(delta[:, hs_], ev[:, hs_], AF.Ln, bias=1.0)
        nc.vector.tensor_mul(dx[:, hs_], delta[:, hs_], xact[:, hs_])

    # ---- s-loop: single s=0 first (via stride-0 direct broadcasts and a
    # half-chained scan, so the first scan starts during the Act ramp),
    # then 7 pairs off precomputed B/C rows, then single s=15 ----
    ysn = big.tile([DI, T], FH, tag="ysn", name=f"ysn{lid}")
    yP = big.tile([DI, 2 * T], FH, tag="yP", name=f"yP{lid}")

    dA0 = big.tile([DI, T], FH, tag="dAs", name=f"dA{lid}_s0")
    dBu0 = big.tile([DI, T], FH, tag="dBus", name=f"dBu{lid}_s0")
    hs0 = big.tile([DI, T], FH, tag="hss", name=f"hs{lid}_s0")
    bps0 = ps.tile([DI, T], FP, tag="bc", name=f"bps{lid}_0")
    bw = hb[:, _B_BC:_B_BC + 1].broadcast_to((DI, DI))
    for h in range(2):
        hh = slice(h * H, (h + 1) * H)
        nc.scalar.activation(dA0[:, hh], delta[:, hh], AF.Exp,
                             scale=Acols[:, 0:1])
        for c in (2 * h, 2 * h + 1):
            nc.tensor.matmul(bps0[:, c * 512:(c + 1) * 512], bw,
                             xact[:, c * 512:(c + 1) * 512],
                             start=True, stop=True)
        nc.vector.tensor_mul(dBu0[:, hh], dx[:, hh], bps0[:, hh])
        nc.vector.tensor_tensor_scan(
            hs0[:, hh], dA0[:, hh], dBu0[:, hh],
            0.0 if h == 0 else hs0[:, H - 1:H], AX.mult, AX.add)
    cw = hb[:, _B_BC + DS:_B_BC + DS + 1].broadcast_to((DI, DI))
    cps0 = ps.tile([DI, T], FP, tag="bc", name=f"cps{lid}_0")
    for c in range(NCH):
        nc.tensor.matmul(cps0[:, c * 512:(c + 1) * 512], cw,
                         xact[:, c * 512:(c + 1) * 512], start=True, stop=True)
    nc.vector.tensor_mul(ysn[:], hs0[:], cps0[:])


    # pairs (1,2) .. (13,14); pair 1 uses the stride-0 direct broadcast so
    # it doesn't wait for the bcr rows (emitted after it, below)
    def pair(p):
        s0, s1 = 2 * p - 1, 2 * p
        dA = sl.tile([DI, 2 * T], FH, tag="dA", name=f"dA{lid}_{p}")
        nc.scalar.activation(dA[:, 0:T], delta[:], AF.Exp,
                             scale=Acols[:, s0:s0 + 1])
        nc.scalar.activation(dA[:, T:2 * T], delta[:], AF.Exp,
                             scale=Acols[:, s1:s1 + 1])
        # zero the boundary column so the scan carry resets between states
        nc.scalar.activation(dA[:, T:T + 1], gl["zcol"][:], AF.Copy)
        bcrep = sl.tile([DI, 2 * T], FH, tag="bcrep", name=f"brep{lid}_{p}")
        dBu = sl.tile([DI, 2 * T], FH, tag="dBu", name=f"dBu{lid}_{p}")
        for i, s in ((0, s0), (1, s1)):
            bps = bc_mm("bc", s, f"bps{lid}_{s}", direct=(p == 1))
            nc.scalar.activation(bcrep[:, i * T:(i + 1) * T], bps[:], AF.Copy)
            nc.vector.tensor_mul(dBu[:, i * T:(i + 1) * T], dx[:],
                                 bcrep[:, i * T:(i + 1) * T])
        hs = sl.tile([DI, 2 * T], FH, tag="hs", name=f"hs{lid}_{p}")
        nc.vector.tensor_tensor_scan(hs[:], dA[:], dBu[:], 0.0,
                                     AX.mult, AX.add)
        ccrep = sl.tile([DI, 2 * T], FH, tag="ccrep", name=f"crep{lid}_{p}")
        for i, s in ((0, s0), (1, s1)):
            cps = bc_mm("bc", DS + s, f"cps{lid}_{s}", direct=(p == 1))
            nc.scalar.activation(ccrep[:, i * T:(i + 1) * T], cps[:], AF.Copy)
        if p == 1:
            nc.vector.tensor_mul(yP[:], hs[:], ccrep[:])
        else:
            hsc = sl.tile([DI, 2 * T], FH, tag="hsc", name=f"hsc{lid}_{p}")
            nc.vector.tensor_mul(hsc[:], hs[:], ccrep[:])
            nc.vector.tensor_add(yP[:], yP[:], hsc[:])

    pair(1)

    # ---- B/C projection rows (32, T) for pairs 2..7 ----
    bcr = big.tile([2 * DS, T], FH, tag="bcr", name=f"bcr{lid}")
    mmb = ps.tile([DI, T], FP, tag="bc", name=f"mmb{lid}")
    for c in range(NCH):
        o = c * 512
        nc.tensor.matmul(mmb[0:2 * DS, o:o + 512], hb[:, _B_BC:_B_BC + 2 * DS],
                         xact[:, o:o + 512], start=True, stop=True)
    nc.scalar.activation(bcr[:], mmb[0:2 * DS, :], AF.Copy)
    for p in range(2, 8):
        pair(p)

    # s = 15
    dA15 = big.tile([DI, T], FH, tag="dAs2", name=f"dA{lid}_s15")
    nc.scalar.activation(dA15[:], delta[:], AF.Exp, scale=Acols[:, 15:16])
    bps15 = bc_mm("bc", 15, f"bps{lid}_15")
    brep15 = big.tile([DI, T], FH, tag="dBus2", name=f"brep{lid}_15")
    nc.scalar.activation(brep15[:], bps15[:], AF.Copy)
    dBu15 = big.tile([DI, T], FH, tag="dBuf", name=f"dBu{lid}_15")
    nc.vector.tensor_mul(dBu15[:], dx[:], brep15[:])
    hs15 = big.tile([DI, T], FH, tag="hss2", name=f"hs{lid}_s15")
    nc.vector.tensor_tensor_scan(hs15[:], dA15[:], dBu15[:], 0.0,
                                 AX.mult, AX.add)
    cps15 = bc_mm("bc", DS + 15, f"cps{lid}_15")
    crep15 = big.tile([DI, T], FH, tag="creps", name=f"crep{lid}_15")
    nc.scalar.activation(crep15[:], cps15[:], AF.Copy)
    hsc15 = big.tile([DI, T], FH, tag="hscs", name=f"hsc{lid}_15")
    nc.vector.tensor_mul(hsc15[:], hs15[:], crep15[:])
    nc.vector.tensor_add(ysn[:], ysn[:], hsc15[:])

    # ---- z-proj late (keeps the Act head short; silu set reloads once) ----
    zs = big.tile([DI, T], FH, tag="zs", name=f"zs{lid}")
    mmz = ps.tile([DI, T], FP, tag="bc", name=f"mmz{lid}")
    for c in range(NCH):
        o = c * 512
        nc.tensor.matmul(mmz[:, o:o + 512], zT, upo[0:DM, o + 2:o + 514],
                         start=True, stop=True)
    nc.scalar.activation(zs[:], mmz[:], AF.Silu)
    xsz = big.tile([DI, T], FH, tag="xsz", name=f"xsz{lid}")
    nc.vector.tensor_mul(xsz[:], xact[:], zs[:])

    # ---- y = (sum_s hs*C)*silu(z); out = out_w@y + (out_w*D)@(x*silu(z)) ----
    yf = big.tile([DI, T], FH, tag="yf", name=f"yf{lid}")
    for q in range(4):
        qq = slice(q * 512, (q + 1) * 512)
        qT = slice(T + q * 512, T + (q + 1) * 512)
        nc.vector.tensor_add(yf[:, qq], yP[:, qq], yP[:, qT])
        nc.vector.tensor_add(yf[:, qq], yf[:, qq], ysn[:, qq])
        nc.vector.tensor_mul(yf[:, qq], yf[:, qq], zs[:, qq])

    mmo = ps.tile([DI, T], FP, tag="bc", name=f"mmo{lid}")
    for c in range(NCH):
        o = c * 512
        nc.tensor.matmul(mmo[0:DM, o:o + 512], outDT, xsz[:, o:o + 512],
                         start=True, stop=False)
    for c in range(NCH):
        o = c * 512
        nc.tensor.matmul(mmo[0:DM, o:o + 512], outT, yf[:, o:o + 512],
                         start=False, stop=True)
    # chunked output copies: chunk q feeds the next layer's head ops / DMA
    NQ = 2 if len(out_specs) > 1 else 4
    Q = T // NQ
    for q in range(NQ):
        src = mmo[0:DM, q * Q:(q + 1) * Q]
        for j, (tl, off) in enumerate(out_specs):
            dst = tl[:, off + q * Q:off + (q + 1) * Q]
            if j == 0:
                nc.scalar.activation(dst, src, AF.Copy)
            else:
                nc.vector.tensor_copy(dst, src)
        if out_dma is not None:
            nc.sync.dma_start(out_dma[:, q * Q:(q + 1) * Q],
                              out_specs[0][0][:, out_specs[0][1] + q * Q:
                                              out_specs[0][1] + (q + 1) * Q])


def _build_kernel(ctx, tc, u0, u0o, hblobs, fblobs, outs):
    nc = tc.nc
    const = ctx.enter_context(tc.tile_pool(name="const", bufs=1))
    big = ctx.enter_context(tc.tile_pool(name="big", bufs=1))
    sl = ctx.enter_context(tc.tile_pool(name="sl", bufs=2))
    ps = ctx.enter_context(tc.tile_pool(name="ps", bufs=2, space="PSUM"))

    hb = [const.tile([DI, _HBLOB_W], FH, tag=f"hb{l}", name=f"hb{l}")
          for l in range(NL)]
    fb = [const.tile([DI, 20], FP, tag=f"fb{l}", name=f"fb{l}")
          for l in range(NL)]
    upA = const.tile([DI, UF], FH, tag="upA", name="upA")
    upAo = const.tile([DI, UF], FH, tag="upAo", name="upAo")
    nc.sync.dma_start(hb[0][:], hblobs[0][:])
    nc.sync.dma_start(upA[:], u0[:])
    nc.sync.dma_start(upAo[:], u0o[:])
    nc.sync.dma_start(fb[0][:], fblobs[0][:])
    nc.sync.dma_start(hb[1][:], hblobs[1][:])
    nc.sync.dma_start(fb[1][:], fblobs[1][:])
    upB = const.tile([DM, UF], FH, tag="upB", name="upB")
    upBo = const.tile([DM, UF], FH, tag="upBo", name="upBo")
    nc.gpsimd.memset(upB[:, 0:K - 1], 0.0)
    nc.gpsimd.memset(upB[:, UF - 1:UF], 0.0)
    nc.gpsimd.memset(upBo[:, 0:K - 2], 0.0)
    nc.gpsimd.memset(upBo[:, UF - 2:UF], 0.0)
    o2 = const.tile([DM, T], FH, tag="o2", name="o2")

    # short PE warm-up on zero weights/data (near-zero switching power)
    # so the first in_proj matmuls run at the 8/8 HAM clock.
    wz = const.tile([DI, 512], FH, tag="wz", name="wz")
    nc.gpsimd.memset(wz[:], 0.0)
    wps = ps.tile([DI, T // 2], FP, tag="bc", name="warm")
    for i in range(6):
        nc.tensor.matmul(wps[:, 0:512], wz[:, 0:DI], wz[:],
                         start=True, stop=True)


    zcol = const.tile([DI, 1], FH, tag="zcol", name="zcol")
    nc.gpsimd.memset(zcol[:], 0.0)

    pools = (const, big, sl, ps, {"lid": 0, "zcol": zcol, "stacked": True})
    # layer 1: outputs go to upB[:, 3:3+T] and upBo[:, 2:2+T]
    _build_layer(nc, pools, hb[0], fb[0], upA, upAo,
                 [(upB, K - 1), (upBo, K - 2)], outs[0])
    pools = (const, big, sl, ps, {"lid": 1, "zcol": zcol})
    _build_layer(nc, pools, hb[1], fb[1], upB, upBo, [(o2, 0)], outs[1])


def build_program():
    nc = bacc.Bacc("TRN2", target_bir_lowering=False, debug=False)
    u0 = nc.dram_tensor("u0", [DI, UF], FH, kind="ExternalInput").ap()
    u0o = nc.dram_tensor("u0o", [DI, UF], FH, kind="ExternalInput").ap()
    hblobs = [nc.dram_tensor(f"hblob{l}", [DI, _HBLOB_W], FH,
                             kind="ExternalInput").ap() for l in range(NL)]
    fblobs = [nc.dram_tensor(f"fblob{l}", [DI, 20], FP,
                             kind="ExternalInput").ap() for l in range(NL)]
    outs = [nc.dram_tensor(f"o{l + 1}T", [DM, T], FH,
                           kind="ExternalOutput").ap() for l in range(NL)]
    with tile.TileContext(nc) as tc:
        with ExitStack() as ctx:
            _build_kernel(ctx, tc, u0, u0o, hblobs, fblobs, outs)
    nc.compile()
    return nc


_PROG = None


def _get_prog():
    global _PROG
    if _PROG is None:
        _PROG = build_program()
    return _PROG


def _pad_u(u):
    """u: (64, T) f32 -> stacked (u2, u2o) fp16 (128, UF).

    Rows 0:64 hold u_pad (left pad K-1) / its 1-shift; rows 64:128 hold the
    same shifted 2 further, so layer 1's conv taps (0,2) and (1,3) each fold
    into one contraction-128 matmul."""
    up = np.zeros((DM, UF), np.float16)
    up[:, K - 1:K - 1 + T] = u.astype(np.float16)
    u2 = np.zeros((DI, UF), np.float16)
    u2o = np.zeros((DI, UF), np.float16)
    u2[0:DM] = up
    u2[DM:, 0:UF - 2] = up[:, 2:UF]
    u2o[0:DM, 0:UF - 1] = up[:, 1:UF]
    u2o[DM:, 0:UF - 3] = up[:, 3:UF]
    return u2, u2o


def _run_launch(u_list_T, raw, trace=False, trace_kwargs=None):
    """u_list_T: list of 8 arrays (64, 2048) f32. raw: param dict (np).
    Returns (o1_list, o2_list, res) with (64, 2048) fp16 outputs."""
    nc = _get_prog()
    blobs = [_pack_blobs(raw, l) for l in range(NL)]
    in_maps = []
    for b in range(8):
        up, upo = _pad_u(np.asarray(u_list_T[b], np.float32))
        in_maps.append({
            "u0": up, "u0o": upo,
            "hblob0": blobs[0][0], "fblob0": blobs[0][1],
            "hblob1": blobs[1][0], "fblob1": blobs[1][1],
        })
    res = bass_utils.run_bass_kernel_spmd(
        nc, in_maps, core_ids=list(range(8)), trace=trace,
        **(trace_kwargs or {}))
    o1 = [res.results[b]["o1T"] for b in range(8)]
    o2 = [res.results[b]["o2T"] for b in range(8)]
    return o1, o2, res


def kernel(**inputs):
    inp = {k: np.asarray(v, np.float32) for k, v in inputs.items()}
    Ms = inp["Ms_feature"]
    Pan = inp["Pan_feature"]
    h = C // 2
    names = ("in_w", "conv_w", "conv_b", "xp_w", "dt_w", "dt_b",
             "A_log", "D", "out_w")
    rawa = {n: inp["a_" + n] for n in names}
    rawb = {n: inp["b_" + n] for n in names}

    cf1 = np.concatenate([Ms[:, :h], Pan[:, h:]], axis=1)
    cf2 = np.concatenate([Pan[:, :h], Ms[:, h:]], axis=1)
    u_list = [cf1[b].T for b in range(B)] + [cf2[b].T for b in range(B)]
    o1, o2, _ = _run_launch(u_list, rawa)
    cf1_1 = np.stack([o1[b].T.astype(np.float32) for b in range(B)])
    cf2_1 = np.stack([o1[B + b].T.astype(np.float32) for b in range(B)])
    cf1_2 = np.stack([o2[b].T.astype(np.float32) for b in range(B)])
    cf2_2 = np.stack([o2[B + b].T.astype(np.float32) for b in range(B)])
    Ms1 = np.maximum((cf1_1 + cf2_1) * 0.5 + Ms, 0.0)
    Ms2 = np.maximum((cf1_2 + cf2_2) * 0.5 + Ms1, 0.0)

    cf3 = np.stack([Pan[:, ::2], Ms2[:, 1::2]], axis=2).reshape(B, C, DM)
    cf4 = np.stack([Ms2[:, ::2], Pan[:, 1::2]], axis=2).reshape(B, C, DM)
    u_list = [cf3[b].T for b in range(B)] + [cf4[b].T for b in range(B)]
    o1, o2, _ = _run_launch(u_list, rawb)
    cf3_1 = np.stack([o1[b].T.astype(np.float32) for b in range(B)])
    cf4_1 = np.stack([o1[B + b].T.astype(np.float32) for b in range(B)])
    cf3_2 = np.stack([o2[b].T.astype(np.float32) for b in range(B)])
    cf4_2 = np.stack([o2[B + b].T.astype(np.float32) for b in range(B)])
    Pan1 = np.maximum((cf3_1 + cf4_1) * 0.5 + Pan, 0.0)
    Pan2 = np.maximum((cf3_2 + cf4_2) * 0.5 + Pan1, 0.0)
    return Ms2, Pan2


# revision 26
# speedup vs baseline: 1.2184x; 1.0094x over previous
"""Trainium2 Bass kernel for nn_CMCI_Mamba.

Strategy: data-parallel over the 2B=8 mamba streams (1 sequence per core).
Each launch runs 2 chained mamba layers fully on-chip in d-major layout
(features on partitions, time on the free axis).

Engine assignment (per layer):
- PE (fp16): in_proj with the causal conv FOLDED IN (4 shifted matmuls with
  host-prescaled weights diag(conv_w_k) @ in_w), z-proj, fused
  dt_w@xp_w[dt] projection, 32 stride-0 B/C broadcast matmuls, out_proj.
- Act: Silu(conv) / Silu(z) straight from PSUM, softplus via Exp+Ln (one
  table set), the 16 per-state dA = exp(A_s * delta) passes, PSUM->SBUF
  fp16 copies of the B/C broadcasts, layer-output copies.
- DVE: the 16 SSM scans (tensor_tensor_scan, batched 2 states per
  instruction with a zeroed dA column resetting the carry), all dBu and
  hs*C muls and the y accumulation.  GPSIMD is deliberately idle: its
  tensor ops starve the DVE's shared SBUF port (measured 7x slowdown on
  concurrent DVE tensor_tensor).

Host does the cheap cross-stream elementwise combines between launches.
"""
import sys
import numpy as np
from contextlib import ExitStack

for _p in ("/opt/trn_rl_repo",):
    if _p not in sys.path:
        sys.path.insert(0, _p)

import concourse.bass as bass
import concourse.bacc as bacc
import concourse.tile as tile
from concourse import mybir
from concourse import bass_utils

T, DM, DI, DS, DR, K, NL = 2048, 64, 128, 16, 4, 4, 2
B, C = 4, 2048
UF = T + K  # padded u width (2052)
FP = mybir.dt.float32
FH = mybir.dt.float16
AX = mybir.AluOpType
AF = mybir.ActivationFunctionType

# fp16 param blob column layout, (128, 1024) per layer
_B_WK = 0       # [0:64, 0:512]    4x conv-scaled in_proj-x lhsT (64,128) each
_B_Z = 512      # [0:64, 512:640]  z lhsT
_B_WD = 640     # [:, 640:768]     (dt_w @ xp_w[:DR]) lhsT
_B_BC = 768     # [:, 768:800]     B/C projection columns (32)
_B_OUT = 800    # [:, 800:864]     out_proj lhsT
_B_OUTD = 864   # [:, 864:928]     out_proj lhsT with D folded (for x*sz term)
_B_EYE = 928    # [0:32, 928:960]  eye(32) one-hot selectors for row broadcast
_B_W02 = 960    # [:, 960:1088]    taps 0+2 stacked lhsT (contraction 128)
_B_W13 = 1088   # [:, 1088:1216]   taps 1+3 stacked lhsT
_HBLOB_W = 1280
# fp32 blob (128, 20): [:, 0:16]=A (=-exp(A_log)), 16=conv_b, 17=dt_b, 18=D


def _pack_blobs(raw, l):
    hb = np.zeros((DI, _HBLOB_W), np.float16)
    in_w = raw["in_w"][l]          # (256, 64)
    conv_w = raw["conv_w"][l]      # (128, 4)
    for k in range(K):
        wk = in_w[:DI] * conv_w[:, k:k + 1]          # (128, 64)
        hb[:DM, _B_WK + 128 * k:_B_WK + 128 * (k + 1)] = wk.T
    hb[:DM, _B_Z:_B_Z + DI] = in_w[DI:2 * DI].T
    wd = raw["dt_w"][l] @ raw["xp_w"][l][:DR]        # (128, 128)
    hb[:, _B_WD:_B_WD + DI] = wd.T
    hb[:, _B_BC:_B_BC + 2 * DS] = raw["xp_w"][l][DR:DR + 2 * DS].T
    hb[:, _B_OUT:_B_OUT + DM] = raw["out_w"][l].T
    # out_proj with D folded in: out += (out_w * D) @ (x * silu(z))
    hb[:, _B_OUTD:_B_OUTD + DM] = (raw["out_w"][l] * raw["D"][l]).T
    hb[0:2 * DS, _B_EYE:_B_EYE + 2 * DS] = np.eye(2 * DS, dtype=np.float16)
    # layer-1 ramp path: taps (0,2) and (1,3) stacked into c=128 matmuls
    for j, (ka, kb) in enumerate(((0, 2), (1, 3))):
        col = (_B_W02, _B_W13)[j]
        hb[0:DM, col:col + DI] = (in_w[:DI] * conv_w[:, ka:ka + 1]).T
        hb[DM:2 * DM, col:col + DI] = (in_w[:DI] * conv_w[:, kb:kb + 1]).T
    fb = np.zeros((DI, 20), np.float32)
    fb[:, 0:DS] = -np.exp(raw["A_log"][l])
    fb[:, 16] = raw["conv_b"][l]
    fb[:, 17] = raw["dt_b"][l]
    fb[:, 18] = raw["D"][l]
    return hb, fb


def _build_layer(nc, pools, hb, fb, up, upo, out_specs, out_dma):
    """One mamba layer. up/upo: (64, UF) fp16 padded input (+1-shifted copy).
    out_specs: list of (tile, col_off) -- the (64, T) layer output is copied
    (in halves, on Act) into tile[:, off:off+T]. out_dma: DRAM ap or None.
    """
    const, big, sl, ps, gl = pools
    NCH = T // 512
    H = T // 2
    lid = gl["lid"]

    wkT = [hb[0:DM, _B_WK + 128 * k:_B_WK + 128 * (k + 1)] for k in range(K)]
    zT = hb[0:DM, _B_Z:_B_Z + DI]
    wdT = hb[:, _B_WD:_B_WD + DI]
    outT = hb[:, _B_OUT:_B_OUT + DM]
    outDT = hb[:, _B_OUTD:_B_OUTD + DM]
    Acols = fb[:, 0:DS]
    convb = fb[:, 16:17]
    dtb = fb[:, 17:18]

    def bc_mm(tag, col, name, direct=False):
        """Row-broadcast matmul. Default: one-hot selector over the
        precomputed B/C rows (4x fewer active MACs than the stride-0
        re-projection - this kernel runs power-throttled). direct=True
        re-projects from xact (used where waiting for bcr would stall)."""
        t = ps.tile([DI, T], FP, tag="bc", name=name)
        if direct:
            w = hb[:, _B_BC + col:_B_BC + col + 1].broadcast_to((DI, DI))
            rhs, np_ = xact, DI
        else:
            w = hb[0:2 * DS, _B_EYE + col:_B_EYE + col + 1].broadcast_to(
                (2 * DS, DI))
            rhs, np_ = bcr, 2 * DS
        for c in range(NCH):
            nc.tensor.matmul(t[:, c * 512:(c + 1) * 512], w,
                             rhs[0:np_, c * 512:(c + 1) * 512] if not direct
                             else rhs[:, c * 512:(c + 1) * 512],
                             start=True, stop=True)
        return t

    # ---- in_proj-x with folded causal conv -> silu -> xact (fp16) ----
    # xc[:, t] = sum_k (diag(conv_w_k) @ in_w_x) @ u[:, t-3+k]; tap k reads
    # u_pad[:, c*512+k:]; odd k uses the 1-shifted copy so every rhs offset
    # stays 4B-aligned.  Silu is applied per half so the delta chain starts
    # as soon as the first half lands.
    xact = big.tile([DI, T], FH, tag="xact", name=f"xact{lid}")
    stacked = gl.get("stacked", False)
    w02T = hb[:, _B_W02:_B_W02 + DI]
    w13T = hb[:, _B_W13:_B_W13 + DI]
    for h in range(2):
        mmx = ps.tile([DI, H], FP, tag="bc", name=f"mmx{lid}_{h}")
        for c in (2 * h, 2 * h + 1):
            o = c * 512
            cs = slice(o - h * H, o - h * H + 512)
            if stacked:
                # up/upo hold [u_pad; u_pad<<2] on 128 partitions: 2 taps/mm
                nc.tensor.matmul(mmx[:, cs], w02T, up[:, o:o + 512],
                                 start=True, stop=False)
                nc.tensor.matmul(mmx[:, cs], w13T, upo[:, o:o + 512],
                                 start=False, stop=True)
            else:
                nc.tensor.matmul(mmx[:, cs], wkT[0], up[:, o:o + 512],
                                 start=True, stop=False)
                nc.tensor.matmul(mmx[:, cs], wkT[1], upo[:, o:o + 512],
                                 start=False, stop=False)
                nc.tensor.matmul(mmx[:, cs], wkT[2], up[:, o + 2:o + 514],
                                 start=False, stop=False)
                nc.tensor.matmul(mmx[:, cs], wkT[3], upo[:, o + 2:o + 514],
                                 start=False, stop=True)
        nc.scalar.activation(xact[:, h * H:(h + 1) * H], mmx[:], AF.Silu,
                             bias=convb)

    # ---- delta = softplus(dt_proj + dt_b) via Exp then Ln(1+x), halves ----
    delta = big.tile([DI, T], FH, tag="delta", name=f"delta{lid}")
    ev = big.tile([DI, T], FH, tag="ev", name=f"ev{lid}")
    dx = big.tile([DI, T], FH, tag="dx", name=f"dx{lid}")
    for h in range(2):
        mmd = ps.tile([DI, H], FP, tag="bc", name=f"mmd{lid}_{h}")
        for c in (2 * h, 2 * h + 1):
            o = c * 512
            nc.tensor.matmul(mmd[:, o - h * H:o - h * H + 512], wdT,
                             xact[:, o:o + 512], start=True, stop=True)
        nc.scalar.activation(ev[:, h * H:(h + 1) * H], mmd[:], AF.Exp,
                             bias=dtb)
    for h in range(2):
        hs_ = slice(h * H, (h + 1) * H)
        nc.scalar.activation(delta[:, hs_], ev[:, hs_], AF.Ln, bias=1.0)
        nc.vector.tensor_mul(dx[:, hs_], delta[:, hs_], xact[:, hs_])

    # ---- s-loop: single s=0 first (via stride-0 direct broadcasts and a
    # half-chained scan, so the first scan starts during the Act ramp),
    # then 7 pairs off precomputed B/C rows, then single s=15 ----
    ysn = big.tile([DI, T], FH, tag="ysn", name=f"ysn{lid}")
    yP = big.tile([DI, 2 * T], FH, tag="yP", name=f"yP{lid}")

    dA0 = big.tile([DI, T], FH, tag="dAs", name=f"dA{lid}_s0")
    dBu0 = big.tile([DI, T], FH, tag="dBus", name=f"dBu{lid}_s0")
    hs0 = big.tile([DI, T], FH, tag="hss", name=f"hs{lid}_s0")
    bps0 = ps.tile([DI, T], FP, tag="bc", name=f"bps{lid}_0")
    bw = hb[:, _B_BC:_B_BC + 1].broadcast_to((DI, DI))
    for h in range(2):
        hh = slice(h * H, (h + 1) * H)
        nc.scalar.activation(dA0[:, hh], delta[:, hh], AF.Exp,
                             scale=Acols[:, 0:1])
        for c in (2 * h, 2 * h + 1):
            nc.tensor.matmul(bps0[:, c * 512:(c + 1) * 512], bw,
                             xact[:, c * 512:(c + 1) * 512],
                             start=True, stop=True)
        nc.vector.tensor_mul(dBu0[:, hh], dx[:, hh], bps0[:, hh])
        nc.vector.tensor_tensor_scan(
            hs0[:, hh], dA0[:, hh], dBu0[:, hh],
            0.0 if h == 0 else hs0[:, H - 1:H], AX.mult, AX.add)
    cw = hb[:, _B_BC + DS:_B_BC + DS + 1].broadcast_to((DI, DI))
    cps0 = ps.tile([DI, T], FP, tag="bc", name=f"cps{lid}_0")
    for c in range(NCH):
        nc.tensor.matmul(cps0[:, c * 512:(c + 1) * 512], cw,
                         xact[:, c * 512:(c + 1) * 512], start=True, stop=True)
    nc.vector.tensor_mul(ysn[:], hs0[:], cps0[:])


    # pairs (1,2) .. (13,14); pair 1 uses the stride-0 direct broadcast so
    # it doesn't wait for the bcr rows (emitted after it, below)
    def pair(p):
        s0, s1 = 2 * p - 1, 2 * p
        bcrep = sl.tile([DI, 2 * T], FH, tag="bcrep", name=f"brep{lid}_{p}")
        dBu = sl.tile([DI, 2 * T], FH, tag="dBu", name=f"dBu{lid}_{p}")
        for i, s in ((0, s0), (1, s1)):
            bps = bc_mm("bc", s, f"bps{lid}_{s}", direct=(p == 1))
            nc.scalar.activation(bcrep[:, i * T:(i + 1) * T], bps[:], AF.Copy)
            nc.vector.tensor_mul(dBu[:, i * T:(i + 1) * T], dx[:],
                                 bcrep[:, i * T:(i + 1) * T])
        dA = sl.tile([DI, 2 * T], FH, tag="dA", name=f"dA{lid}_{p}")
        nc.scalar.activation(dA[:, 0:T], delta[:], AF.Exp,
                             scale=Acols[:, s0:s0 + 1])
        nc.scalar.activation(dA[:, T:2 * T], delta[:], AF.Exp,
                             scale=Acols[:, s1:s1 + 1])
        # zero the boundary column so the scan carry resets between states
        nc.scalar.activation(dA[:, T:T + 1], gl["zcol"][:], AF.Copy)
        hs = sl.tile([DI, 2 * T], FH, tag="hs", name=f"hs{lid}_{p}")
        nc.vector.tensor_tensor_scan(hs[:], dA[:], dBu[:], 0.0,
                                     AX.mult, AX.add)
        ccrep = sl.tile([DI, 2 * T], FH, tag="ccrep", name=f"crep{lid}_{p}")
        for i, s in ((0, s0), (1, s1)):
            cps = bc_mm("bc", DS + s, f"cps{lid}_{s}", direct=(p == 1))
            nc.scalar.activation(ccrep[:, i * T:(i + 1) * T], cps[:], AF.Copy)
        if p == 1:
            nc.vector.tensor_mul(yP[:], hs[:], ccrep[:])
        else:
            hsc = sl.tile([DI, 2 * T], FH, tag="hsc", name=f"hsc{lid}_{p}")
            nc.vector.tensor_mul(hsc[:], hs[:], ccrep[:])
            nc.vector.tensor_add(yP[:], yP[:], hsc[:])

    pair(1)

    # ---- B/C projection rows (32, T) for pairs 2..7 ----
    bcr = big.tile([2 * DS, T], FH, tag="bcr", name=f"bcr{lid}")
    mmb = ps.tile([DI, T], FP, tag="bc", name=f"mmb{lid}")
    for c in range(NCH):
        o = c * 512
        nc.tensor.matmul(mmb[0:2 * DS, o:o + 512], hb[:, _B_BC:_B_BC + 2 * DS],
                         xact[:, o:o + 512], start=True, stop=True)
    nc.scalar.activation(bcr[:], mmb[0:2 * DS, :], AF.Copy)
    for p in range(2, 8):
        pair(p)

    # s = 15
    dA15 = big.tile([DI, T], FH, tag="dAs2", name=f"dA{lid}_s15")
    nc.scalar.activation(dA15[:], delta[:], AF.Exp, scale=Acols[:, 15:16])
    bps15 = bc_mm("bc", 15, f"bps{lid}_15")
    brep15 = big.tile([DI, T], FH, tag="dBus2", name=f"brep{lid}_15")
    nc.scalar.activation(brep15[:], bps15[:], AF.Copy)
    dBu15 = big.tile([DI, T], FH, tag="dBuf", name=f"dBu{lid}_15")
    nc.vector.tensor_mul(dBu15[:], dx[:], brep15[:])
    hs15 = big.tile([DI, T], FH, tag="hss2", name=f"hs{lid}_s15")
    nc.vector.tensor_tensor_scan(hs15[:], dA15[:], dBu15[:], 0.0,
                                 AX.mult, AX.add)
    cps15 = bc_mm("bc", DS + 15, f"cps{lid}_15")
    crep15 = big.tile([DI, T], FH, tag="creps", name=f"crep{lid}_15")
    nc.scalar.activation(crep15[:], cps15[:], AF.Copy)
    hsc15 = big.tile([DI, T], FH, tag="hscs", name=f"hsc{lid}_15")
    nc.vector.tensor_mul(hsc15[:], hs15[:], crep15[:])
    nc.vector.tensor_add(ysn[:], ysn[:], hsc15[:])

    # ---- z-proj late (keeps the Act head short; silu set reloads once) ----
    zs = big.tile([DI, T], FH, tag="zs", name=f"zs{lid}")
    mmz = ps.tile([DI, T], FP, tag="bc", name=f"mmz{lid}")
    for c in range(NCH):
        o = c * 512
        nc.tensor.matmul(mmz[:, o:o + 512], zT, upo[0:DM, o + 2:o + 514],
                         start=True, stop=True)
    nc.scalar.activation(zs[:], mmz[:], AF.Silu)
    xsz = big.tile([DI, T], FH, tag="xsz", name=f"xsz{lid}")
    nc.vector.tensor_mul(xsz[:], xact[:], zs[:])

    # ---- y = (sum_s hs*C)*silu(z); out = out_w@y + (out_w*D)@(x*silu(z)) ----
    yf = big.tile([DI, T], FH, tag="yf", name=f"yf{lid}")
    for q in range(4):
        qq = slice(q * 512, (q + 1) * 512)
        qT = slice(T + q * 512, T + (q + 1) * 512)
        nc.vector.tensor_add(yf[:, qq], yP[:, qq], yP[:, qT])
        nc.vector.tensor_add(yf[:, qq], yf[:, qq], ysn[:, qq])
        nc.vector.tensor_mul(yf[:, qq], yf[:, qq], zs[:, qq])

    mmo = ps.tile([DI, T], FP, tag="bc", name=f"mmo{lid}")
    for c in range(NCH):
        o = c * 512
        nc.tensor.matmul(mmo[0:DM, o:o + 512], outDT, xsz[:, o:o + 512],
                         start=True, stop=False)
    for c in range(NCH):
        o = c * 512
        nc.tensor.matmul(mmo[0:DM, o:o + 512], outT, yf[:, o:o + 512],
                         start=False, stop=True)
    # chunked output copies: chunk q feeds the next layer's head ops / DMA
    NQ = 2 if len(out_specs) > 1 else 4
    Q = T // NQ
    for q in range(NQ):
        src = mmo[0:DM, q * Q:(q + 1) * Q]
        for j, (tl, off) in enumerate(out_specs):
            dst = tl[:, off + q * Q:off + (q + 1) * Q]
            if j == 0:
                nc.scalar.activation(dst, src, AF.Copy)
            else:
                nc.vector.tensor_copy(dst, src)
        if out_dma is not None:
            nc.sync.dma_start(out_dma[:, q * Q:(q + 1) * Q],
                              out_specs[0][0][:, out_specs[0][1] + q * Q:
                                              out_specs[0][1] + (q + 1) * Q])


def _build_kernel(ctx, tc, u0, u0o, hblobs, fblobs, outs):
    nc = tc.nc
    const = ctx.enter_context(tc.tile_pool(name="const", bufs=1))
    big = ctx.enter_context(tc.tile_pool(name="big", bufs=1))
    sl = ctx.enter_context(tc.tile_pool(name="sl", bufs=2))
    ps = ctx.enter_context(tc.tile_pool(name="ps", bufs=2, space="PSUM"))

    hb = [const.tile([DI, _HBLOB_W], FH, tag=f"hb{l}", name=f"hb{l}")
          for l in range(NL)]
    fb = [const.tile([DI, 20], FP, tag=f"fb{l}", name=f"fb{l}")
          for l in range(NL)]
    upA = const.tile([DI, UF], FH, tag="upA", name="upA")
    upAo = const.tile([DI, UF], FH, tag="upAo", name="upAo")
    nc.sync.dma_start(hb[0][:], hblobs[0][:])
    HF = UF // 2
    nc.sync.dma_start(upA[:, 0:HF], u0[:, 0:HF])
    nc.sync.dma_start(upAo[:, 0:HF], u0o[:, 0:HF])
    nc.sync.dma_start(upA[:, HF:UF], u0[:, HF:UF])
    nc.sync.dma_start(upAo[:, HF:UF], u0o[:, HF:UF])
    nc.sync.dma_start(fb[0][:], fblobs[0][:])
    nc.sync.dma_start(hb[1][:], hblobs[1][:])
    nc.sync.dma_start(fb[1][:], fblobs[1][:])
    upB = const.tile([DM, UF], FH, tag="upB", name="upB")
    upBo = const.tile([DM, UF], FH, tag="upBo", name="upBo")
    nc.gpsimd.memset(upB[:, 0:K - 1], 0.0)
    nc.gpsimd.memset(upB[:, UF - 1:UF], 0.0)
    nc.gpsimd.memset(upBo[:, 0:K - 2], 0.0)
    nc.gpsimd.memset(upBo[:, UF - 2:UF], 0.0)
    o2 = const.tile([DM, T], FH, tag="o2", name="o2")

    # short PE warm-up on zero weights/data (near-zero switching power)
    # so the first in_proj matmuls run at the 8/8 HAM clock.
    wz = const.tile([DI, 512], FH, tag="wz", name="wz")
    nc.gpsimd.memset(wz[:], 0.0)
    wps = ps.tile([DI, T // 2], FP, tag="bc", name="warm")
    for i in range(6):
        nc.tensor.matmul(wps[:, 0:512], wz[:, 0:DI], wz[:],
                         start=True, stop=True)


    zcol = const.tile([DI, 1], FH, tag="zcol", name="zcol")
    nc.gpsimd.memset(zcol[:], 0.0)
    tl_warm = const.tile([DI, 1], FH, tag="tlw", name="tlw")
    nc.scalar.activation(tl_warm[:], zcol[:], AF.Silu)

    pools = (const, big, sl, ps, {"lid": 0, "zcol": zcol, "stacked": True})
    # layer 1: outputs go to upB[:, 3:3+T] and upBo[:, 2:2+T]
    _build_layer(nc, pools, hb[0], fb[0], upA, upAo,
                 [(upB, K - 1), (upBo, K - 2)], outs[0])
    pools = (const, big, sl, ps, {"lid": 1, "zcol": zcol})
    _build_layer(nc, pools, hb[1], fb[1], upB, upBo, [(o2, 0)], outs[1])


def build_program():
    nc = bacc.Bacc("TRN2", target_bir_lowering=False, debug=False)
    u0 = nc.dram_tensor("u0", [DI, UF], FH, kind="ExternalInput").ap()
    u0o = nc.dram_tensor("u0o", [DI, UF], FH, kind="ExternalInput").ap()
    hblobs = [nc.dram_tensor(f"hblob{l}", [DI, _HBLOB_W], FH,
                             kind="ExternalInput").ap() for l in range(NL)]
    fblobs = [nc.dram_tensor(f"fblob{l}", [DI, 20], FP,
                             kind="ExternalInput").ap() for l in range(NL)]
    outs = [nc.dram_tensor(f"o{l + 1}T", [DM, T], FH,
                           kind="ExternalOutput").ap() for l in range(NL)]
    with tile.TileContext(nc) as tc:
        with ExitStack() as ctx:
            _build_kernel(ctx, tc, u0, u0o, hblobs, fblobs, outs)
    nc.compile()
    return nc


_PROG = None


def _get_prog():
    global _PROG
    if _PROG is None:
        _PROG = build_program()
    return _PROG


def _pad_u(u):
    """u: (64, T) f32 -> stacked (u2, u2o) fp16 (128, UF).

    Rows 0:64 hold u_pad (left pad K-1) / its 1-shift; rows 64:128 hold the
    same shifted 2 further, so layer 1's conv taps (0,2) and (1,3) each fold
    into one contraction-128 matmul."""
    up = np.zeros((DM, UF), np.float16)
    up[:, K - 1:K - 1 + T] = u.astype(np.float16)
    u2 = np.zeros((DI, UF), np.float16)
    u2o = np.zeros((DI, UF), np.float16)
    u2[0:DM] = up
    u2[DM:, 0:UF - 2] = up[:, 2:UF]
    u2o[0:DM, 0:UF - 1] = up[:, 1:UF]
    u2o[DM:, 0:UF - 3] = up[:, 3:UF]
    return u2, u2o


def _run_launch(u_list_T, raw, trace=False, trace_kwargs=None):
    """u_list_T: list of 8 arrays (64, 2048) f32. raw: param dict (np).
    Returns (o1_list, o2_list, res) with (64, 2048) fp16 outputs."""
    nc = _get_prog()
    blobs = [_pack_blobs(raw, l) for l in range(NL)]
    in_maps = []
    for b in range(8):
        up, upo = _pad_u(np.asarray(u_list_T[b], np.float32))
        in_maps.append({
            "u0": up, "u0o": upo,
            "hblob0": blobs[0][0], "fblob0": blobs[0][1],
            "hblob1": blobs[1][0], "fblob1": blobs[1][1],
        })
    res = bass_utils.run_bass_kernel_spmd(
        nc, in_maps, core_ids=list(range(8)), trace=trace,
        **(trace_kwargs or {}))
    o1 = [res.results[b]["o1T"] for b in range(8)]
    o2 = [res.results[b]["o2T"] for b in range(8)]
    return o1, o2, res


def kernel(**inputs):
    inp = {k: np.asarray(v, np.float32) for k, v in inputs.items()}
    Ms = inp["Ms_feature"]
    Pan = inp["Pan_feature"]
    h = C // 2
    names = ("in_w", "conv_w", "conv_b", "xp_w", "dt_w", "dt_b",
             "A_log", "D", "out_w")
    rawa = {n: inp["a_" + n] for n in names}
    rawb = {n: inp["b_" + n] for n in names}

    cf1 = np.concatenate([Ms[:, :h], Pan[:, h:]], axis=1)
    cf2 = np.concatenate([Pan[:, :h], Ms[:, h:]], axis=1)
    u_list = [cf1[b].T for b in range(B)] + [cf2[b].T for b in range(B)]
    o1, o2, _ = _run_launch(u_list, rawa)
    cf1_1 = np.stack([o1[b].T.astype(np.float32) for b in range(B)])
    cf2_1 = np.stack([o1[B + b].T.astype(np.float32) for b in range(B)])
    cf1_2 = np.stack([o2[b].T.astype(np.float32) for b in range(B)])
    cf2_2 = np.stack([o2[B + b].T.astype(np.float32) for b in range(B)])
    Ms1 = np.maximum((cf1_1 + cf2_1) * 0.5 + Ms, 0.0)
    Ms2 = np.maximum((cf1_2 + cf2_2) * 0.5 + Ms1, 0.0)

    cf3 = np.stack([Pan[:, ::2], Ms2[:, 1::2]], axis=2).reshape(B, C, DM)
    cf4 = np.stack([Ms2[:, ::2], Pan[:, 1::2]], axis=2).reshape(B, C, DM)
    u_list = [cf3[b].T for b in range(B)] + [cf4[b].T for b in range(B)]
    o1, o2, _ = _run_launch(u_list, rawb)
    cf3_1 = np.stack([o1[b].T.astype(np.float32) for b in range(B)])
    cf4_1 = np.stack([o1[B + b].T.astype(np.float32) for b in range(B)])
    cf3_2 = np.stack([o2[b].T.astype(np.float32) for b in range(B)])
    cf4_2 = np.stack([o2[B + b].T.astype(np.float32) for b in range(B)])
    Pan1 = np.maximum((cf3_1 + cf4_1) * 0.5 + Pan, 0.0)
    Pan2 = np.maximum((cf3_2 + cf4_2) * 0.5 + Pan1, 0.0)
    return Ms2, Pan2


# revision 28
# speedup vs baseline: 1.2209x; 1.0020x over previous
"""Trainium2 Bass kernel for nn_CMCI_Mamba.

Strategy: data-parallel over the 2B=8 mamba streams (1 sequence per core).
Each launch runs 2 chained mamba layers fully on-chip in d-major layout
(features on partitions, time on the free axis).

Engine assignment (per layer):
- PE (fp16): in_proj with the causal conv FOLDED IN (4 shifted matmuls with
  host-prescaled weights diag(conv_w_k) @ in_w), z-proj, fused
  dt_w@xp_w[dt] projection, 32 stride-0 B/C broadcast matmuls, out_proj.
- Act: Silu(conv) / Silu(z) straight from PSUM, softplus via Exp+Ln (one
  table set), the 16 per-state dA = exp(A_s * delta) passes, PSUM->SBUF
  fp16 copies of the B/C broadcasts, layer-output copies.
- DVE: the 16 SSM scans (tensor_tensor_scan, batched 2 states per
  instruction with a zeroed dA column resetting the carry), all dBu and
  hs*C muls and the y accumulation.  GPSIMD is deliberately idle: its
  tensor ops starve the DVE's shared SBUF port (measured 7x slowdown on
  concurrent DVE tensor_tensor).

Host does the cheap cross-stream elementwise combines between launches.
"""
import sys
import numpy as np
from contextlib import ExitStack

for _p in ("/opt/trn_rl_repo",):
    if _p not in sys.path:
        sys.path.insert(0, _p)

import concourse.bass as bass
import concourse.bacc as bacc
import concourse.tile as tile
from concourse import mybir
from concourse import bass_utils

T, DM, DI, DS, DR, K, NL = 2048, 64, 128, 16, 4, 4, 2
B, C = 4, 2048
UF = T + K  # padded u width (2052)
FP = mybir.dt.float32
FH = mybir.dt.float16
AX = mybir.AluOpType
AF = mybir.ActivationFunctionType

# fp16 param blob column layout, (128, 1024) per layer
_B_WK = 0       # [0:64, 0:512]    4x conv-scaled in_proj-x lhsT (64,128) each
_B_Z = 512      # [0:64, 512:640]  z lhsT
_B_WD = 640     # [:, 640:768]     (dt_w @ xp_w[:DR]) lhsT
_B_BC = 768     # [:, 768:800]     B/C projection columns (32)
_B_OUT = 800    # [:, 800:864]     out_proj lhsT
_B_OUTD = 864   # [:, 864:928]     out_proj lhsT with D folded (for x*sz term)
_B_EYE = 928    # [0:32, 928:960]  eye(32) one-hot selectors for row broadcast
_B_W02 = 960    # [:, 960:1088]    taps 0+2 stacked lhsT (contraction 128)
_B_W13 = 1088   # [:, 1088:1216]   taps 1+3 stacked lhsT
_HBLOB_W = 1280
# fp32 blob (128, 20): [:, 0:16]=A (=-exp(A_log)), 16=conv_b, 17=dt_b, 18=D


def _pack_blobs(raw, l):
    hb = np.zeros((DI, _HBLOB_W), np.float16)
    in_w = raw["in_w"][l]          # (256, 64)
    conv_w = raw["conv_w"][l]      # (128, 4)
    for k in range(K):
        wk = in_w[:DI] * conv_w[:, k:k + 1]          # (128, 64)
        hb[:DM, _B_WK + 128 * k:_B_WK + 128 * (k + 1)] = wk.T
    hb[:DM, _B_Z:_B_Z + DI] = in_w[DI:2 * DI].T
    wd = raw["dt_w"][l] @ raw["xp_w"][l][:DR]        # (128, 128)
    hb[:, _B_WD:_B_WD + DI] = wd.T
    hb[:, _B_BC:_B_BC + 2 * DS] = raw["xp_w"][l][DR:DR + 2 * DS].T
    hb[:, _B_OUT:_B_OUT + DM] = raw["out_w"][l].T
    # out_proj with D folded in: out += (out_w * D) @ (x * silu(z))
    hb[:, _B_OUTD:_B_OUTD + DM] = (raw["out_w"][l] * raw["D"][l]).T
    hb[0:2 * DS, _B_EYE:_B_EYE + 2 * DS] = np.eye(2 * DS, dtype=np.float16)
    # layer-1 ramp path: taps (0,2) and (1,3) stacked into c=128 matmuls
    for j, (ka, kb) in enumerate(((0, 2), (1, 3))):
        col = (_B_W02, _B_W13)[j]
        hb[0:DM, col:col + DI] = (in_w[:DI] * conv_w[:, ka:ka + 1]).T
        hb[DM:2 * DM, col:col + DI] = (in_w[:DI] * conv_w[:, kb:kb + 1]).T
    fb = np.zeros((DI, 20), np.float32)
    fb[:, 0:DS] = -np.exp(raw["A_log"][l])
    fb[:, 16] = raw["conv_b"][l]
    fb[:, 17] = raw["dt_b"][l]
    fb[:, 18] = raw["D"][l]
    return hb, fb


def _build_layer(nc, pools, hb, fb, up, upo, out_specs, out_dma):
    """One mamba layer. up/upo: (64, UF) fp16 padded input (+1-shifted copy).
    out_specs: list of (tile, col_off) -- the (64, T) layer output is copied
    (in halves, on Act) into tile[:, off:off+T]. out_dma: DRAM ap or None.
    """
    const, big, sl, ps, gl = pools
    NCH = T // 512
    H = T // 2
    lid = gl["lid"]

    wkT = [hb[0:DM, _B_WK + 128 * k:_B_WK + 128 * (k + 1)] for k in range(K)]
    zT = hb[0:DM, _B_Z:_B_Z + DI]
    wdT = hb[:, _B_WD:_B_WD + DI]
    outT = hb[:, _B_OUT:_B_OUT + DM]
    outDT = hb[:, _B_OUTD:_B_OUTD + DM]
    Acols = fb[:, 0:DS]
    convb = fb[:, 16:17]
    dtb = fb[:, 17:18]

    def bc_mm(tag, col, name, direct=False):
        """Row-broadcast matmul. Default: one-hot selector over the
        precomputed B/C rows (4x fewer active MACs than the stride-0
        re-projection - this kernel runs power-throttled). direct=True
        re-projects from xact (used where waiting for bcr would stall)."""
        t = ps.tile([DI, T], FP, tag="bc", name=name)
        if direct:
            w = hb[:, _B_BC + col:_B_BC + col + 1].broadcast_to((DI, DI))
            rhs, np_ = xact, DI
        else:
            w = hb[0:2 * DS, _B_EYE + col:_B_EYE + col + 1].broadcast_to(
                (2 * DS, DI))
            rhs, np_ = bcr, 2 * DS
        for c in range(NCH):
            nc.tensor.matmul(t[:, c * 512:(c + 1) * 512], w,
                             rhs[0:np_, c * 512:(c + 1) * 512] if not direct
                             else rhs[:, c * 512:(c + 1) * 512],
                             start=True, stop=True)
        return t

    # ---- in_proj-x with folded causal conv -> silu -> xact (fp16) ----
    # xc[:, t] = sum_k (diag(conv_w_k) @ in_w_x) @ u[:, t-3+k]; tap k reads
    # u_pad[:, c*512+k:]; odd k uses the 1-shifted copy so every rhs offset
    # stays 4B-aligned.  Silu is applied per half so the delta chain starts
    # as soon as the first half lands.
    xact = big.tile([DI, T], FH, tag="xact", name=f"xact{lid}")
    stacked = gl.get("stacked", False)
    w02T = hb[:, _B_W02:_B_W02 + DI]
    w13T = hb[:, _B_W13:_B_W13 + DI]
    for h in range(2):
        mmx = ps.tile([DI, H], FP, tag="bc", name=f"mmx{lid}_{h}")
        for c in (2 * h, 2 * h + 1):
            o = c * 512
            cs = slice(o - h * H, o - h * H + 512)
            if stacked:
                # up/upo hold [u_pad; u_pad<<2] on 128 partitions: 2 taps/mm
                nc.tensor.matmul(mmx[:, cs], w02T, up[:, o:o + 512],
                                 start=True, stop=False)
                nc.tensor.matmul(mmx[:, cs], w13T, upo[:, o:o + 512],
                                 start=False, stop=True)
            else:
                nc.tensor.matmul(mmx[:, cs], wkT[0], up[:, o:o + 512],
                                 start=True, stop=False)
                nc.tensor.matmul(mmx[:, cs], wkT[1], upo[:, o:o + 512],
                                 start=False, stop=False)
                nc.tensor.matmul(mmx[:, cs], wkT[2], up[:, o + 2:o + 514],
                                 start=False, stop=False)
                nc.tensor.matmul(mmx[:, cs], wkT[3], upo[:, o + 2:o + 514],
                                 start=False, stop=True)
        nc.scalar.activation(xact[:, h * H:(h + 1) * H], mmx[:], AF.Silu,
                             bias=convb)

    # ---- delta = softplus(dt_proj + dt_b) via Exp then Ln(1+x), halves ----
    delta = big.tile([DI, T], FH, tag="delta", name=f"delta{lid}")
    ev = big.tile([DI, T], FH, tag="ev", name=f"ev{lid}")
    dx = big.tile([DI, T], FH, tag="dx", name=f"dx{lid}")
    for h in range(2):
        mmd = ps.tile([DI, H], FP, tag="bc", name=f"mmd{lid}_{h}")
        for c in (2 * h, 2 * h + 1):
            o = c * 512
            nc.tensor.matmul(mmd[:, o - h * H:o - h * H + 512], wdT,
                             xact[:, o:o + 512], start=True, stop=True)
        nc.scalar.activation(ev[:, h * H:(h + 1) * H], mmd[:], AF.Exp,
                             bias=dtb)
    for h in range(2):
        hs_ = slice(h * H, (h + 1) * H)
        nc.scalar.activation(delta[:, hs_], ev[:, hs_], AF.Ln, bias=1.0)
        nc.vector.tensor_mul(dx[:, hs_], delta[:, hs_], xact[:, hs_])

    # ---- s-loop: single s=0 first (via stride-0 direct broadcasts and a
    # half-chained scan, so the first scan starts during the Act ramp),
    # then 7 pairs off precomputed B/C rows, then single s=15 ----
    ysn = big.tile([DI, T], FH, tag="ysn", name=f"ysn{lid}")
    yP = big.tile([DI, 2 * T], FH, tag="yP", name=f"yP{lid}")

    dA0 = big.tile([DI, T], FH, tag="dAs", name=f"dA{lid}_s0")
    dBu0 = big.tile([DI, T], FH, tag="dBus", name=f"dBu{lid}_s0")
    hs0 = big.tile([DI, T], FH, tag="hss", name=f"hs{lid}_s0")
    bps0 = ps.tile([DI, T], FP, tag="bc", name=f"bps{lid}_0")
    bw = hb[:, _B_BC:_B_BC + 1].broadcast_to((DI, DI))
    for h in range(2):
        hh = slice(h * H, (h + 1) * H)
        nc.scalar.activation(dA0[:, hh], delta[:, hh], AF.Exp,
                             scale=Acols[:, 0:1])
        for c in (2 * h, 2 * h + 1):
            nc.tensor.matmul(bps0[:, c * 512:(c + 1) * 512], bw,
                             xact[:, c * 512:(c + 1) * 512],
                             start=True, stop=True)
        nc.vector.tensor_mul(dBu0[:, hh], dx[:, hh], bps0[:, hh])
        nc.vector.tensor_tensor_scan(
            hs0[:, hh], dA0[:, hh], dBu0[:, hh],
            0.0 if h == 0 else hs0[:, H - 1:H], AX.mult, AX.add)
    cw = hb[:, _B_BC + DS:_B_BC + DS + 1].broadcast_to((DI, DI))
    cps0 = ps.tile([DI, T], FP, tag="bc", name=f"cps{lid}_0")
    for c in range(NCH):
        nc.tensor.matmul(cps0[:, c * 512:(c + 1) * 512], cw,
                         xact[:, c * 512:(c + 1) * 512], start=True, stop=True)
    nc.vector.tensor_mul(ysn[:], hs0[:], cps0[:])


    # pairs (1,2) .. (13,14); pair 1 uses the stride-0 direct broadcast so
    # it doesn't wait for the bcr rows (emitted after it, below)
    def pair(p):
        s0, s1 = 2 * p - 1, 2 * p
        bcrep = sl.tile([DI, 2 * T], FH, tag="bcrep", name=f"brep{lid}_{p}")
        dBu = sl.tile([DI, 2 * T], FH, tag="dBu", name=f"dBu{lid}_{p}")
        for i, s in ((0, s0), (1, s1)):
            bps = bc_mm("bc", s, f"bps{lid}_{s}", direct=(p == 1))
            nc.scalar.activation(bcrep[:, i * T:(i + 1) * T], bps[:], AF.Copy)
            nc.vector.tensor_mul(dBu[:, i * T:(i + 1) * T], dx[:],
                                 bcrep[:, i * T:(i + 1) * T])
        dA = sl.tile([DI, 2 * T], FH, tag="dA", name=f"dA{lid}_{p}")
        nc.scalar.activation(dA[:, 0:T], delta[:], AF.Exp,
                             scale=Acols[:, s0:s0 + 1])
        nc.scalar.activation(dA[:, T:2 * T], delta[:], AF.Exp,
                             scale=Acols[:, s1:s1 + 1])
        # zero the boundary column so the scan carry resets between states
        nc.scalar.activation(dA[:, T:T + 1], gl["zcol"][:], AF.Copy)
        hs = sl.tile([DI, 2 * T], FH, tag="hs", name=f"hs{lid}_{p}")
        nc.vector.tensor_tensor_scan(hs[:], dA[:], dBu[:], 0.0,
                                     AX.mult, AX.add)
        ccrep = sl.tile([DI, 2 * T], FH, tag="ccrep", name=f"crep{lid}_{p}")
        for i, s in ((0, s0), (1, s1)):
            cps = bc_mm("bc", DS + s, f"cps{lid}_{s}", direct=(p == 1))
            nc.scalar.activation(ccrep[:, i * T:(i + 1) * T], cps[:], AF.Copy)
        if p == 1:
            nc.vector.tensor_mul(yP[:], hs[:], ccrep[:])
        else:
            hsc = sl.tile([DI, 2 * T], FH, tag="hsc", name=f"hsc{lid}_{p}")
            nc.vector.tensor_mul(hsc[:], hs[:], ccrep[:])
            nc.vector.tensor_add(yP[:], yP[:], hsc[:])

    pair(1)

    # ---- B/C projection rows (32, T) for pairs 2..7 ----
    bcr = big.tile([2 * DS, T], FH, tag="bcr", name=f"bcr{lid}")
    mmb = ps.tile([DI, T], FP, tag="bc", name=f"mmb{lid}")
    for c in range(NCH):
        o = c * 512
        nc.tensor.matmul(mmb[0:2 * DS, o:o + 512], hb[:, _B_BC:_B_BC + 2 * DS],
                         xact[:, o:o + 512], start=True, stop=True)
    nc.scalar.activation(bcr[:], mmb[0:2 * DS, :], AF.Copy)
    for p in range(2, 8):
        pair(p)

    # s = 15
    dA15 = big.tile([DI, T], FH, tag="dAs2", name=f"dA{lid}_s15")
    nc.scalar.activation(dA15[:], delta[:], AF.Exp, scale=Acols[:, 15:16])
    bps15 = bc_mm("bc", 15, f"bps{lid}_15")
    brep15 = big.tile([DI, T], FH, tag="dBus2", name=f"brep{lid}_15")
    nc.scalar.activation(brep15[:], bps15[:], AF.Copy)
    dBu15 = big.tile([DI, T], FH, tag="dBuf", name=f"dBu{lid}_15")
    nc.vector.tensor_mul(dBu15[:], dx[:], brep15[:])
    hs15 = big.tile([DI, T], FH, tag="hss2", name=f"hs{lid}_s15")
    nc.vector.tensor_tensor_scan(hs15[:], dA15[:], dBu15[:], 0.0,
                                 AX.mult, AX.add)
    cps15 = bc_mm("bc", DS + 15, f"cps{lid}_15")
    crep15 = big.tile([DI, T], FH, tag="creps", name=f"crep{lid}_15")
    nc.scalar.activation(crep15[:], cps15[:], AF.Copy)
    hsc15 = big.tile([DI, T], FH, tag="hscs", name=f"hsc{lid}_15")
    nc.vector.tensor_mul(hsc15[:], hs15[:], crep15[:])
    nc.vector.tensor_add(ysn[:], ysn[:], hsc15[:])

    # ---- z-proj late (keeps the Act head short; silu set reloads once) ----
    zs = big.tile([DI, T], FH, tag="zs", name=f"zs{lid}")
    mmz = ps.tile([DI, T], FP, tag="bc", name=f"mmz{lid}")
    for c in range(NCH):
        o = c * 512
        nc.tensor.matmul(mmz[:, o:o + 512], zT, upo[0:DM, o + 2:o + 514],
                         start=True, stop=True)
    nc.scalar.activation(zs[:], mmz[:], AF.Silu)
    xsz = big.tile([DI, T], FH, tag="xsz", name=f"xsz{lid}")
    nc.vector.tensor_mul(xsz[:], xact[:], zs[:])

    # ---- y = (sum_s hs*C)*silu(z); out = out_w@y + (out_w*D)@(x*silu(z)) ----
    yf = big.tile([DI, T], FH, tag="yf", name=f"yf{lid}")
    for q in range(4):
        qq = slice(q * 512, (q + 1) * 512)
        qT = slice(T + q * 512, T + (q + 1) * 512)
        nc.vector.tensor_add(yf[:, qq], yP[:, qq], yP[:, qT])
        nc.vector.tensor_add(yf[:, qq], yf[:, qq], ysn[:, qq])
        nc.vector.tensor_mul(yf[:, qq], yf[:, qq], zs[:, qq])

    mmo = ps.tile([DI, T], FP, tag="bc", name=f"mmo{lid}")
    for c in range(NCH):
        o = c * 512
        nc.tensor.matmul(mmo[0:DM, o:o + 512], outDT, xsz[:, o:o + 512],
                         start=True, stop=False)
    for c in range(NCH):
        o = c * 512
        nc.tensor.matmul(mmo[0:DM, o:o + 512], outT, yf[:, o:o + 512],
                         start=False, stop=True)
    # chunked output copies: chunk q feeds the next layer's head ops / DMA
    NQ = 2 if len(out_specs) > 1 else 4
    Q = T // NQ
    for q in range(NQ):
        src = mmo[0:DM, q * Q:(q + 1) * Q]
        for j, (tl, off) in enumerate(out_specs):
            dst = tl[:, off + q * Q:off + (q + 1) * Q]
            if j == 0:
                nc.scalar.activation(dst, src, AF.Copy)
            else:
                nc.vector.tensor_copy(dst, src)
        if out_dma is not None:
            nc.sync.dma_start(out_dma[:, q * Q:(q + 1) * Q],
                              out_specs[0][0][:, out_specs[0][1] + q * Q:
                                              out_specs[0][1] + (q + 1) * Q])


def _build_kernel(ctx, tc, u0, u0o, hblobs, fblobs, outs):
    nc = tc.nc
    const = ctx.enter_context(tc.tile_pool(name="const", bufs=1))
    big = ctx.enter_context(tc.tile_pool(name="big", bufs=1))
    sl = ctx.enter_context(tc.tile_pool(name="sl", bufs=2))
    ps = ctx.enter_context(tc.tile_pool(name="ps", bufs=2, space="PSUM"))

    hb = [const.tile([DI, _HBLOB_W], FH, tag=f"hb{l}", name=f"hb{l}")
          for l in range(NL)]
    fb = [const.tile([DI, 20], FP, tag=f"fb{l}", name=f"fb{l}")
          for l in range(NL)]
    upA = const.tile([DI, UF], FH, tag="upA", name="upA")
    upAo = const.tile([DI, UF], FH, tag="upAo", name="upAo")
    nc.sync.dma_start(hb[0][:], hblobs[0][:])
    HF = UF // 2
    nc.sync.dma_start(upA[:, 0:HF], u0[:, 0:HF])
    nc.sync.dma_start(upAo[:, 0:HF], u0o[:, 0:HF])
    nc.sync.dma_start(upA[:, HF:UF], u0[:, HF:UF])
    nc.sync.dma_start(upAo[:, HF:UF], u0o[:, HF:UF])
    nc.sync.dma_start(fb[0][:], fblobs[0][:])
    nc.sync.dma_start(hb[1][:], hblobs[1][:])
    nc.sync.dma_start(fb[1][:], fblobs[1][:])
    upB = const.tile([DM, UF], FH, tag="upB", name="upB")
    upBo = const.tile([DM, UF], FH, tag="upBo", name="upBo")
    nc.gpsimd.memset(upB[:, 0:K - 1], 0.0)
    nc.gpsimd.memset(upB[:, UF - 1:UF], 0.0)
    nc.gpsimd.memset(upBo[:, 0:K - 2], 0.0)
    nc.gpsimd.memset(upBo[:, UF - 2:UF], 0.0)
    o2 = const.tile([DM, T], FH, tag="o2", name="o2")

    # short PE warm-up on zero weights/data (near-zero switching power)
    # so the first in_proj matmuls run at the 8/8 HAM clock.
    wz = const.tile([DI, 512], FH, tag="wz", name="wz")
    nc.gpsimd.memset(wz[:], 0.0)
    wps = ps.tile([DI, T // 2], FP, tag="bc", name="warm")
    for i in range(6):
        nc.tensor.matmul(wps[:, 0:512], wz[:, 0:DI], wz[:],
                         start=True, stop=True)


    zcol = const.tile([DI, 1], FH, tag="zcol", name="zcol")
    nc.gpsimd.memset(zcol[:], 0.0)
    tl_warm = const.tile([DI, 1], FH, tag="tlw", name="tlw")
    nc.scalar.activation(tl_warm[:], zcol[:], AF.Silu)

    pools = (const, big, sl, ps, {"lid": 0, "zcol": zcol, "stacked": True})
    # layer 1: outputs go to upB[:, 3:3+T] and upBo[:, 2:2+T]
    _build_layer(nc, pools, hb[0], fb[0], upA, upAo,
                 [(upB, K - 1), (upBo, K - 2)], outs[0])
    pools = (const, big, sl, ps, {"lid": 1, "zcol": zcol})
    _build_layer(nc, pools, hb[1], fb[1], upB, upBo, [(o2, 0)], outs[1])


def build_program():
    nc = bacc.Bacc("TRN2", target_bir_lowering=False, debug=False)
    u0 = nc.dram_tensor("u0", [DI, UF], FH, kind="ExternalInput").ap()
    u0o = nc.dram_tensor("u0o", [DI, UF], FH, kind="ExternalInput").ap()
    hblobs = [nc.dram_tensor(f"hblob{l}", [DI, _HBLOB_W], FH,
                             kind="ExternalInput").ap() for l in range(NL)]
    fblobs = [nc.dram_tensor(f"fblob{l}", [DI, 20], FP,
                             kind="ExternalInput").ap() for l in range(NL)]
    outs = [nc.dram_tensor(f"o{l + 1}T", [DM, T], FH,
                           kind="ExternalOutput").ap() for l in range(NL)]
    with tile.TileContext(nc) as tc:
        with ExitStack() as ctx:
            _build_kernel(ctx, tc, u0, u0o, hblobs, fblobs, outs)
    nc.compile()
    return nc


_PROG = None


def _get_prog():
    global _PROG
    if _PROG is None:
        _PROG = build_program()
    return _PROG


def _pad_u(u):
    """u: (64, T) f32 -> stacked (u2, u2o) fp16 (128, UF).

    Rows 0:64 hold u_pad (left pad K-1) / its 1-shift; rows 64:128 hold the
    same shifted 2 further, so layer 1's conv taps (0,2) and (1,3) each fold
    into one contraction-128 matmul."""
    up = np.zeros((DM, UF), np.float16)
    up[:, K - 1:K - 1 + T] = u.astype(np.float16)
    u2 = np.zeros((DI, UF), np.float16)
    u2o = np.zeros((DI, UF), np.float16)
    u2[0:DM] = up
    u2[DM:, 0:UF - 2] = up[:, 2:UF]
    u2o[0:DM, 0:UF - 1] = up[:, 1:UF]
    u2o[DM:, 0:UF - 3] = up[:, 3:UF]
    return u2, u2o


def _run_launch(u_list_T, raw, trace=False, trace_kwargs=None):
    """u_list_T: list of 8 arrays (64, 2048) f32. raw: param dict (np).
    Returns (o1_list, o2_list, res) with (64, 2048) fp16 outputs."""
    nc = _get_prog()
    blobs = [_pack_blobs(raw, l) for l in range(NL)]
    in_maps = []
    for b in range(8):
        up, upo = _pad_u(np.asarray(u_list_T[b], np.float32))
        in_maps.append({
            "u0": up, "u0o": upo,
            "hblob0": blobs[0][0], "fblob0": blobs[0][1],
            "hblob1": blobs[1][0], "fblob1": blobs[1][1],
        })
    res = bass_utils.run_bass_kernel_spmd(
        nc, in_maps, core_ids=list(range(8)), trace=trace,
        **(trace_kwargs or {}))
    o1 = [res.results[b]["o1T"] for b in range(8)]
    o2 = [res.results[b]["o2T"] for b in range(8)]
    return o1, o2, res


def kernel(**inputs):
    inp = {k: np.asarray(v, np.float32) for k, v in inputs.items()}
    Ms = inp["Ms_feature"]
    Pan = inp["Pan_feature"]
    h = C // 2
    names = ("in_w", "conv_w", "conv_b", "xp_w", "dt_w", "dt_b",
             "A_log", "D", "out_w")
    rawa = {n: inp["a_" + n] for n in names}
    rawb = {n: inp["b_" + n] for n in names}

    cf1 = np.concatenate([Ms[:, :h], Pan[:, h:]], axis=1)
    cf2 = np.concatenate([Pan[:, :h], Ms[:, h:]], axis=1)
    u_list = [cf1[b].T for b in range(B)] + [cf2[b].T for b in range(B)]
    o1, o2, _ = _run_launch(u_list, rawa)
    cf1_1 = np.stack([o1[b].T.astype(np.float32) for b in range(B)])
    cf2_1 = np.stack([o1[B + b].T.astype(np.float32) for b in range(B)])
    cf1_2 = np.stack([o2[b].T.astype(np.float32) for b in range(B)])
    cf2_2 = np.stack([o2[B + b].T.astype(np.float32) for b in range(B)])
    Ms1 = np.maximum((cf1_1 + cf2_1) * 0.5 + Ms, 0.0)
    Ms2 = np.maximum((cf1_2 + cf2_2) * 0.5 + Ms1, 0.0)

    cf3 = np.stack([Pan[:, ::2], Ms2[:, 1::2]], axis=2).reshape(B, C, DM)
    cf4 = np.stack([Ms2[:, ::2], Pan[:, 1::2]], axis=2).reshape(B, C, DM)
    u_list = [cf3[b].T for b in range(B)] + [cf4[b].T for b in range(B)]
    o1, o2, _ = _run_launch(u_list, rawb)
    cf3_1 = np.stack([o1[b].T.astype(np.float32) for b in range(B)])
    cf4_1 = np.stack([o1[B + b].T.astype(np.float32) for b in range(B)])
    cf3_2 = np.stack([o2[b].T.astype(np.float32) for b in range(B)])
    cf4_2 = np.stack([o2[B + b].T.astype(np.float32) for b in range(B)])
    Pan1 = np.maximum((cf3_1 + cf4_1) * 0.5 + Pan, 0.0)
    Pan2 = np.maximum((cf3_2 + cf4_2) * 0.5 + Pan1, 0.0)
    return Ms2, Pan2


# revision 29
# speedup vs baseline: 1.2232x; 1.0019x over previous
"""Trainium2 Bass kernel for nn_CMCI_Mamba.

Strategy: data-parallel over the 2B=8 mamba streams (1 sequence per core).
Each launch runs 2 chained mamba layers fully on-chip in d-major layout
(features on partitions, time on the free axis).

Engine assignment (per layer):
- PE (fp16): in_proj with the causal conv FOLDED IN (4 shifted matmuls with
  host-prescaled weights diag(conv_w_k) @ in_w), z-proj, fused
  dt_w@xp_w[dt] projection, 32 stride-0 B/C broadcast matmuls, out_proj.
- Act: Silu(conv) / Silu(z) straight from PSUM, softplus via Exp+Ln (one
  table set), the 16 per-state dA = exp(A_s * delta) passes, PSUM->SBUF
  fp16 copies of the B/C broadcasts, layer-output copies.
- DVE: the 16 SSM scans (tensor_tensor_scan, batched 2 states per
  instruction with a zeroed dA column resetting the carry), all dBu and
  hs*C muls and the y accumulation.  GPSIMD is deliberately idle: its
  tensor ops starve the DVE's shared SBUF port (measured 7x slowdown on
  concurrent DVE tensor_tensor).

Host does the cheap cross-stream elementwise combines between launches.
"""
import sys
import numpy as np
from contextlib import ExitStack

for _p in ("/opt/trn_rl_repo",):
    if _p not in sys.path:
        sys.path.insert(0, _p)

import concourse.bass as bass
import concourse.bacc as bacc
import concourse.tile as tile
from concourse import mybir
from concourse import bass_utils

T, DM, DI, DS, DR, K, NL = 2048, 64, 128, 16, 4, 4, 2
B, C = 4, 2048
UF = T + K  # padded u width (2052)
FP = mybir.dt.float32
FH = mybir.dt.float16
AX = mybir.AluOpType
AF = mybir.ActivationFunctionType

# fp16 param blob column layout, (128, 1024) per layer
_B_WK = 0       # [0:64, 0:512]    4x conv-scaled in_proj-x lhsT (64,128) each
_B_Z = 512      # [0:64, 512:640]  z lhsT
_B_WD = 640     # [:, 640:768]     (dt_w @ xp_w[:DR]) lhsT
_B_BC = 768     # [:, 768:800]     B/C projection columns (32)
_B_OUT = 800    # [:, 800:864]     out_proj lhsT
_B_OUTD = 864   # [:, 864:928]     out_proj lhsT with D folded (for x*sz term)
_B_EYE = 928    # [0:32, 928:960]  eye(32) one-hot selectors for row broadcast
_B_W02 = 960    # [:, 960:1088]    taps 0+2 stacked lhsT (contraction 128)
_B_W13 = 1088   # [:, 1088:1216]   taps 1+3 stacked lhsT
_HBLOB_W = 1280
# fp32 blob (128, 20): [:, 0:16]=A (=-exp(A_log)), 16=conv_b, 17=dt_b, 18=D


def _pack_blobs(raw, l):
    hb = np.zeros((DI, _HBLOB_W), np.float16)
    in_w = raw["in_w"][l]          # (256, 64)
    conv_w = raw["conv_w"][l]      # (128, 4)
    for k in range(K):
        wk = in_w[:DI] * conv_w[:, k:k + 1]          # (128, 64)
        hb[:DM, _B_WK + 128 * k:_B_WK + 128 * (k + 1)] = wk.T
    hb[:DM, _B_Z:_B_Z + DI] = in_w[DI:2 * DI].T
    wd = raw["dt_w"][l] @ raw["xp_w"][l][:DR]        # (128, 128)
    hb[:, _B_WD:_B_WD + DI] = wd.T
    hb[:, _B_BC:_B_BC + 2 * DS] = raw["xp_w"][l][DR:DR + 2 * DS].T
    hb[:, _B_OUT:_B_OUT + DM] = raw["out_w"][l].T
    # out_proj with D folded in: out += (out_w * D) @ (x * silu(z))
    hb[:, _B_OUTD:_B_OUTD + DM] = (raw["out_w"][l] * raw["D"][l]).T
    hb[0:2 * DS, _B_EYE:_B_EYE + 2 * DS] = np.eye(2 * DS, dtype=np.float16)
    # layer-1 ramp path: taps (0,2) and (1,3) stacked into c=128 matmuls
    for j, (ka, kb) in enumerate(((0, 2), (1, 3))):
        col = (_B_W02, _B_W13)[j]
        hb[0:DM, col:col + DI] = (in_w[:DI] * conv_w[:, ka:ka + 1]).T
        hb[DM:2 * DM, col:col + DI] = (in_w[:DI] * conv_w[:, kb:kb + 1]).T
    fb = np.zeros((DI, 20), np.float32)
    fb[:, 0:DS] = -np.exp(raw["A_log"][l])
    fb[:, 16] = raw["conv_b"][l]
    fb[:, 17] = raw["dt_b"][l]
    fb[:, 18] = raw["D"][l]
    return hb, fb


def _build_layer(nc, pools, hb, fb, up, upo, out_specs, out_dma):
    """One mamba layer. up/upo: (64, UF) fp16 padded input (+1-shifted copy).
    out_specs: list of (tile, col_off) -- the (64, T) layer output is copied
    (in halves, on Act) into tile[:, off:off+T]. out_dma: DRAM ap or None.
    """
    const, big, sl, ps, gl = pools
    NCH = T // 512
    H = T // 2
    lid = gl["lid"]

    wkT = [hb[0:DM, _B_WK + 128 * k:_B_WK + 128 * (k + 1)] for k in range(K)]
    zT = hb[0:DM, _B_Z:_B_Z + DI]
    wdT = hb[:, _B_WD:_B_WD + DI]
    outT = hb[:, _B_OUT:_B_OUT + DM]
    outDT = hb[:, _B_OUTD:_B_OUTD + DM]
    Acols = fb[:, 0:DS]
    convb = fb[:, 16:17]
    dtb = fb[:, 17:18]

    def bc_mm(tag, col, name, direct=False):
        """Row-broadcast matmul. Default: one-hot selector over the
        precomputed B/C rows (4x fewer active MACs than the stride-0
        re-projection - this kernel runs power-throttled). direct=True
        re-projects from xact (used where waiting for bcr would stall)."""
        t = ps.tile([DI, T], FP, tag="bc", name=name)
        if direct:
            w = hb[:, _B_BC + col:_B_BC + col + 1].broadcast_to((DI, DI))
            rhs, np_ = xact, DI
        else:
            w = hb[0:2 * DS, _B_EYE + col:_B_EYE + col + 1].broadcast_to(
                (2 * DS, DI))
            rhs, np_ = bcr, 2 * DS
        for c in range(NCH):
            nc.tensor.matmul(t[:, c * 512:(c + 1) * 512], w,
                             rhs[0:np_, c * 512:(c + 1) * 512] if not direct
                             else rhs[:, c * 512:(c + 1) * 512],
                             start=True, stop=True)
        return t

    # ---- in_proj-x with folded causal conv -> silu -> xact (fp16) ----
    # xc[:, t] = sum_k (diag(conv_w_k) @ in_w_x) @ u[:, t-3+k]; tap k reads
    # u_pad[:, c*512+k:]; odd k uses the 1-shifted copy so every rhs offset
    # stays 4B-aligned.  Silu is applied per half so the delta chain starts
    # as soon as the first half lands.
    xact = big.tile([DI, T], FH, tag="xact", name=f"xact{lid}")
    stacked = gl.get("stacked", False)
    w02T = hb[:, _B_W02:_B_W02 + DI]
    w13T = hb[:, _B_W13:_B_W13 + DI]
    for h in range(2):
        mmx = ps.tile([DI, H], FP, tag="bc", name=f"mmx{lid}_{h}")
        for c in (2 * h, 2 * h + 1):
            o = c * 512
            cs = slice(o - h * H, o - h * H + 512)
            if stacked:
                # up/upo hold [u_pad; u_pad<<2] on 128 partitions: 2 taps/mm
                nc.tensor.matmul(mmx[:, cs], w02T, up[:, o:o + 512],
                                 start=True, stop=False)
                nc.tensor.matmul(mmx[:, cs], w13T, upo[:, o:o + 512],
                                 start=False, stop=True)
            else:
                nc.tensor.matmul(mmx[:, cs], wkT[0], up[:, o:o + 512],
                                 start=True, stop=False)
                nc.tensor.matmul(mmx[:, cs], wkT[1], upo[:, o:o + 512],
                                 start=False, stop=False)
                nc.tensor.matmul(mmx[:, cs], wkT[2], up[:, o + 2:o + 514],
                                 start=False, stop=False)
                nc.tensor.matmul(mmx[:, cs], wkT[3], upo[:, o + 2:o + 514],
                                 start=False, stop=True)
        nc.scalar.activation(xact[:, h * H:(h + 1) * H], mmx[:], AF.Silu,
                             bias=convb)

    # ---- delta = softplus(dt_proj + dt_b) via Exp then Ln(1+x), halves ----
    delta = big.tile([DI, T], FH, tag="delta", name=f"delta{lid}")
    ev = big.tile([DI, T], FH, tag="ev", name=f"ev{lid}")
    dx = big.tile([DI, T], FH, tag="dx", name=f"dx{lid}")
    for h in range(2):
        mmd = ps.tile([DI, H], FP, tag="bc", name=f"mmd{lid}_{h}")
        for c in (2 * h, 2 * h + 1):
            o = c * 512
            nc.tensor.matmul(mmd[:, o - h * H:o - h * H + 512], wdT,
                             xact[:, o:o + 512], start=True, stop=True)
        nc.scalar.activation(ev[:, h * H:(h + 1) * H], mmd[:], AF.Exp,
                             bias=dtb)
    for h in range(2):
        hs_ = slice(h * H, (h + 1) * H)
        nc.scalar.activation(delta[:, hs_], ev[:, hs_], AF.Ln, bias=1.0)
        nc.vector.tensor_mul(dx[:, hs_], delta[:, hs_], xact[:, hs_])

    # ---- s-loop: single s=0 first (via stride-0 direct broadcasts and a
    # half-chained scan, so the first scan starts during the Act ramp),
    # then 7 pairs off precomputed B/C rows, then single s=15 ----
    ysn = big.tile([DI, T], FH, tag="ysn", name=f"ysn{lid}")
    yP = big.tile([DI, 2 * T], FH, tag="yP", name=f"yP{lid}")

    dA0 = big.tile([DI, T], FH, tag="dAs", name=f"dA{lid}_s0")
    dBu0 = big.tile([DI, T], FH, tag="dBus", name=f"dBu{lid}_s0")
    hs0 = big.tile([DI, T], FH, tag="hss", name=f"hs{lid}_s0")
    bps0 = ps.tile([DI, T], FP, tag="bc", name=f"bps{lid}_0")
    bw = hb[:, _B_BC:_B_BC + 1].broadcast_to((DI, DI))
    for h in range(2):
        hh = slice(h * H, (h + 1) * H)
        nc.scalar.activation(dA0[:, hh], delta[:, hh], AF.Exp,
                             scale=Acols[:, 0:1])
        for c in (2 * h, 2 * h + 1):
            nc.tensor.matmul(bps0[:, c * 512:(c + 1) * 512], bw,
                             xact[:, c * 512:(c + 1) * 512],
                             start=True, stop=True)
        nc.vector.tensor_mul(dBu0[:, hh], dx[:, hh], bps0[:, hh])
        nc.vector.tensor_tensor_scan(
            hs0[:, hh], dA0[:, hh], dBu0[:, hh],
            0.0 if h == 0 else hs0[:, H - 1:H], AX.mult, AX.add)
    cw = hb[:, _B_BC + DS:_B_BC + DS + 1].broadcast_to((DI, DI))
    cps0 = ps.tile([DI, T], FP, tag="bc", name=f"cps{lid}_0")
    for c in range(NCH):
        nc.tensor.matmul(cps0[:, c * 512:(c + 1) * 512], cw,
                         xact[:, c * 512:(c + 1) * 512], start=True, stop=True)
    nc.vector.tensor_mul(ysn[:], hs0[:], cps0[:])


    # pairs (1,2) .. (13,14); pair 1 uses the stride-0 direct broadcast so
    # it doesn't wait for the bcr rows (emitted after it, below)
    def pair(p):
        s0, s1 = 2 * p - 1, 2 * p
        bcrep = sl.tile([DI, 2 * T], FH, tag="bcrep", name=f"brep{lid}_{p}")
        dBu = sl.tile([DI, 2 * T], FH, tag="dBu", name=f"dBu{lid}_{p}")
        for i, s in ((0, s0), (1, s1)):
            bps = bc_mm("bc", s, f"bps{lid}_{s}", direct=(p == 1))
            nc.scalar.activation(bcrep[:, i * T:(i + 1) * T], bps[:], AF.Copy)
            nc.vector.tensor_mul(dBu[:, i * T:(i + 1) * T], dx[:],
                                 bcrep[:, i * T:(i + 1) * T])
        dA = sl.tile([DI, 2 * T], FH, tag="dA", name=f"dA{lid}_{p}")
        nc.scalar.activation(dA[:, 0:T], delta[:], AF.Exp,
                             scale=Acols[:, s0:s0 + 1])
        nc.scalar.activation(dA[:, T:2 * T], delta[:], AF.Exp,
                             scale=Acols[:, s1:s1 + 1])
        # zero the boundary column so the scan carry resets between states
        nc.scalar.activation(dA[:, T:T + 1], gl["zcol"][:], AF.Copy)
        hs = sl.tile([DI, 2 * T], FH, tag="hs", name=f"hs{lid}_{p}")
        nc.vector.tensor_tensor_scan(hs[:], dA[:], dBu[:], 0.0,
                                     AX.mult, AX.add)
        ccrep = sl.tile([DI, 2 * T], FH, tag="ccrep", name=f"crep{lid}_{p}")
        for i, s in ((0, s0), (1, s1)):
            cps = bc_mm("bc", DS + s, f"cps{lid}_{s}", direct=(p == 1))
            nc.scalar.activation(ccrep[:, i * T:(i + 1) * T], cps[:], AF.Copy)
        if p == 1:
            nc.vector.tensor_mul(yP[:], hs[:], ccrep[:])
        else:
            hsc = sl.tile([DI, 2 * T], FH, tag="hsc", name=f"hsc{lid}_{p}")
            nc.vector.tensor_mul(hsc[:], hs[:], ccrep[:])
            nc.vector.tensor_add(yP[:], yP[:], hsc[:])

    pair(1)

    # ---- B/C projection rows (32, T) for pairs 2..7 ----
    bcr = big.tile([2 * DS, T], FH, tag="bcr", name=f"bcr{lid}")
    mmb = ps.tile([DI, T], FP, tag="bc", name=f"mmb{lid}")
    for c in range(NCH):
        o = c * 512
        nc.tensor.matmul(mmb[0:2 * DS, o:o + 512], hb[:, _B_BC:_B_BC + 2 * DS],
                         xact[:, o:o + 512], start=True, stop=True)
    nc.scalar.activation(bcr[:], mmb[0:2 * DS, :], AF.Copy)
    for p in range(2, 8):
        pair(p)

    # s = 15
    dA15 = big.tile([DI, T], FH, tag="dAs2", name=f"dA{lid}_s15")
    nc.scalar.activation(dA15[:], delta[:], AF.Exp, scale=Acols[:, 15:16])
    bps15 = bc_mm("bc", 15, f"bps{lid}_15")
    brep15 = big.tile([DI, T], FH, tag="dBus2", name=f"brep{lid}_15")
    nc.scalar.activation(brep15[:], bps15[:], AF.Copy)
    dBu15 = big.tile([DI, T], FH, tag="dBuf", name=f"dBu{lid}_15")
    nc.vector.tensor_mul(dBu15[:], dx[:], brep15[:])
    hs15 = big.tile([DI, T], FH, tag="hss2", name=f"hs{lid}_s15")
    nc.vector.tensor_tensor_scan(hs15[:], dA15[:], dBu15[:], 0.0,
                                 AX.mult, AX.add)
    cps15 = bc_mm("bc", DS + 15, f"cps{lid}_15")
    crep15 = big.tile([DI, T], FH, tag="creps", name=f"crep{lid}_15")
    nc.scalar.activation(crep15[:], cps15[:], AF.Copy)
    hsc15 = big.tile([DI, T], FH, tag="hscs", name=f"hsc{lid}_15")
    nc.vector.tensor_mul(hsc15[:], hs15[:], crep15[:])
    nc.vector.tensor_add(ysn[:], ysn[:], hsc15[:])

    # ---- z-proj late (keeps the Act head short; silu set reloads once) ----
    zs = big.tile([DI, T], FH, tag="zs", name=f"zs{lid}")
    mmz = ps.tile([DI, T], FP, tag="bc", name=f"mmz{lid}")
    for c in range(NCH):
        o = c * 512
        nc.tensor.matmul(mmz[:, o:o + 512], zT, upo[0:DM, o + 2:o + 514],
                         start=True, stop=True)
    nc.scalar.activation(zs[:], mmz[:], AF.Silu)
    xsz = big.tile([DI, T], FH, tag="xsz", name=f"xsz{lid}")
    nc.vector.tensor_mul(xsz[:], xact[:], zs[:])

    # ---- y = (sum_s hs*C)*silu(z); out = out_w@y + (out_w*D)@(x*silu(z)) ----
    yf = big.tile([DI, T], FH, tag="yf", name=f"yf{lid}")
    for q in range(4):
        qq = slice(q * 512, (q + 1) * 512)
        qT = slice(T + q * 512, T + (q + 1) * 512)
        nc.vector.tensor_add(yf[:, qq], yP[:, qq], yP[:, qT])
        nc.vector.tensor_add(yf[:, qq], yf[:, qq], ysn[:, qq])
        nc.vector.tensor_mul(yf[:, qq], yf[:, qq], zs[:, qq])

    mmo = ps.tile([DI, T], FP, tag="bc", name=f"mmo{lid}")
    for c in range(NCH):
        o = c * 512
        nc.tensor.matmul(mmo[0:DM, o:o + 512], outDT, xsz[:, o:o + 512],
                         start=True, stop=False)
    for c in range(NCH):
        o = c * 512
        nc.tensor.matmul(mmo[0:DM, o:o + 512], outT, yf[:, o:o + 512],
                         start=False, stop=True)
    # chunked output copies: chunk q feeds the next layer's head ops / DMA.
    # Only Act reads the PSUM (concurrent ScalarE+VectorE reads of the same
    # PSUM bank are a fatal collision); the shifted secondary copy is made
    # by DVE from the SBUF primary.
    NQ = 2 if len(out_specs) > 1 else 4
    Q = T // NQ
    t0, off0 = out_specs[0]
    for q in range(NQ):
        src = mmo[0:DM, q * Q:(q + 1) * Q]
        nc.scalar.activation(t0[:, off0 + q * Q:off0 + (q + 1) * Q],
                             src, AF.Copy)
        for tl, off in out_specs[1:]:
            nc.vector.tensor_copy(tl[:, off + q * Q:off + (q + 1) * Q],
                                  t0[:, off0 + q * Q:off0 + (q + 1) * Q])
        if out_dma is not None:
            nc.sync.dma_start(out_dma[:, q * Q:(q + 1) * Q],
                              t0[:, off0 + q * Q:off0 + (q + 1) * Q])


def _build_kernel(ctx, tc, u0, u0o, hblobs, fblobs, outs):
    nc = tc.nc
    const = ctx.enter_context(tc.tile_pool(name="const", bufs=1))
    big = ctx.enter_context(tc.tile_pool(name="big", bufs=1))
    sl = ctx.enter_context(tc.tile_pool(name="sl", bufs=2))
    ps = ctx.enter_context(tc.tile_pool(name="ps", bufs=2, space="PSUM"))

    hb = [const.tile([DI, _HBLOB_W], FH, tag=f"hb{l}", name=f"hb{l}")
          for l in range(NL)]
    fb = [const.tile([DI, 20], FP, tag=f"fb{l}", name=f"fb{l}")
          for l in range(NL)]
    upA = const.tile([DI, UF], FH, tag="upA", name="upA")
    upAo = const.tile([DI, UF], FH, tag="upAo", name="upAo")
    nc.sync.dma_start(hb[0][:], hblobs[0][:])
    HF = UF // 2
    nc.sync.dma_start(upA[:, 0:HF], u0[:, 0:HF])
    nc.sync.dma_start(upAo[:, 0:HF], u0o[:, 0:HF])
    nc.sync.dma_start(upA[:, HF:UF], u0[:, HF:UF])
    nc.sync.dma_start(upAo[:, HF:UF], u0o[:, HF:UF])
    nc.sync.dma_start(fb[0][:], fblobs[0][:])
    nc.sync.dma_start(hb[1][:], hblobs[1][:])
    nc.sync.dma_start(fb[1][:], fblobs[1][:])
    upB = const.tile([DM, UF], FH, tag="upB", name="upB")
    upBo = const.tile([DM, UF], FH, tag="upBo", name="upBo")
    nc.gpsimd.memset(upB[:, 0:K - 1], 0.0)
    nc.gpsimd.memset(upB[:, UF - 1:UF], 0.0)
    nc.gpsimd.memset(upBo[:, 0:K - 2], 0.0)
    nc.gpsimd.memset(upBo[:, UF - 2:UF], 0.0)
    o2 = const.tile([DM, T], FH, tag="o2", name="o2")

    # short PE warm-up on zero weights/data (near-zero switching power)
    # so the first in_proj matmuls run at the 8/8 HAM clock.
    wz = const.tile([DI, 512], FH, tag="wz", name="wz")
    nc.gpsimd.memset(wz[:], 0.0)
    wps = ps.tile([DI, T // 2], FP, tag="bc", name="warm")
    for i in range(6):
        nc.tensor.matmul(wps[:, 0:512], wz[:, 0:DI], wz[:],
                         start=True, stop=True)


    zcol = const.tile([DI, 1], FH, tag="zcol", name="zcol")
    nc.gpsimd.memset(zcol[:], 0.0)
    tl_warm = const.tile([DI, 1], FH, tag="tlw", name="tlw")
    nc.scalar.activation(tl_warm[:], zcol[:], AF.Silu)

    pools = (const, big, sl, ps, {"lid": 0, "zcol": zcol, "stacked": True})
    # layer 1: outputs go to upB[:, 3:3+T] and upBo[:, 2:2+T]
    _build_layer(nc, pools, hb[0], fb[0], upA, upAo,
                 [(upB, K - 1), (upBo, K - 2)], outs[0])
    pools = (const, big, sl, ps, {"lid": 1, "zcol": zcol})
    _build_layer(nc, pools, hb[1], fb[1], upB, upBo, [(o2, 0)], outs[1])


def build_program():
    nc = bacc.Bacc("TRN2", target_bir_lowering=False, debug=False)
    u0 = nc.dram_tensor("u0", [DI, UF], FH, kind="ExternalInput").ap()
    u0o = nc.dram_tensor("u0o", [DI, UF], FH, kind="ExternalInput").ap()
    hblobs = [nc.dram_tensor(f"hblob{l}", [DI, _HBLOB_W], FH,
                             kind="ExternalInput").ap() for l in range(NL)]
    fblobs = [nc.dram_tensor(f"fblob{l}", [DI, 20], FP,
                             kind="ExternalInput").ap() for l in range(NL)]
    outs = [nc.dram_tensor(f"o{l + 1}T", [DM, T], FH,
                           kind="ExternalOutput").ap() for l in range(NL)]
    with tile.TileContext(nc) as tc:
        with ExitStack() as ctx:
            _build_kernel(ctx, tc, u0, u0o, hblobs, fblobs, outs)
    nc.compile()
    return nc


_PROG = None


def _get_prog():
    global _PROG
    if _PROG is None:
        _PROG = build_program()
    return _PROG


def _pad_u(u):
    """u: (64, T) f32 -> stacked (u2, u2o) fp16 (128, UF).

    Rows 0:64 hold u_pad (left pad K-1) / its 1-shift; rows 64:128 hold the
    same shifted 2 further, so layer 1's conv taps (0,2) and (1,3) each fold
    into one contraction-128 matmul."""
    up = np.zeros((DM, UF), np.float16)
    up[:, K - 1:K - 1 + T] = u.astype(np.float16)
    u2 = np.zeros((DI, UF), np.float16)
    u2o = np.zeros((DI, UF), np.float16)
    u2[0:DM] = up
    u2[DM:, 0:UF - 2] = up[:, 2:UF]
    u2o[0:DM, 0:UF - 1] = up[:, 1:UF]
    u2o[DM:, 0:UF - 3] = up[:, 3:UF]
    return u2, u2o


def _run_launch(u_list_T, raw, trace=False, trace_kwargs=None):
    """u_list_T: list of 8 arrays (64, 2048) f32. raw: param dict (np).
    Returns (o1_list, o2_list, res) with (64, 2048) fp16 outputs."""
    nc = _get_prog()
    blobs = [_pack_blobs(raw, l) for l in range(NL)]
    in_maps = []
    for b in range(8):
        up, upo = _pad_u(np.asarray(u_list_T[b], np.float32))
        in_maps.append({
            "u0": up, "u0o": upo,
            "hblob0": blobs[0][0], "fblob0": blobs[0][1],
            "hblob1": blobs[1][0], "fblob1": blobs[1][1],
        })
    res = bass_utils.run_bass_kernel_spmd(
        nc, in_maps, core_ids=list(range(8)), trace=trace,
        **(trace_kwargs or {}))
    o1 = [res.results[b]["o1T"] for b in range(8)]
    o2 = [res.results[b]["o2T"] for b in range(8)]
    return o1, o2, res


def kernel(**inputs):
    inp = {k: np.asarray(v, np.float32) for k, v in inputs.items()}
    Ms = inp["Ms_feature"]
    Pan = inp["Pan_feature"]
    h = C // 2
    names = ("in_w", "conv_w", "conv_b", "xp_w", "dt_w", "dt_b",
             "A_log", "D", "out_w")
    rawa = {n: inp["a_" + n] for n in names}
    rawb = {n: inp["b_" + n] for n in names}

    cf1 = np.concatenate([Ms[:, :h], Pan[:, h:]], axis=1)
    cf2 = np.concatenate([Pan[:, :h], Ms[:, h:]], axis=1)
    u_list = [cf1[b].T for b in range(B)] + [cf2[b].T for b in range(B)]
    o1, o2, _ = _run_launch(u_list, rawa)
    cf1_1 = np.stack([o1[b].T.astype(np.float32) for b in range(B)])
    cf2_1 = np.stack([o1[B + b].T.astype(np.float32) for b in range(B)])
    cf1_2 = np.stack([o2[b].T.astype(np.float32) for b in range(B)])
    cf2_2 = np.stack([o2[B + b].T.astype(np.float32) for b in range(B)])
    Ms1 = np.maximum((cf1_1 + cf2_1) * 0.5 + Ms, 0.0)
    Ms2 = np.maximum((cf1_2 + cf2_2) * 0.5 + Ms1, 0.0)

    cf3 = np.stack([Pan[:, ::2], Ms2[:, 1::2]], axis=2).reshape(B, C, DM)
    cf4 = np.stack([Ms2[:, ::2], Pan[:, 1::2]], axis=2).reshape(B, C, DM)
    u_list = [cf3[b].T for b in range(B)] + [cf4[b].T for b in range(B)]
    o1, o2, _ = _run_launch(u_list, rawb)
    cf3_1 = np.stack([o1[b].T.astype(np.float32) for b in range(B)])
    cf4_1 = np.stack([o1[B + b].T.astype(np.float32) for b in range(B)])
    cf3_2 = np.stack([o2[b].T.astype(np.float32) for b in range(B)])
    cf4_2 = np.stack([o2[B + b].T.astype(np.float32) for b in range(B)])
    Pan1 = np.maximum((cf3_1 + cf4_1) * 0.5 + Pan, 0.0)
    Pan2 = np.maximum((cf3_2 + cf4_2) * 0.5 + Pan1, 0.0)
    return Ms2, Pan2
